# revision 47
# baseline (speedup 1.0000x reference)
"""TRN2 Bass kernel for nn_Adapter (dense_cnn): ViT adapter with two branches
  main:   h1 = xs@w1+b1 ; y = dwconv3d_3x3x3(h1)+cb ; y@w2+b2
  offset: g = xs@ow1    ; d = tdiff(g) ; oc = dwconv_1x3x3(d)+bias' ; oc@ow2
  out = x with patch tokens += main + offset   (CLS rows pass through)

Data-parallel over 8 NeuronCores: 2 clips (16 frames) per core; adapter
weights replicated. Per-core kernel (raw bass, explicit semaphores):
  - fp8-e4m3 DoubleRow matmuls for m1 / depthwise conv (diagonal lhsT) / m2
  - 240-stride padded planes (15 rows x 16 cols); adjacent planes share the
    zero halo row, saving 1/16 of all PE conv streaming
  - m1 is j-major so PE starts on the first 512 columns of x^T; w1/diag are
    DMA'd in need-order pieces
  - evict work is spread across three engines: DVE evicts off-branch m1
    planes then computes wide per-(g,c) frame diffs; ACT evicts main m1
    planes + all conv outputs (two planes per op); m2 evict+residual pairs
    alternate DVE / GPSIMD
  - GPSIMD also zero-fills every halo/guard via memsets (no zeros DMA)
  - weights scaled x16/stage to keep fp8 out of subnormals; the final evict
    multiplies by 1/16^3 and adds the bf16 residual tokens
  - bf16 token stream + bf16 output (cast to f32 on host)

Self-contained: hardcodes shapes for x:[128,197,768], T=8 (asserts).
"""
import numpy as np
import ml_dtypes

import concourse.bass as bass
import concourse.mybir as mybir
from concourse.bass_utils import run_bass_kernel_spmd

F32 = mybir.dt.float32
BF16 = mybir.dt.bfloat16
F8 = mybir.dt.float8e4
AOT = mybir.AluOpType
AFT = mybir.ActivationFunctionType
DR = mybir.MatmulPerfMode.DoubleRow
BF = ml_dtypes.bfloat16
F8NP = ml_dtypes.float8_e4m3

# ---- problem constants (per core) ----
C = 768
CA = 384
T = 8
NPL = 240                 # padded plane stride: 15 rows x 16 cols
CLIPS = 2
NPIX_CLIP = T * 14 * 14
NPIX = CLIPS * NPIX_CLIP
KC = C // 128
NG = CA // 128
GROW = T * NPL            # one (g,c) row of unpadded-t planes (gp/diffp)
H1ROW = (T + 2) * NPL     # one (g,c) row incl t-guard planes (h1p)
H1PAD = NG * CLIPS * H1ROW
GPAD = NG * CLIPS * GROW
GF, GB = 32, 304          # front/back OOB guards for conv rhs over-reads
NTIL2 = (NPIX + 127) // 128
M1_CH = 392
OUT_ROWS = NPIX + 16
CVS = 16.0   # weight up-scale per stage; /CVS**3 folded into final evict

M_ORDER = [3, 4, 5, 0, 1, 2]          # m-block order (offs first); w1c is
                                      # stored column-permuted to match

# tap (dt, dh, dw) lists grouped by dw so DR pairs share dw (step % 16 == 0)
def _pairs(taps):
    by_dw = {}
    for tp in taps:
        by_dw.setdefault(tp[2], []).append(tp)
    prs = []
    for dw in sorted(by_dw):
        grp = by_dw[dw]
        for i in range(0, len(grp) - 1, 2):
            prs.append((grp[i], grp[i + 1]))
        if len(grp) % 2:
            prs.append((grp[-1], None))
    return prs

MAIN_TAPS = [(kd - 1, kh - 1, kw - 1)
             for kd in range(3) for kh in range(3) for kw in range(3)]
OFF_TAPS = [(0, kh - 1, kw - 1) for kh in range(3) for kw in range(3)]
MAIN_PAIRS = _pairs(MAIN_TAPS)   # 15 (12 pairs + 3 singles)
OFF_PAIRS = _pairs(OFF_TAPS)     # 6 (3 pairs + 3 singles)
NPR_MAIN = len(MAIN_PAIRS)
NPR_OFF = len(OFF_PAIRS)
NPR_TOT = (NPR_MAIN + NPR_OFF) * NG   # 63

# j-major; band 7 runs mains first so DVE's last off-evicts aren't needed
# by the earliest conv chunks' psum-bank reuse
M1_CHUNKS = ([(j, mi) for j in range(7) for mi in range(6)]
             + [(7, mi) for mi in (3, 4, 5, 0, 1, 2)])
CHUNK_IDX = {ch: q for q, ch in enumerate(M1_CHUNKS)}
M_POS = {m: mi for mi, m in enumerate(M_ORDER)}
CONV_CHUNKS = [(br, g, c, tc)
               for c in range(2) for tc in range(4)
               for br in (1, 0) for g in range(NG)]    # mains first per block
# diff(g,c) -> engine: Pool does most (it idles after memsets); DVE keeps
# (2,0) appended after its evict stream
DIFF_ENG = {(0, 0): "pl", (1, 0): "pl", (2, 0): "dve",
            (0, 1): "pl", (1, 1): "pl", (2, 1): "pl"}
N_M1 = len(M1_CHUNKS)      # 48
N_CONV = len(CONV_CHUNKS)  # 48

# m2 evict tile -> engine: even tiles DVE (stt w/ residual); odd tiles get
# the residual via a 4096*I bf16 matmul on PE, then a pure scaled ACT copy.
M2E_DVE = {i: i % 2 == 0 for i in range(25)}


def build(debug=False):
    nc = bass.Bass()
    xT = nc.declare_dram_parameter("xT", [C, NPIX], F8, isOutput=False)
    xtok = nc.declare_dram_parameter("xtok", [NPIX, C], BF16, isOutput=False)
    xcls = nc.declare_dram_parameter("xcls", [16, C], BF16, isOutput=False)
    w1c = nc.declare_dram_parameter("w1c", [128, KC // 2 * 2 * C], F8, isOutput=False)
    w2c = nc.declare_dram_parameter("w2c", [128, KC // 2 * 2 * C], F8, isOutput=False)
    diag = nc.declare_dram_parameter("diag", [128, NPR_TOT * 2 * 128], F8, isOutput=False)
    b1c = nc.declare_dram_parameter("b1c", [128, KC], F32, isOutput=False)
    cbc = nc.declare_dram_parameter("cbc", [128, KC], F32, isOutput=False)
    idt = nc.declare_dram_parameter("idt", [128, 128], BF16, isOutput=False)
    out = nc.declare_dram_parameter("out", [OUT_ROWS, C], BF16, isOutput=True)
    if debug:
        dbg_h1 = nc.declare_dram_parameter("dbg_h1", [128, GF + H1PAD + GB], F8, isOutput=True)
        dbg_df = nc.declare_dram_parameter("dbg_df", [128, GF + GPAD + GB], F8, isOutput=True)
        dbg_cv = nc.declare_dram_parameter("dbg_cv", [128, KC * NPIX], F8, isOutput=True)

    xT_sb = nc.alloc_sbuf_tensor([128, KC * NPIX], F8)
    w1_sb = nc.alloc_sbuf_tensor([128, KC // 2 * 2 * C], F8)   # [pair][s][mblk]
    w2_sb = nc.alloc_sbuf_tensor([128, KC // 2 * 2 * C], F8)
    diag_sb = nc.alloc_sbuf_tensor([128, NPR_TOT * 2 * 128], F8)  # [pr][s][m]
    b1_sb = nc.alloc_sbuf_tensor([128, KC], F32)
    cb_sb = nc.alloc_sbuf_tensor([128, KC], F32)
    idt_sb = nc.alloc_sbuf_tensor([128, 128], BF16)
    h1p = nc.alloc_sbuf_tensor([128, GF + H1PAD + GB], F8)
    gp = nc.alloc_sbuf_tensor([128, GPAD], F8)
    dfp = nc.alloc_sbuf_tensor([128, GF + GPAD + GB], F8)
    cvo = nc.alloc_sbuf_tensor([128, KC * NPIX], F8)
    xtk = nc.alloc_sbuf_tensor([128, NTIL2 * C], BF16)
    ost = nc.alloc_sbuf_tensor([128, 8 * C], BF16)
    ps = nc.alloc_psum_tensor([128, 4096], F32)

    def h1_plane(g, c, tpad):
        return GF + (g * CLIPS + c) * H1ROW + tpad * NPL

    def g_plane(g, c, t):
        return (g * CLIPS + c) * GROW + t * NPL

    def df_plane(g, c, t):
        return GF + (g * CLIPS + c) * GROW + t * NPL

    def sv3(buf, ext, offset, dims):
        """3D free view [part + dims] of an sbuf tensor via explicit AP."""
        return bass.AP(buf, offset, [[ext, 128]] + [list(d) for d in dims])

    XT_EXT = KC * NPIX
    W_EXT = KC // 2 * 2 * C
    DG_EXT = NPR_TOT * 2 * 128
    H1_EXT = GF + H1PAD + GB
    GP_EXT = GPAD
    DF_EXT = GF + GPAD + GB
    CV_EXT = KC * NPIX

    # ---------- static evict/producer schedules ----------
    # DVE program positions (1-based sem thresholds after inc):
    #   per j-band: 3 off-evict ops (g=0,1,2); after bands 3 and 7: 3 diffs.
    dve_pos = {}
    pos = 0
    for j in range(8):
        for g in range(NG):
            pos += 1
            dve_pos[("ev", 3 + g, j)] = pos     # evict of m1 chunk (m=3+g, j)
            if j == 3:                          # c=0 diff right after its evict
                pos += 1
                dve_pos[("diff", g, 0)] = pos
    for g in range(NG):                         # c=1 diffs after all evicts
        pos += 1
        dve_pos[("diff", g, 1)] = pos
    DVE_PRE = pos                               # evicts+diffs before m2 evicts
    for i in range(NTIL2):
        if M2E_DVE[i]:
            pos += 1
            dve_pos[("m2e", i)] = pos
    N_PL_MS = 13                                # memset count on Pool (below)

    # ACT program positions: 24 main m1 evicts (j-major, m inner), then conv,
    # then the odd m2-tile evicts
    act_pos = {}
    pos = 0
    for j in range(8):
        for m in range(3):
            pos += 1
            act_pos[("ev", m, j)] = pos
    for qc, ch in enumerate(CONV_CHUNKS):
        pos += 1
        act_pos[("cv", qc)] = pos
    for i in range(NTIL2):
        if not M2E_DVE[i]:
            pos += 1
            act_pos[("m2e", i)] = pos

    def m1_evict_wait(q):
        """(sem_name, thr) for 'm1 chunk q's psum bank has been evicted'."""
        j, mi = M1_CHUNKS[q]
        m = M_ORDER[mi]
        if m >= 3:
            return ("dve", dve_pos[("ev", m, j)])
        return ("act", act_pos[("ev", m, j)])

    def main_data_thr(g, c, tc):
        """ACT threshold: h1 planes t<=2tc+1 of (g,c) evicted."""
        j = c * 4 + min(tc + 1, 3)
        return act_pos[("ev", g, j)]

    M2_THR = []
    for i in range(NTIL2):
        p_hi = (min(128 * (i + 1), NPIX) - 1) // 196
        c_hi, t_hi = divmod(p_hi, T)
        blocks = c_hi * 4 + t_hi // 2       # completed (c,tc) blocks before
        M2_THR.append(act_pos[("cv", blocks * 6 + 5)])

    from contextlib import ExitStack
    _sems = ExitStack()
    xk = [_sems.enter_context(nc.semaphore(f"s_xk{i}")) for i in range(8)]
    ot = [_sems.enter_context(nc.semaphore(f"s_ot{i}")) for i in range(8)]
    s_xt = [_sems.enter_context(nc.semaphore(f"s_xt{i}")) for i in range(4)]
    with (
        _sems,
        nc.Block() as block,
        nc.semaphore("s_ld") as s_ld,
        nc.semaphore("s_w1a") as s_w1a,
        nc.semaphore("s_w1b") as s_w1b,
        nc.semaphore("s_dgo") as s_dgo,
        nc.semaphore("s_dgm") as s_dgm,
        nc.semaphore("s_w2") as s_w2,
        nc.semaphore("s_idt") as s_idt,
        nc.semaphore("s_pe") as s_pe,
        nc.semaphore("s_act") as s_act,
        nc.semaphore("s_dve") as s_dve,
        nc.semaphore("s_pl") as s_pl,
        nc.semaphore("s_cls") as s_cls,
        nc.semaphore("s_dbg") as s_dbg,
    ):
        # ================= SP: all DMA (serial, need-ordered) =================
        @block.sync
        def _(sync):
            w1v = w1_sb[:].rearrange("p (q m) -> p q m", q=KC)    # q=(pr,s)
            w1d = w1c[:].rearrange("p (q m) -> p q m", q=KC)
            sync.dma_start(out=w1v[:, :, 0:128], in_=w1d[:, :, 0:128]
                           ).then_inc(s_w1a, 16)
            xtv = xT_sb[:].rearrange("p (k n) -> p k n", k=KC)
            xtd = xT[:].rearrange("(k p) n -> p k n", p=128)
            sync.dma_start(out=xtv[:, :, 0:784],
                           in_=xtd[:, :, 0:784]).then_inc(s_xt[0], 16)
            sync.dma_start(out=w1v[:, :, 128:768], in_=w1d[:, :, 128:768]
                           ).then_inc(s_w1b, 16)
            sync.dma_start(out=b1_sb[:], in_=b1c[:]).then_inc(s_ld, 16)
            sync.dma_start(out=cb_sb[:], in_=cbc[:]).then_inc(s_ld, 16)
            for qq in (1, 2, 3):
                sync.dma_start(out=xtv[:, :, qq * 784:(qq + 1) * 784],
                               in_=xtd[:, :, qq * 784:(qq + 1) * 784]
                               ).then_inc(s_xt[qq], 16)
            doff = NPR_MAIN * NG * 256
            sync.dma_start(out=diag_sb[:, doff:], in_=diag[:, doff:]
                           ).then_inc(s_dgo, 16)
            for g in range(NG):      # main diag, grouped by g (host layout)
                lo, hi = g * NPR_MAIN * 256, (g + 1) * NPR_MAIN * 256
                sync.dma_start(out=diag_sb[:, lo:hi], in_=diag[:, lo:hi]
                               ).then_inc(s_dgm, 16)
            sync.dma_start(out=idt_sb[:], in_=idt[:]).then_inc(s_idt, 16)
            sync.dma_start(out=w2_sb[:], in_=w2c[:]).then_inc(s_w2, 16)
            sync.dma_start(out=out[NPIX:OUT_ROWS, :], in_=xcls[:]).then_inc(s_cls, 16)
            for pj in range(12):     # xtok pair-loads
                j = 2 * pj
                sync.dma_start(
                    out=xtk[:, j * C:(j + 2) * C].rearrange("p (b c) -> p b c", b=2),
                    in_=xtok[j * 128:(j + 2) * 128, :].rearrange("(b r) c -> r b c", b=2),
                ).then_inc(xk[pj % 8], 16)
            sync.dma_start(out=xtk[:64, bass.ts(24, C)],
                           in_=xtok[24 * 128:NPIX, :]).then_inc(xk[12 % 8], 16)
            if debug:
                sync.wait_ge(s_act, act_pos[("ev", 2, 7)])
                sync.wait_ge(s_dve, DVE_PRE)
                sync.dma_start(out=dbg_h1[:], in_=h1p[:]).then_inc(s_dbg, 16)
                sync.dma_start(out=dbg_df[:], in_=dfp[:]).then_inc(s_dbg, 16)
                sync.wait_ge(s_act, act_pos[("cv", N_CONV - 1)])
                sync.dma_start(out=dbg_cv[:], in_=cvo[:]).then_inc(s_dbg, 16)
            for i in range(NTIL2):       # out stores, single tiles
                rows = min(128, NPIX - i * 128)
                if M2E_DVE[i]:
                    sync.wait_ge(s_dve, dve_pos[("m2e", i)])
                else:
                    sync.wait_ge(s_act, act_pos[("m2e", i)])
                sync.dma_start(out=out[i * 128:i * 128 + rows, :],
                               in_=ost[:rows, bass.ts(i % 8, C)]
                               ).then_inc(ot[i % 8], 16)
            if debug:
                sync.wait_ge(s_dbg, 48)

        # ================= Pool: halo/guard memsets, then m2-evict share ======
        @block.gpsimd
        def _(gpsimd):
            ms = [
                # gp halos: row 0 of each plane + cols 0/15 of the 14 data rows
                bass.AP(gp, 0, [[GP_EXT, 128], [NPL, 48], [1, 16]]),
                bass.AP(gp, 16, [[GP_EXT, 128], [NPL, 48], [16, 14]]),
                bass.AP(gp, 31, [[GP_EXT, 128], [NPL, 48], [16, 14]]),
                # (trailing halo row of each (g,c) row is the next row's
                # plane-0 row-0, zeroed above; diff output inherits zeros)
                # dfp: front/back OOB guards + plane-0 of each (g,c) row
                bass.AP(dfp, 0, [[DF_EXT, 128], [1, GF]]),
                bass.AP(dfp, GF + GPAD, [[DF_EXT, 128], [1, GB]]),
                bass.AP(dfp, GF, [[DF_EXT, 128], [GROW, 6], [1, NPL]]),
                # h1p: front/back guards, t-guard planes (tpad 0/9), halos
                bass.AP(h1p, 0, [[H1_EXT, 128], [1, GF]]),
                bass.AP(h1p, GF + H1PAD, [[H1_EXT, 128], [1, GB]]),
                bass.AP(h1p, GF, [[H1_EXT, 128], [H1ROW, 6], [1, NPL]]),
                bass.AP(h1p, GF + 9 * NPL, [[H1_EXT, 128], [H1ROW, 6], [1, NPL]]),
                bass.AP(h1p, GF, [[H1_EXT, 128], [NPL, 60], [1, 16]]),
                bass.AP(h1p, GF + 16, [[H1_EXT, 128], [NPL, 60], [16, 14]]),
                bass.AP(h1p, GF + 31, [[H1_EXT, 128], [NPL, 60], [16, 14]]),
            ]
            assert len(ms) == N_PL_MS, len(ms)
            for m in ms:
                gpsimd.memset(m, 0.0).then_inc(s_pl, 1)

        # ================= PE =================
        @block.tensor
        def _(tensor):
            tensor.wait_ge(s_w1a, 16)
            tensor.wait_ge(s_xt[0], 16)
            # ---- matmul1 (DR), j-major, banks 0..7 rotating ----
            for q, (j, mi) in enumerate(M1_CHUNKS):
                bank = q % 8
                if q == 1:
                    tensor.wait_ge(s_w1b, 16)
                if j in (2, 4, 6) and mi == 0:
                    tensor.wait_ge(s_xt[j // 2], 16)
                if q >= 8:
                    sem, thr = m1_evict_wait(q - 8)
                    tensor.wait_ge(s_dve if sem == "dve" else s_act, thr)
                pv = ps[:, bank * 512: bank * 512 + M1_CH]
                for pr in range(KC // 2):
                    lhsT = sv3(w1_sb, W_EXT, pr * 2 * C + mi * 128,
                               [(C, 2), (1, 128)])
                    rhs = sv3(xT_sb, XT_EXT, (pr * 2) * NPIX + j * M1_CH,
                              [(NPIX, 2), (1, M1_CH)])
                    mm = tensor.matmul(pv, lhsT, rhs, perf_mode=DR,
                                       start=(pr == 0), stop=(pr == KC // 2 - 1))
                mm.then_inc(s_pe, 1)
            # ---- conv (DR pairs), banks 0..7 rotating ----
            tensor.wait_ge(s_dgo, 16)
            tensor.wait_ge(s_pl, N_PL_MS)
            dgm_seen = 0
            for qc, (br, g, c, tc) in enumerate(CONV_CHUNKS):
                bank = qc % 8
                if br and 16 * (g + 1) > dgm_seen:
                    dgm_seen = 16 * (g + 1)
                    tensor.wait_ge(s_dgm, dgm_seen)
                if qc < 8:
                    sem, thr = m1_evict_wait(40 + qc)
                    tensor.wait_ge(s_dve if sem == "dve" else s_act, thr)
                else:
                    tensor.wait_ge(s_act, act_pos[("cv", qc - 8)])
                if br:
                    tensor.wait_ge(s_act, main_data_thr(g, c, tc))
                else:
                    tensor.wait_ge(s_dve, dve_pos[("diff", g, c)])
                pairs = MAIN_PAIRS if br else OFF_PAIRS
                pv = ps[:, bank * 512: bank * 512 + 480]
                for ip, (tA, tB) in enumerate(pairs):
                    dtA, dhA, dwA = tA
                    if br:
                        offA = h1_plane(g, c, 2 * tc + 1 + dtA) + dhA * 16 + dwA
                        buf, ext = h1p, H1_EXT
                    else:
                        offA = df_plane(g, c, 2 * tc + dtA) + dhA * 16 + dwA
                        buf, ext = dfp, DF_EXT
                    if tB is None:
                        sstep = 16
                    else:
                        dtB, dhB, dwB = tB
                        sstep = (dtB - dtA) * NPL + (dhB - dhA) * 16
                    plane = (g * NPR_MAIN + ip) if br else (NPR_MAIN * NG + ip * NG + g)
                    lhsT = sv3(diag_sb, DG_EXT, plane * 256,
                               [(128, 2), (1, 128)])
                    rhs = sv3(buf, ext, offA, [(sstep, 2), (1, 480)])
                    mm = tensor.matmul(pv, lhsT, rhs, perf_mode=DR,
                                       start=(ip == 0), stop=(ip == len(pairs) - 1),
                                       skip_group_check=True)
                mm.then_inc(s_pe, 1)
            # ---- matmul2 (DR, group-pairs), psum pairs rotating ----
            tensor.wait_ge(s_w2, 16)
            tensor.wait_ge(s_idt, 16)
            for i in range(NTIL2):
                rows = min(128, NPIX - i * 128)
                odd = i % 2 == 1
                tensor.wait_ge(s_act, M2_THR[i])
                if i < 4:
                    # banks 2i, 2i+1 last used by conv chunks 40+2i, 41+2i
                    tensor.wait_ge(s_act, act_pos[("cv", 41 + 2 * i)])
                elif M2E_DVE[i - 4]:
                    tensor.wait_ge(s_dve, dve_pos[("m2e", i - 4)])
                else:
                    tensor.wait_ge(s_act, act_pos[("m2e", i - 4)])
                if odd:
                    pj = i // 2
                    tensor.wait_ge(xk[pj % 8], 16 * (pj // 8 + 1))
                pv = ps[:rows, (i % 4) * 1024:(i % 4) * 1024 + 768]
                for pr in range(KC // 2):
                    last = (pr == KC // 2 - 1) and not odd
                    lhsT = sv3(cvo, CV_EXT, (pr * 2) * NPIX + i * 128,
                               [(NPIX, 2), (1, rows)])
                    tensor.matmul(pv[:, 0:512], lhsT,
                                  sv3(w2_sb, W_EXT, pr * 2 * C, [(C, 2), (1, 512)]),
                                  perf_mode=DR,
                                  start=(pr == 0), stop=last,
                                  skip_group_check=True)
                    mm1 = tensor.matmul(pv[:, 512:768], lhsT,
                                        sv3(w2_sb, W_EXT, pr * 2 * C + 512,
                                            [(C, 2), (1, 256)]),
                                        perf_mode=DR,
                                        start=(pr == 0), stop=last,
                                        skip_group_check=True)
                if odd:
                    # accumulate 4096 * x residual so ACT can evict with a
                    # pure scaled copy
                    ilh = bass.AP(idt_sb, 0, [[128, 128], [1, 128]])
                    tensor.matmul(pv[:, 0:512], ilh,
                                  sv3(xtk, NTIL2 * C, i * C, [(1, 512)]),
                                  start=False, stop=True, skip_group_check=True)
                    mm1 = tensor.matmul(pv[:, 512:768], ilh,
                                        sv3(xtk, NTIL2 * C, i * C + 512, [(1, 256)]),
                                        start=False, stop=True,
                                        skip_group_check=True)
                mm1.then_inc(s_pe, 1)

        # ================= ACT: main m1 evicts + all conv evicts =============
        @block.scalar
        def _(scalar):
            scalar.wait_ge(s_ld, 32)
            for j in range(8):
                c, t0 = divmod(2 * j, T)
                for m in range(3):
                    q = j * 6 + 3 + m          # chunk (j, mi=3+m) => main m
                    bank = q % 8
                    scalar.wait_ge(s_pe, q + 1)
                    src = sv3(ps, 4096, bank * 512, [(196, 2), (14, 14), (1, 14)])
                    dst = sv3(h1p, H1_EXT,
                              h1_plane(m, c, t0 + 1) + 17,
                              [(NPL, 2), (16, 14), (1, 14)])
                    scalar.activation(dst, src, AFT.Identity,
                                      bias=b1_sb[:, m:m + 1]).then_inc(s_act, 1)
            for qc, (br, g, c, tc) in enumerate(CONV_CHUNKS):
                bank = qc % 8
                scalar.wait_ge(s_pe, N_M1 + qc + 1)
                grp = g if br else 3 + g
                src = sv3(ps, 4096, bank * 512 + 17,
                          [(NPL, 2), (16, 14), (1, 14)])
                dst = sv3(cvo, CV_EXT,
                          grp * NPIX + c * NPIX_CLIP + 2 * tc * 196,
                          [(196, 2), (14, 14), (1, 14)])
                scalar.activation(dst, src, AFT.Identity,
                                  bias=cb_sb[:, grp:grp + 1]).then_inc(s_act, 1)
            # ---- m2 evict, odd tiles (residual pre-added on PE) ----
            for i in range(1, NTIL2, 2):
                rows = min(128, NPIX - i * 128)
                scalar.wait_ge(s_pe, N_M1 + N_CONV + i + 1)
                if i >= 8:
                    jj = i - 8
                    scalar.wait_ge(ot[jj % 8], 16 * (jj // 8 + 1))
                scalar.activation(
                    ost[:rows, bass.ts(i % 8, C)],
                    ps[:rows, (i % 4) * 1024:(i % 4) * 1024 + 768],
                    AFT.Identity, scale=1.0 / (CVS ** 3)).then_inc(s_act, 1)

        # ================= DVE: off m1 evicts + diffs + m2-evict share =======
        @block.vector
        def _(vector):
            for j in range(8):
                c, t0 = divmod(2 * j, T)
                for g in range(NG):
                    q = j * 6 + g              # chunk (j, mi=g) => m=3+g
                    bank = q % 8
                    vector.wait_ge(s_pe, q + 1)
                    src = sv3(ps, 4096, bank * 512, [(196, 2), (14, 14), (1, 14)])
                    dst = sv3(gp, GP_EXT, g_plane(g, c, t0) + 17,
                              [(NPL, 2), (16, 14), (1, 14)])
                    vector.tensor_copy(dst, src).then_inc(s_dve, 1)
                    if j == 3:
                        if g == 0:
                            vector.wait_ge(s_pl, 3)   # gp halos zeroed
                        a = g_plane(g, 0, 1)
                        b = g_plane(g, 0, 0)
                        d = df_plane(g, 0, 1)
                        vector.tensor_tensor(
                            dfp[:, d:d + 7 * NPL],
                            gp[:, a:a + 7 * NPL], gp[:, b:b + 7 * NPL],
                            op=AOT.subtract).then_inc(s_dve, 1)
            for g in range(NG):                       # c=1 diffs
                a = g_plane(g, 1, 1)
                b = g_plane(g, 1, 0)
                d = df_plane(g, 1, 1)
                vector.tensor_tensor(
                    dfp[:, d:d + 7 * NPL],
                    gp[:, a:a + 7 * NPL], gp[:, b:b + 7 * NPL],
                    op=AOT.subtract).then_inc(s_dve, 1)
            # ---- m2 evict + residual, even tiles (stt from psum) ----
            for i in range(0, NTIL2, 2):
                rows = min(128, NPIX - i * 128)
                vector.wait_ge(s_pe, N_M1 + N_CONV + i + 1)
                pj = i // 2
                vector.wait_ge(xk[pj % 8], 16 * (pj // 8 + 1))
                if i >= 8:
                    jj = i - 8
                    vector.wait_ge(ot[jj % 8], 16 * (jj // 8 + 1))
                vector.scalar_tensor_tensor(
                    ost[:rows, bass.ts(i % 8, C)],
                    ps[:rows, (i % 4) * 1024:(i % 4) * 1024 + 768],
                    1.0 / (CVS ** 3),
                    xtk[:rows, bass.ts(i, C)],
                    op0=AOT.mult, op1=AOT.add).then_inc(s_dve, 1)

    return nc


# ---------------- host side ----------------
_NC_CACHE = {}


def _get_nc():
    if "nc" not in _NC_CACHE:
        _NC_CACHE["nc"] = build()
    return _NC_CACHE["nc"]


def _dr_pack(W):
    """[768(k), M] -> per-partition DR layout [128(ki), pair, s, M] flattened."""
    M = W.shape[1]
    out = np.zeros((128, KC // 2, 2, M), np.float32)
    for pr in range(KC // 2):
        for s in range(2):
            out[:, pr, s, :] = W[pr * 256 + s * 128: pr * 256 + (s + 1) * 128, :]
    return out.reshape(128, KC // 2 * 2 * M)


def _prep_weights(w1, b1, cw, cb, w2, b2, ow1, ob1, ocw, ocb, ow2, ob2):
    w1cat = np.hstack([w1, ow1]) * CVS          # [768, 768], col m-blocks
    perm = np.concatenate([np.arange(m * 128, (m + 1) * 128) for m in M_ORDER])
    w1c = _dr_pack(w1cat[:, perm]).astype(F8NP)
    w2c = _dr_pack(np.vstack([w2, ow2]) * CVS).astype(F8NP)
    # diag DR pairs: [128(ki), pr_tot, s, 128(m)] with diagonal per s
    diag = np.zeros((128, NPR_TOT, 2, 128), np.float32)
    eye = np.eye(128, dtype=bool)

    def tapw(w_, tp, main):
        dt, dh, dw = tp
        if main:
            return w_[:, 0, dt + 1, dh + 1, dw + 1]
        return w_[:, 0, 0, dh + 1, dw + 1]

    for br, (pairs, w_) in enumerate([(MAIN_PAIRS, cw), (OFF_PAIRS, ocw)]):
        for ip, (tA, tB) in enumerate(pairs):
            for g in range(NG):
                # main planes grouped by g (contiguous per-g DMA pieces)
                pi = (g * NPR_MAIN + ip) if br == 0 else (NPR_MAIN * NG + ip * NG + g)
                vA = tapw(w_, tA, br == 0) * CVS
                diag[:, pi, 0, :][eye] = vA[g * 128:(g + 1) * 128]
                if tB is not None:
                    vB = tapw(w_, tB, br == 0) * CVS
                    diag[:, pi, 1, :][eye] = vB[g * 128:(g + 1) * 128]
    b1cv = np.ascontiguousarray(
        (np.concatenate([b1, ob1]) * CVS).reshape(KC, 128).T).astype(np.float32)
    # off-branch ob1 folded through the (linear) dwconv into its evict bias:
    # conv(diff + ob1) = conv(diff) + ob1 * sum(ocw taps)
    ocb_f = ocb + ob1 * ocw.sum(axis=(1, 2, 3, 4))
    cbcv = np.ascontiguousarray(
        (np.concatenate([cb, ocb_f]) * CVS * CVS).reshape(KC, 128).T).astype(np.float32)
    bias2 = (b2 + ob2).astype(np.float32)
    idt = (np.eye(128, dtype=np.float32) * (CVS ** 3)).astype(BF)
    return dict(w1c=w1c, w2c=w2c,
                diag=diag.reshape(128, NPR_TOT * 2 * 128).astype(F8NP),
                b1c=b1cv, cbc=cbcv, idt=idt), bias2


def kernel(**inputs):
    x = np.asarray(inputs["x"], dtype=np.float32)
    Tv = int(np.asarray(inputs["T"]))
    assert Tv == T and x.shape == (128, 197, C)
    wd, bias2 = _prep_weights(
        *[np.asarray(inputs[k], dtype=np.float32) for k in
          ("w1", "b1", "cw", "cb", "w2", "b2", "ow1", "ob1", "ocw", "ocb", "ow2", "ob2")])

    in_maps = []
    for core in range(8):
        xs = x[core * 16:(core + 1) * 16]
        xpat = np.ascontiguousarray(xs[:, 1:, :]).reshape(NPIX, C)
        m = dict(wd)
        m["xT"] = np.ascontiguousarray(xpat.T).astype(F8NP)
        m["xtok"] = (xpat + bias2).astype(BF)
        m["xcls"] = np.ascontiguousarray(xs[:, 0, :]).astype(BF)
        in_maps.append(m)

    nc = _get_nc()
    res = run_bass_kernel_spmd(nc, in_maps, core_ids=list(range(8)))

    full = np.empty((128, 197, C), np.float32)
    for core in range(8):
        o = np.asarray(res.results[core]["out"]).astype(np.float32)
        full[core * 16:(core + 1) * 16, 0, :] = o[NPIX:NPIX + 16]
        full[core * 16:(core + 1) * 16, 1:, :] = o[:NPIX].reshape(16, 196, C)
    return full


# revision 52
# speedup vs baseline: 1.0586x; 1.0586x over previous
"""TRN2 Bass kernel for nn_Adapter (dense_cnn): ViT adapter with two branches
  main:   h1 = xs@w1+b1 ; y = dwconv3d_3x3x3(h1)+cb ; y@w2+b2
  offset: g = xs@ow1    ; d = tdiff(g) ; oc = dwconv_1x3x3(d)+bias' ; oc@ow2
  out = x with patch tokens += main + offset   (CLS rows pass through)

Data-parallel over 8 NeuronCores: 2 clips (16 frames) per core; adapter
weights replicated. Per-core kernel (raw bass, explicit semaphores):
  - fp8-e4m3 DoubleRow matmuls for m1 / depthwise conv (diagonal lhsT) / m2
  - 240-stride padded planes (15 rows x 16 cols); adjacent planes share the
    zero halo row, saving 1/16 of all PE conv streaming
  - m1 is j-major so PE starts on the first 512 columns of x^T; w1/diag are
    DMA'd in need-order pieces
  - evict work is spread across three engines: DVE evicts off-branch m1
    planes then computes wide per-(g,c) frame diffs; ACT evicts main m1
    planes + all conv outputs (two planes per op); m2 evict+residual pairs
    alternate DVE / GPSIMD
  - GPSIMD also zero-fills every halo/guard via memsets (no zeros DMA)
  - weights scaled x16/stage to keep fp8 out of subnormals; the final evict
    multiplies by 1/16^3 and adds the bf16 residual tokens
  - bf16 token stream + bf16 output (cast to f32 on host)

Self-contained: hardcodes shapes for x:[128,197,768], T=8 (asserts).
"""
import numpy as np
import ml_dtypes

import concourse.bass as bass
import concourse.mybir as mybir
from concourse.bass_utils import run_bass_kernel_spmd

F32 = mybir.dt.float32
BF16 = mybir.dt.bfloat16
F8 = mybir.dt.float8e4
AOT = mybir.AluOpType
AFT = mybir.ActivationFunctionType
DR = mybir.MatmulPerfMode.DoubleRow
BF = ml_dtypes.bfloat16
F8NP = ml_dtypes.float8_e4m3

# ---- problem constants (per core) ----
C = 768
CA = 384
T = 8
NPL = 240                 # padded plane stride: 15 rows x 16 cols
CLIPS = 2
NPIX_CLIP = T * 14 * 14
NPIX = CLIPS * NPIX_CLIP
KC = C // 128
NG = CA // 128
GROW = T * NPL            # one (g,c) row of unpadded-t planes (gp/diffp)
H1ROW = (T + 2) * NPL     # one (g,c) row incl t-guard planes (h1p)
H1PAD = NG * CLIPS * H1ROW
GPAD = NG * CLIPS * GROW
GF, GB = 32, 304          # front/back OOB guards for conv rhs over-reads
NTIL2 = (NPIX + 127) // 128
M1_CH = 392
OUT_ROWS = NPIX + 16
CVS = 16.0   # weight up-scale per stage; /CVS**3 folded into final evict

M_ORDER = [3, 4, 5, 0, 1, 2]          # m-block order (offs first); w1c is
                                      # stored column-permuted to match

# tap (dt, dh, dw) lists grouped by dw so DR pairs share dw (step % 16 == 0)
def _pairs(taps):
    by_dw = {}
    for tp in taps:
        by_dw.setdefault(tp[2], []).append(tp)
    prs = []
    for dw in sorted(by_dw):
        grp = by_dw[dw]
        for i in range(0, len(grp) - 1, 2):
            prs.append((grp[i], grp[i + 1]))
        if len(grp) % 2:
            prs.append((grp[-1], None))
    return prs

MAIN_TAPS = [(kd - 1, kh - 1, kw - 1)
             for kd in range(3) for kh in range(3) for kw in range(3)]
OFF_TAPS = [(0, kh - 1, kw - 1) for kh in range(3) for kw in range(3)]
MAIN_PAIRS = _pairs(MAIN_TAPS)   # 15 (12 pairs + 3 singles)
OFF_PAIRS = _pairs(OFF_TAPS)     # 6 (3 pairs + 3 singles)
NPR_MAIN = len(MAIN_PAIRS)
NPR_OFF = len(OFF_PAIRS)
NPR_TOT = (NPR_MAIN + NPR_OFF) * NG   # 63

# j-major; band 7 runs mains first so DVE's last off-evicts aren't needed
# by the earliest conv chunks' psum-bank reuse
M1_CHUNKS = ([(j, mi) for j in range(7) for mi in range(6)]
             + [(7, mi) for mi in (3, 4, 5, 0, 1, 2)])
CHUNK_IDX = {ch: q for q, ch in enumerate(M1_CHUNKS)}
M_POS = {m: mi for mi, m in enumerate(M_ORDER)}
CONV_CHUNKS = [(br, g, c, tc)
               for c in range(2) for tc in range(4)
               for br in (1, 0) for g in range(NG)]    # mains first per block
# diff(g,c) -> engine: Pool does most (it idles after memsets); DVE keeps
# (2,0) appended after its evict stream
DIFF_ENG = {(0, 0): "pl", (1, 0): "pl", (2, 0): "dve",
            (0, 1): "pl", (1, 1): "pl", (2, 1): "pl"}
N_M1 = len(M1_CHUNKS)      # 48
N_CONV = len(CONV_CHUNKS)  # 48

# m2 evict tile -> engine: even tiles DVE (stt w/ residual); odd tiles get
# the residual via a 4096*I bf16 matmul on PE, then a pure scaled ACT copy.
M2E_DVE = {i: i % 2 == 0 for i in range(25)}


def build(debug=False):
    nc = bass.Bass()
    xT = nc.declare_dram_parameter("xT", [C, NPIX], F8, isOutput=False)
    xtok = nc.declare_dram_parameter("xtok", [NPIX, C], BF16, isOutput=False)
    xcls = nc.declare_dram_parameter("xcls", [16, C], BF16, isOutput=False)
    w1c = nc.declare_dram_parameter("w1c", [128, KC // 2 * 2 * C], F8, isOutput=False)
    w2c = nc.declare_dram_parameter("w2c", [128, KC // 2 * 2 * C], F8, isOutput=False)
    diag = nc.declare_dram_parameter("diag", [128, NPR_TOT * 2 * 128], F8, isOutput=False)
    b1c = nc.declare_dram_parameter("b1c", [128, KC], F32, isOutput=False)
    cbc = nc.declare_dram_parameter("cbc", [128, KC], F32, isOutput=False)
    idt = nc.declare_dram_parameter("idt", [128, 128], BF16, isOutput=False)
    out = nc.declare_dram_parameter("out", [OUT_ROWS, C], BF16, isOutput=True)
    if debug:
        dbg_h1 = nc.declare_dram_parameter("dbg_h1", [128, GF + H1PAD + GB], F8, isOutput=True)
        dbg_df = nc.declare_dram_parameter("dbg_df", [128, GF + GPAD + GB], F8, isOutput=True)
        dbg_cv = nc.declare_dram_parameter("dbg_cv", [128, KC * NPIX], F8, isOutput=True)

    xT_sb = nc.alloc_sbuf_tensor([128, KC * NPIX], F8)
    w1_sb = nc.alloc_sbuf_tensor([128, KC // 2 * 2 * C], F8)   # [pair][s][mblk]
    w2_sb = nc.alloc_sbuf_tensor([128, KC // 2 * 2 * C], F8)
    diag_sb = nc.alloc_sbuf_tensor([128, NPR_TOT * 2 * 128], F8)  # [pr][s][m]
    b1_sb = nc.alloc_sbuf_tensor([128, KC], F32)
    cb_sb = nc.alloc_sbuf_tensor([128, KC], F32)
    idt_sb = nc.alloc_sbuf_tensor([128, 128], BF16)
    h1p = nc.alloc_sbuf_tensor([128, GF + H1PAD + GB], F8)
    gp = nc.alloc_sbuf_tensor([128, GPAD], F8)
    dfp = nc.alloc_sbuf_tensor([128, GF + GPAD + GB], F8)
    cvo = nc.alloc_sbuf_tensor([128, KC * NPIX], F8)
    xtk = nc.alloc_sbuf_tensor([128, NTIL2 * C], BF16)
    ost = nc.alloc_sbuf_tensor([128, 8 * C], BF16)
    ps = nc.alloc_psum_tensor([128, 4096], F32)

    def h1_plane(g, c, tpad):
        return GF + (g * CLIPS + c) * H1ROW + tpad * NPL

    def g_plane(g, c, t):
        return (g * CLIPS + c) * GROW + t * NPL

    def df_plane(g, c, t):
        return GF + (g * CLIPS + c) * GROW + t * NPL

    def sv3(buf, ext, offset, dims):
        """3D free view [part + dims] of an sbuf tensor via explicit AP."""
        return bass.AP(buf, offset, [[ext, 128]] + [list(d) for d in dims])

    XT_EXT = KC * NPIX
    W_EXT = KC // 2 * 2 * C
    DG_EXT = NPR_TOT * 2 * 128
    H1_EXT = GF + H1PAD + GB
    GP_EXT = GPAD
    DF_EXT = GF + GPAD + GB
    CV_EXT = KC * NPIX

    # ---------- static evict/producer schedules ----------
    # DVE program positions (1-based sem thresholds after inc):
    #   per j-band: 3 off-evict ops (g=0,1,2); after bands 3 and 7: 3 diffs.
    dve_pos = {}
    pos = 0
    for j in range(8):
        for g in range(NG):
            pos += 1
            dve_pos[("ev", 3 + g, j)] = pos     # evict of m1 chunk (m=3+g, j)
    for gc, eng in DIFF_ENG.items():
        if eng == "dve":
            pos += 1
            dve_pos[("diff",) + gc] = pos
    DVE_PRE = pos                               # evicts+diffs before m2 evicts
    for i in range(NTIL2):
        if M2E_DVE[i]:
            pos += 1
            dve_pos[("m2e", i)] = pos
    N_PL_MS = 13                                # memset count on Pool (below)
    pl_pos = {}
    pos = N_PL_MS
    for gc, eng in DIFF_ENG.items():
        if eng == "pl":
            pos += 1
            pl_pos[("diff",) + gc] = pos

    def diff_wait(g, c):
        if DIFF_ENG[(g, c)] == "dve":
            return ("dve", dve_pos[("diff", g, c)])
        return ("pl", pl_pos[("diff", g, c)])

    # ACT program positions: 24 main m1 evicts (j-major, m inner), then conv,
    # then the odd m2-tile evicts
    act_pos = {}
    pos = 0
    for j in range(8):
        for m in range(3):
            pos += 1
            act_pos[("ev", m, j)] = pos
    for qc, ch in enumerate(CONV_CHUNKS):
        pos += 1
        act_pos[("cv", qc)] = pos
    for i in range(NTIL2):
        if not M2E_DVE[i]:
            pos += 1
            act_pos[("m2e", i)] = pos

    def m1_evict_wait(q):
        """(sem_name, thr) for 'm1 chunk q's psum bank has been evicted'."""
        j, mi = M1_CHUNKS[q]
        m = M_ORDER[mi]
        if m >= 3:
            return ("dve", dve_pos[("ev", m, j)])
        return ("act", act_pos[("ev", m, j)])

    def main_data_thr(g, c, tc):
        """ACT threshold: h1 planes t<=2tc+1 of (g,c) evicted."""
        j = c * 4 + min(tc + 1, 3)
        return act_pos[("ev", g, j)]

    M2_THR = []
    for i in range(NTIL2):
        p_hi = (min(128 * (i + 1), NPIX) - 1) // 196
        c_hi, t_hi = divmod(p_hi, T)
        blocks = c_hi * 4 + t_hi // 2       # completed (c,tc) blocks before
        M2_THR.append(act_pos[("cv", blocks * 6 + 5)])

    from contextlib import ExitStack
    _sems = ExitStack()
    xk = [_sems.enter_context(nc.semaphore(f"s_xk{i}")) for i in range(8)]
    ot = [_sems.enter_context(nc.semaphore(f"s_ot{i}")) for i in range(8)]
    s_xt = [_sems.enter_context(nc.semaphore(f"s_xt{i}")) for i in range(4)]
    with (
        _sems,
        nc.Block() as block,
        nc.semaphore("s_ld") as s_ld,
        nc.semaphore("s_w1a") as s_w1a,
        nc.semaphore("s_w1b") as s_w1b,
        nc.semaphore("s_dgo") as s_dgo,
        nc.semaphore("s_dgm") as s_dgm,
        nc.semaphore("s_w2") as s_w2,
        nc.semaphore("s_idt") as s_idt,
        nc.semaphore("s_pe") as s_pe,
        nc.semaphore("s_act") as s_act,
        nc.semaphore("s_dve") as s_dve,
        nc.semaphore("s_pl") as s_pl,
        nc.semaphore("s_cls") as s_cls,
        nc.semaphore("s_dbg") as s_dbg,
    ):
        # ================= SP: all DMA (serial, need-ordered) =================
        @block.sync
        def _(sync):
            w1v = w1_sb[:].rearrange("p (q m) -> p q m", q=KC)    # q=(pr,s)
            w1d = w1c[:].rearrange("p (q m) -> p q m", q=KC)
            sync.dma_start(out=w1v[:, :, 0:128], in_=w1d[:, :, 0:128]
                           ).then_inc(s_w1a, 16)
            xtv = xT_sb[:].rearrange("p (k n) -> p k n", k=KC)
            xtd = xT[:].rearrange("(k p) n -> p k n", p=128)
            sync.dma_start(out=xtv[:, :, 0:784],
                           in_=xtd[:, :, 0:784]).then_inc(s_xt[0], 16)
            sync.dma_start(out=w1v[:, :, 128:768], in_=w1d[:, :, 128:768]
                           ).then_inc(s_w1b, 16)
            sync.dma_start(out=b1_sb[:], in_=b1c[:]).then_inc(s_ld, 16)
            sync.dma_start(out=cb_sb[:], in_=cbc[:]).then_inc(s_ld, 16)
            for qq in (1, 2, 3):
                sync.dma_start(out=xtv[:, :, qq * 784:(qq + 1) * 784],
                               in_=xtd[:, :, qq * 784:(qq + 1) * 784]
                               ).then_inc(s_xt[qq], 16)
            doff = NPR_MAIN * NG * 256
            sync.dma_start(out=diag_sb[:, doff:], in_=diag[:, doff:]
                           ).then_inc(s_dgo, 16)
            for g in range(NG):      # main diag, grouped by g (host layout)
                lo, hi = g * NPR_MAIN * 256, (g + 1) * NPR_MAIN * 256
                sync.dma_start(out=diag_sb[:, lo:hi], in_=diag[:, lo:hi]
                               ).then_inc(s_dgm, 16)
            sync.dma_start(out=idt_sb[:], in_=idt[:]).then_inc(s_idt, 16)
            sync.dma_start(out=w2_sb[:], in_=w2c[:]).then_inc(s_w2, 16)
            sync.dma_start(out=out[NPIX:OUT_ROWS, :], in_=xcls[:]).then_inc(s_cls, 16)
            for pj in range(12):     # xtok pair-loads
                j = 2 * pj
                sync.dma_start(
                    out=xtk[:, j * C:(j + 2) * C].rearrange("p (b c) -> p b c", b=2),
                    in_=xtok[j * 128:(j + 2) * 128, :].rearrange("(b r) c -> r b c", b=2),
                ).then_inc(xk[pj % 8], 16)
            sync.dma_start(out=xtk[:64, bass.ts(24, C)],
                           in_=xtok[24 * 128:NPIX, :]).then_inc(xk[12 % 8], 16)
            if debug:
                sync.wait_ge(s_act, act_pos[("ev", 2, 7)])
                sync.wait_ge(s_dve, DVE_PRE)
                sync.dma_start(out=dbg_h1[:], in_=h1p[:]).then_inc(s_dbg, 16)
                sync.dma_start(out=dbg_df[:], in_=dfp[:]).then_inc(s_dbg, 16)
                sync.wait_ge(s_act, act_pos[("cv", N_CONV - 1)])
                sync.dma_start(out=dbg_cv[:], in_=cvo[:]).then_inc(s_dbg, 16)
            for i in range(NTIL2):       # out stores, single tiles
                rows = min(128, NPIX - i * 128)
                if M2E_DVE[i]:
                    sync.wait_ge(s_dve, dve_pos[("m2e", i)])
                else:
                    sync.wait_ge(s_act, act_pos[("m2e", i)])
                sync.dma_start(out=out[i * 128:i * 128 + rows, :],
                               in_=ost[:rows, bass.ts(i % 8, C)]
                               ).then_inc(ot[i % 8], 16)
            if debug:
                sync.wait_ge(s_dbg, 48)

        # ================= Pool: halo/guard memsets, then m2-evict share ======
        @block.gpsimd
        def _(gpsimd):
            ms = [
                # gp halos: row 0 of each plane + cols 0/15 of the 14 data rows
                bass.AP(gp, 0, [[GP_EXT, 128], [NPL, 48], [1, 16]]),
                bass.AP(gp, 16, [[GP_EXT, 128], [NPL, 48], [16, 14]]),
                bass.AP(gp, 31, [[GP_EXT, 128], [NPL, 48], [16, 14]]),
                # (trailing halo row of each (g,c) row is the next row's
                # plane-0 row-0, zeroed above; diff output inherits zeros)
                # dfp: front/back OOB guards + plane-0 of each (g,c) row
                bass.AP(dfp, 0, [[DF_EXT, 128], [1, GF]]),
                bass.AP(dfp, GF + GPAD, [[DF_EXT, 128], [1, GB]]),
                bass.AP(dfp, GF, [[DF_EXT, 128], [GROW, 6], [1, NPL]]),
                # h1p: front/back guards, t-guard planes (tpad 0/9), halos
                bass.AP(h1p, 0, [[H1_EXT, 128], [1, GF]]),
                bass.AP(h1p, GF + H1PAD, [[H1_EXT, 128], [1, GB]]),
                bass.AP(h1p, GF, [[H1_EXT, 128], [H1ROW, 6], [1, NPL]]),
                bass.AP(h1p, GF + 9 * NPL, [[H1_EXT, 128], [H1ROW, 6], [1, NPL]]),
                bass.AP(h1p, GF, [[H1_EXT, 128], [NPL, 60], [1, 16]]),
                bass.AP(h1p, GF + 16, [[H1_EXT, 128], [NPL, 60], [16, 14]]),
                bass.AP(h1p, GF + 31, [[H1_EXT, 128], [NPL, 60], [16, 14]]),
            ]
            assert len(ms) == N_PL_MS, len(ms)
            for m in ms:
                gpsimd.memset(m, 0.0).then_inc(s_pl, 1)
            # frame-diff share: each waits the gp evicts it reads (t planes
            # of (g,c) are complete once band 3 (c=0) / 7 (c=1) evict g lands)
            for (g, c), eng in DIFF_ENG.items():
                if eng != "pl":
                    continue
                gpsimd.wait_ge(s_dve, dve_pos[("ev", 3 + g, 4 * c + 3)])
                a = g_plane(g, c, 1)
                b = g_plane(g, c, 0)
                d = df_plane(g, c, 1)
                gpsimd.tensor_tensor(
                    dfp[:, d:d + 7 * NPL],
                    gp[:, a:a + 7 * NPL], gp[:, b:b + 7 * NPL],
                    op=AOT.subtract).then_inc(s_pl, 1)

        # ================= PE =================
        @block.tensor
        def _(tensor):
            tensor.wait_ge(s_w1a, 16)
            tensor.wait_ge(s_xt[0], 16)
            # ---- matmul1 (DR), j-major, banks 0..7 rotating ----
            for q, (j, mi) in enumerate(M1_CHUNKS):
                bank = q % 8
                if q == 1:
                    tensor.wait_ge(s_w1b, 16)
                if j in (2, 4, 6) and mi == 0:
                    tensor.wait_ge(s_xt[j // 2], 16)
                if q >= 8:
                    sem, thr = m1_evict_wait(q - 8)
                    tensor.wait_ge(s_dve if sem == "dve" else s_act, thr)
                pv = ps[:, bank * 512: bank * 512 + M1_CH]
                for pr in range(KC // 2):
                    lhsT = sv3(w1_sb, W_EXT, pr * 2 * C + mi * 128,
                               [(C, 2), (1, 128)])
                    rhs = sv3(xT_sb, XT_EXT, (pr * 2) * NPIX + j * M1_CH,
                              [(NPIX, 2), (1, M1_CH)])
                    mm = tensor.matmul(pv, lhsT, rhs, perf_mode=DR,
                                       start=(pr == 0), stop=(pr == KC // 2 - 1))
                mm.then_inc(s_pe, 1)
            # ---- conv (DR pairs), banks 0..7 rotating ----
            tensor.wait_ge(s_dgo, 16)
            tensor.wait_ge(s_pl, N_PL_MS)
            dgm_seen = 0
            for qc, (br, g, c, tc) in enumerate(CONV_CHUNKS):
                bank = qc % 8
                if br and 16 * (g + 1) > dgm_seen:
                    dgm_seen = 16 * (g + 1)
                    tensor.wait_ge(s_dgm, dgm_seen)
                if qc < 8:
                    sem, thr = m1_evict_wait(40 + qc)
                    tensor.wait_ge(s_dve if sem == "dve" else s_act, thr)
                else:
                    tensor.wait_ge(s_act, act_pos[("cv", qc - 8)])
                if br:
                    tensor.wait_ge(s_act, main_data_thr(g, c, tc))
                else:
                    sem, thr = diff_wait(g, c)
                    tensor.wait_ge(s_dve if sem == "dve" else s_pl, thr)
                pairs = MAIN_PAIRS if br else OFF_PAIRS
                pv = ps[:, bank * 512: bank * 512 + 480]
                for ip, (tA, tB) in enumerate(pairs):
                    dtA, dhA, dwA = tA
                    if br:
                        offA = h1_plane(g, c, 2 * tc + 1 + dtA) + dhA * 16 + dwA
                        buf, ext = h1p, H1_EXT
                    else:
                        offA = df_plane(g, c, 2 * tc + dtA) + dhA * 16 + dwA
                        buf, ext = dfp, DF_EXT
                    if tB is None:
                        sstep = 16
                    else:
                        dtB, dhB, dwB = tB
                        sstep = (dtB - dtA) * NPL + (dhB - dhA) * 16
                    plane = (g * NPR_MAIN + ip) if br else (NPR_MAIN * NG + ip * NG + g)
                    lhsT = sv3(diag_sb, DG_EXT, plane * 256,
                               [(128, 2), (1, 128)])
                    rhs = sv3(buf, ext, offA, [(sstep, 2), (1, 480)])
                    mm = tensor.matmul(pv, lhsT, rhs, perf_mode=DR,
                                       start=(ip == 0), stop=(ip == len(pairs) - 1),
                                       skip_group_check=True)
                mm.then_inc(s_pe, 1)
            # ---- matmul2 (DR, group-pairs), psum pairs rotating ----
            tensor.wait_ge(s_w2, 16)
            tensor.wait_ge(s_idt, 16)
            for i in range(NTIL2):
                rows = min(128, NPIX - i * 128)
                odd = i % 2 == 1
                tensor.wait_ge(s_act, M2_THR[i])
                if i < 4:
                    # banks 2i, 2i+1 last used by conv chunks 40+2i, 41+2i
                    tensor.wait_ge(s_act, act_pos[("cv", 41 + 2 * i)])
                elif M2E_DVE[i - 4]:
                    tensor.wait_ge(s_dve, dve_pos[("m2e", i - 4)])
                else:
                    tensor.wait_ge(s_act, act_pos[("m2e", i - 4)])
                if odd:
                    pj = i // 2
                    tensor.wait_ge(xk[pj % 8], 16 * (pj // 8 + 1))
                pv = ps[:rows, (i % 4) * 1024:(i % 4) * 1024 + 768]
                for pr in range(KC // 2):
                    last = (pr == KC // 2 - 1) and not odd
                    lhsT = sv3(cvo, CV_EXT, (pr * 2) * NPIX + i * 128,
                               [(NPIX, 2), (1, rows)])
                    tensor.matmul(pv[:, 0:512], lhsT,
                                  sv3(w2_sb, W_EXT, pr * 2 * C, [(C, 2), (1, 512)]),
                                  perf_mode=DR,
                                  start=(pr == 0), stop=last,
                                  skip_group_check=True)
                    mm1 = tensor.matmul(pv[:, 512:768], lhsT,
                                        sv3(w2_sb, W_EXT, pr * 2 * C + 512,
                                            [(C, 2), (1, 256)]),
                                        perf_mode=DR,
                                        start=(pr == 0), stop=last,
                                        skip_group_check=True)
                if odd:
                    # accumulate 4096 * x residual so ACT can evict with a
                    # pure scaled copy
                    ilh = bass.AP(idt_sb, 0, [[128, 128], [1, 128]])
                    tensor.matmul(pv[:, 0:512], ilh,
                                  sv3(xtk, NTIL2 * C, i * C, [(1, 512)]),
                                  start=False, stop=True, skip_group_check=True)
                    mm1 = tensor.matmul(pv[:, 512:768], ilh,
                                        sv3(xtk, NTIL2 * C, i * C + 512, [(1, 256)]),
                                        start=False, stop=True,
                                        skip_group_check=True)
                mm1.then_inc(s_pe, 1)

        # ================= ACT: main m1 evicts + all conv evicts =============
        @block.scalar
        def _(scalar):
            scalar.wait_ge(s_ld, 32)
            for j in range(8):
                c, t0 = divmod(2 * j, T)
                for m in range(3):
                    q = CHUNK_IDX[(j, M_POS[m])]   # main m's chunk in band j
                    bank = q % 8
                    scalar.wait_ge(s_pe, q + 1)
                    src = sv3(ps, 4096, bank * 512, [(196, 2), (14, 14), (1, 14)])
                    dst = sv3(h1p, H1_EXT,
                              h1_plane(m, c, t0 + 1) + 17,
                              [(NPL, 2), (16, 14), (1, 14)])
                    scalar.activation(dst, src, AFT.Identity,
                                      bias=b1_sb[:, m:m + 1]).then_inc(s_act, 1)
            for qc, (br, g, c, tc) in enumerate(CONV_CHUNKS):
                bank = qc % 8
                scalar.wait_ge(s_pe, N_M1 + qc + 1)
                grp = g if br else 3 + g
                src = sv3(ps, 4096, bank * 512 + 17,
                          [(NPL, 2), (16, 14), (1, 14)])
                dst = sv3(cvo, CV_EXT,
                          grp * NPIX + c * NPIX_CLIP + 2 * tc * 196,
                          [(196, 2), (14, 14), (1, 14)])
                scalar.activation(dst, src, AFT.Identity,
                                  bias=cb_sb[:, grp:grp + 1]).then_inc(s_act, 1)
            # ---- m2 evict, odd tiles (residual pre-added on PE) ----
            for i in range(1, NTIL2, 2):
                rows = min(128, NPIX - i * 128)
                scalar.wait_ge(s_pe, N_M1 + N_CONV + i + 1)
                if i >= 8:
                    jj = i - 8
                    scalar.wait_ge(ot[jj % 8], 16 * (jj // 8 + 1))
                scalar.activation(
                    ost[:rows, bass.ts(i % 8, C)],
                    ps[:rows, (i % 4) * 1024:(i % 4) * 1024 + 768],
                    AFT.Identity, scale=1.0 / (CVS ** 3)).then_inc(s_act, 1)

        # ================= DVE: off m1 evicts + diffs + m2-evict share =======
        @block.vector
        def _(vector):
            for j in range(8):
                c, t0 = divmod(2 * j, T)
                for g in range(NG):
                    q = CHUNK_IDX[(j, g)]      # chunk (j, mi=g) => m=3+g
                    bank = q % 8
                    vector.wait_ge(s_pe, q + 1)
                    src = sv3(ps, 4096, bank * 512, [(196, 2), (14, 14), (1, 14)])
                    dst = sv3(gp, GP_EXT, g_plane(g, c, t0) + 17,
                              [(NPL, 2), (16, 14), (1, 14)])
                    vector.tensor_copy(dst, src).then_inc(s_dve, 1)
            for (g, c), eng in DIFF_ENG.items():      # DVE's diff share
                if eng != "dve":
                    continue
                vector.wait_ge(s_pl, 3)               # gp halos zeroed
                a = g_plane(g, c, 1)
                b = g_plane(g, c, 0)
                d = df_plane(g, c, 1)
                vector.tensor_tensor(
                    dfp[:, d:d + 7 * NPL],
                    gp[:, a:a + 7 * NPL], gp[:, b:b + 7 * NPL],
                    op=AOT.subtract).then_inc(s_dve, 1)
            # ---- m2 evict + residual, even tiles (stt from psum) ----
            for i in range(0, NTIL2, 2):
                rows = min(128, NPIX - i * 128)
                vector.wait_ge(s_pe, N_M1 + N_CONV + i + 1)
                pj = i // 2
                vector.wait_ge(xk[pj % 8], 16 * (pj // 8 + 1))
                if i >= 8:
                    jj = i - 8
                    vector.wait_ge(ot[jj % 8], 16 * (jj // 8 + 1))
                vector.scalar_tensor_tensor(
                    ost[:rows, bass.ts(i % 8, C)],
                    ps[:rows, (i % 4) * 1024:(i % 4) * 1024 + 768],
                    1.0 / (CVS ** 3),
                    xtk[:rows, bass.ts(i, C)],
                    op0=AOT.mult, op1=AOT.add).then_inc(s_dve, 1)

    return nc


# ---------------- host side ----------------
_NC_CACHE = {}


def _get_nc():
    if "nc" not in _NC_CACHE:
        _NC_CACHE["nc"] = build()
    return _NC_CACHE["nc"]


def _dr_pack(W):
    """[768(k), M] -> per-partition DR layout [128(ki), pair, s, M] flattened."""
    M = W.shape[1]
    out = np.zeros((128, KC // 2, 2, M), np.float32)
    for pr in range(KC // 2):
        for s in range(2):
            out[:, pr, s, :] = W[pr * 256 + s * 128: pr * 256 + (s + 1) * 128, :]
    return out.reshape(128, KC // 2 * 2 * M)


def _prep_weights(w1, b1, cw, cb, w2, b2, ow1, ob1, ocw, ocb, ow2, ob2):
    w1cat = np.hstack([w1, ow1]) * CVS          # [768, 768], col m-blocks
    perm = np.concatenate([np.arange(m * 128, (m + 1) * 128) for m in M_ORDER])
    w1c = _dr_pack(w1cat[:, perm]).astype(F8NP)
    w2c = _dr_pack(np.vstack([w2, ow2]) * CVS).astype(F8NP)
    # diag DR pairs: [128(ki), pr_tot, s, 128(m)] with diagonal per s
    diag = np.zeros((128, NPR_TOT, 2, 128), np.float32)
    eye = np.eye(128, dtype=bool)

    def tapw(w_, tp, main):
        dt, dh, dw = tp
        if main:
            return w_[:, 0, dt + 1, dh + 1, dw + 1]
        return w_[:, 0, 0, dh + 1, dw + 1]

    for br, (pairs, w_) in enumerate([(MAIN_PAIRS, cw), (OFF_PAIRS, ocw)]):
        for ip, (tA, tB) in enumerate(pairs):
            for g in range(NG):
                # main planes grouped by g (contiguous per-g DMA pieces)
                pi = (g * NPR_MAIN + ip) if br == 0 else (NPR_MAIN * NG + ip * NG + g)
                vA = tapw(w_, tA, br == 0) * CVS
                diag[:, pi, 0, :][eye] = vA[g * 128:(g + 1) * 128]
                if tB is not None:
                    vB = tapw(w_, tB, br == 0) * CVS
                    diag[:, pi, 1, :][eye] = vB[g * 128:(g + 1) * 128]
    b1cv = np.ascontiguousarray(
        (np.concatenate([b1, ob1]) * CVS).reshape(KC, 128).T).astype(np.float32)
    # off-branch ob1 folded through the (linear) dwconv into its evict bias:
    # conv(diff + ob1) = conv(diff) + ob1 * sum(ocw taps)
    ocb_f = ocb + ob1 * ocw.sum(axis=(1, 2, 3, 4))
    cbcv = np.ascontiguousarray(
        (np.concatenate([cb, ocb_f]) * CVS * CVS).reshape(KC, 128).T).astype(np.float32)
    bias2 = (b2 + ob2).astype(np.float32)
    idt = (np.eye(128, dtype=np.float32) * (CVS ** 3)).astype(BF)
    return dict(w1c=w1c, w2c=w2c,
                diag=diag.reshape(128, NPR_TOT * 2 * 128).astype(F8NP),
                b1c=b1cv, cbc=cbcv, idt=idt), bias2


def kernel(**inputs):
    x = np.asarray(inputs["x"], dtype=np.float32)
    Tv = int(np.asarray(inputs["T"]))
    assert Tv == T and x.shape == (128, 197, C)
    wd, bias2 = _prep_weights(
        *[np.asarray(inputs[k], dtype=np.float32) for k in
          ("w1", "b1", "cw", "cb", "w2", "b2", "ow1", "ob1", "ocw", "ocb", "ow2", "ob2")])

    in_maps = []
    for core in range(8):
        xs = x[core * 16:(core + 1) * 16]
        xpat = np.ascontiguousarray(xs[:, 1:, :]).reshape(NPIX, C)
        m = dict(wd)
        m["xT"] = np.ascontiguousarray(xpat.T).astype(F8NP)
        m["xtok"] = (xpat + bias2).astype(BF)
        m["xcls"] = np.ascontiguousarray(xs[:, 0, :]).astype(BF)
        in_maps.append(m)

    nc = _get_nc()
    res = run_bass_kernel_spmd(nc, in_maps, core_ids=list(range(8)))

    full = np.empty((128, 197, C), np.float32)
    for core in range(8):
        o = np.asarray(res.results[core]["out"]).astype(np.float32)
        full[core * 16:(core + 1) * 16, 0, :] = o[NPIX:NPIX + 16]
        full[core * 16:(core + 1) * 16, 1:, :] = o[:NPIX].reshape(16, 196, C)
    return full


# revision 75
# speedup vs baseline: 1.0755x; 1.0160x over previous
"""TRN2 Bass kernel for nn_Adapter (dense_cnn): ViT adapter with two branches
  main:   h1 = xs@w1+b1 ; y = dwconv3d_3x3x3(h1)+cb ; y@w2+b2
  offset: g = xs@ow1    ; d = tdiff(g) ; oc = dwconv_1x3x3(d)+bias' ; oc@ow2
  out = x with patch tokens += main + offset   (CLS rows pass through)

Data-parallel over 8 NeuronCores: 2 clips (16 frames) per core; adapter
weights replicated. Per-core kernel (raw bass, explicit semaphores):
  - fp8-e4m3 DoubleRow matmuls for m1 / depthwise conv (diagonal lhsT) / m2
  - 240-stride padded planes (15 rows x 16 cols); adjacent planes share the
    zero halo row, saving 1/16 of all PE conv streaming
  - m1 is j-major so PE starts on the first 512 columns of x^T; w1/diag are
    DMA'd in need-order pieces
  - evict work is spread across three engines: DVE evicts off-branch m1
    planes then computes wide per-(g,c) frame diffs; ACT evicts main m1
    planes + all conv outputs (two planes per op); m2 evict+residual pairs
    alternate DVE / GPSIMD
  - GPSIMD also zero-fills every halo/guard via memsets (no zeros DMA)
  - weights scaled x16/stage to keep fp8 out of subnormals; the final evict
    multiplies by 1/16^3 and adds the bf16 residual tokens
  - bf16 token stream + bf16 output (cast to f32 on host)

Self-contained: hardcodes shapes for x:[128,197,768], T=8 (asserts).
"""
import numpy as np
import ml_dtypes

import concourse.bass as bass
import concourse.mybir as mybir
from concourse.bass_utils import run_bass_kernel_spmd

F32 = mybir.dt.float32
BF16 = mybir.dt.bfloat16
F8 = mybir.dt.float8e4
AOT = mybir.AluOpType
AFT = mybir.ActivationFunctionType
DR = mybir.MatmulPerfMode.DoubleRow
BF = ml_dtypes.bfloat16
F8NP = ml_dtypes.float8_e4m3

# ---- problem constants (per core) ----
C = 768
CA = 384
T = 8
NPL = 240                 # padded plane stride: 15 rows x 16 cols
CLIPS = 2
NPIX_CLIP = T * 14 * 14
NPIX = CLIPS * NPIX_CLIP
KC = C // 128
NG = CA // 128
GROW = T * NPL            # one (g,c) row of unpadded-t planes (gp/diffp)
H1ROW = (T + 2) * NPL     # one (g,c) row incl t-guard planes (h1p)
H1PAD = NG * CLIPS * H1ROW
GPAD = NG * CLIPS * GROW
GF, GB = 32, 304          # front/back OOB guards for conv rhs over-reads
NTIL2 = (NPIX + 127) // 128
M1_CH = 392
OUT_ROWS = NPIX + 16
CVS = 16.0   # weight up-scale per stage; /CVS**3 folded into final evict

M_ORDER = [3, 4, 5, 0, 1, 2]          # m-block order (offs first); w1c is
                                      # stored column-permuted to match

# tap (dt, dh, dw) lists grouped by dw so DR pairs share dw (step % 16 == 0)
def _pairs(taps):
    by_dw = {}
    for tp in taps:
        by_dw.setdefault(tp[2], []).append(tp)
    prs = []
    for dw in sorted(by_dw):
        grp = by_dw[dw]
        for i in range(0, len(grp) - 1, 2):
            prs.append((grp[i], grp[i + 1]))
        if len(grp) % 2:
            prs.append((grp[-1], None))
    return prs

MAIN_TAPS = [(kd - 1, kh - 1, kw - 1)
             for kd in range(3) for kh in range(3) for kw in range(3)]
OFF_TAPS = [(0, kh - 1, kw - 1) for kh in range(3) for kw in range(3)]
MAIN_PAIRS = _pairs(MAIN_TAPS)   # 15 (12 pairs + 3 singles)
OFF_PAIRS = _pairs(OFF_TAPS)     # 6 (3 pairs + 3 singles)
NPR_MAIN = len(MAIN_PAIRS)
NPR_OFF = len(OFF_PAIRS)
NPR_TOT = (NPR_MAIN + NPR_OFF) * NG   # 63

# j-major; band 7 runs mains first so DVE's last off-evicts aren't needed
# by the earliest conv chunks' psum-bank reuse
M1_CHUNKS = ([(j, mi) for j in range(7) for mi in range(6)]
             + [(7, mi) for mi in (3, 4, 5, 0, 1, 2)])
CHUNK_IDX = {ch: q for q, ch in enumerate(M1_CHUNKS)}
M_POS = {m: mi for mi, m in enumerate(M_ORDER)}
CONV_CHUNKS = [(br, g, c, tc)
               for c in range(2) for tc in range(4)
               for br in (1, 0) for g in range(NG)]    # mains first per block
# diff(g,c) -> engine: Pool does most (it idles after memsets); DVE keeps
# (2,0) appended after its evict stream
DIFF_ENG = {(0, 0): "pl", (1, 0): "pl", (2, 0): "dve",
            (0, 1): "pl", (1, 1): "pl", (2, 1): "pl"}
N_M1 = len(M1_CHUNKS)      # 48
N_CONV = len(CONV_CHUNKS)  # 48

# m2 evict tile -> engine: even tiles DVE (stt w/ residual); odd tiles get
# the residual via a 4096*I bf16 matmul on PE, then a pure scaled ACT copy.
M2E_DVE = {i: i % 2 == 0 for i in range(25)}

# m2 tile i is unlocked once conv block (c,tc) covering its last token is
# evicted; PE interleaves each block's conv chunks with its unlocked tiles
def _tile_block(i):
    p_hi = (min(128 * (i + 1), NPIX) - 1) // 196
    c_hi, t_hi = divmod(p_hi, T)
    return c_hi * 4 + t_hi // 2

TILES_BY_BLOCK = [[] for _ in range(8)]
for _i in range(NTIL2):
    TILES_BY_BLOCK[_tile_block(_i)].append(_i)

# PE order: conv block b+1 runs while ACT evicts block b; block b's tiles
# are spread between block b+1's chunks (after chunks 1/3/5) so the 2-slot
# m2 psum rotation never outruns the evict engines
PE_SCHED = []
for _b in range(8):
    _tiles = TILES_BY_BLOCK[_b - 1] if _b >= 1 else []
    for _k in range(6):
        PE_SCHED.append(("cv", _b * 6 + _k))
        if _k % 2 == 1 and _tiles:
            PE_SCHED.append(("m2", _tiles.pop(0)))
    PE_SCHED += [("m2", _i) for _i in _tiles]
PE_SCHED += [("m2", _i) for _i in TILES_BY_BLOCK[7]]
TILES_BY_BLOCK = [[] for _ in range(8)]          # rebuild (popped above)
for _i in range(NTIL2):
    TILES_BY_BLOCK[_tile_block(_i)].append(_i)


def build(debug=False):
    nc = bass.Bass()
    xT = nc.declare_dram_parameter("xT", [C, NPIX], F8, isOutput=False)
    xtok = nc.declare_dram_parameter("xtok", [NPIX, C], BF16, isOutput=False)
    xcls = nc.declare_dram_parameter("xcls", [16, C], BF16, isOutput=False)
    w1c = nc.declare_dram_parameter("w1c", [128, KC // 2 * 2 * C], F8, isOutput=False)
    w2c = nc.declare_dram_parameter("w2c", [128, KC // 2 * 2 * C], F8, isOutput=False)
    diag = nc.declare_dram_parameter("diag", [128, NPR_TOT * 2 * 128], F8, isOutput=False)
    b1c = nc.declare_dram_parameter("b1c", [128, KC], F32, isOutput=False)
    cbc = nc.declare_dram_parameter("cbc", [128, KC], F32, isOutput=False)
    idt = nc.declare_dram_parameter("idt", [128, 128], BF16, isOutput=False)
    out = nc.declare_dram_parameter("out", [OUT_ROWS, C], BF16, isOutput=True)
    if debug:
        dbg_h1 = nc.declare_dram_parameter("dbg_h1", [128, GF + H1PAD + GB], F8, isOutput=True)
        dbg_df = nc.declare_dram_parameter("dbg_df", [128, GF + GPAD + GB], F8, isOutput=True)
        dbg_cv = nc.declare_dram_parameter("dbg_cv", [128, KC * NPIX], F8, isOutput=True)

    xT_sb = nc.alloc_sbuf_tensor([128, KC * NPIX], F8)
    w1_sb = nc.alloc_sbuf_tensor([128, KC // 2 * 2 * C], F8)   # [pair][s][mblk]
    w2_sb = nc.alloc_sbuf_tensor([128, KC // 2 * 2 * C], F8)
    diag_sb = nc.alloc_sbuf_tensor([128, NPR_TOT * 2 * 128], F8)  # [pr][s][m]
    b1_sb = nc.alloc_sbuf_tensor([128, KC], F32)
    cb_sb = nc.alloc_sbuf_tensor([128, KC], F32)
    idt_sb = nc.alloc_sbuf_tensor([128, 128], BF16)
    h1p = nc.alloc_sbuf_tensor([128, GF + H1PAD + GB], F8)
    gp = nc.alloc_sbuf_tensor([128, GPAD], F8)
    dfp = nc.alloc_sbuf_tensor([128, GF + GPAD + GB], F8)
    cvo = nc.alloc_sbuf_tensor([128, KC * NPIX], F8)
    xtk = nc.alloc_sbuf_tensor([128, NTIL2 * C], BF16)
    ost = nc.alloc_sbuf_tensor([128, 8 * C], BF16)
    tmp = nc.alloc_sbuf_tensor([128, 2 * C], BF16)   # odd-tile scaled psum
    warm = nc.alloc_sbuf_tensor([128, 512], F8)
    ps = nc.alloc_psum_tensor([128, 4096], F32)

    def h1_plane(g, c, tpad):
        return GF + (g * CLIPS + c) * H1ROW + tpad * NPL

    def g_plane(g, c, t):
        return (g * CLIPS + c) * GROW + t * NPL

    def df_plane(g, c, t):
        return GF + (g * CLIPS + c) * GROW + t * NPL

    def sv3(buf, ext, offset, dims):
        """3D free view [part + dims] of an sbuf tensor via explicit AP."""
        return bass.AP(buf, offset, [[ext, 128]] + [list(d) for d in dims])

    XT_EXT = KC * NPIX
    W_EXT = KC // 2 * 2 * C
    DG_EXT = NPR_TOT * 2 * 128
    H1_EXT = GF + H1PAD + GB
    GP_EXT = GPAD
    DF_EXT = GF + GPAD + GB
    CV_EXT = KC * NPIX

    # ---------- static evict/producer schedules ----------
    # DVE program positions (1-based sem thresholds after inc):
    #   per j-band: 3 off-evict ops (g=0,1,2); after bands 3 and 7: 3 diffs.
    dve_pos = {}
    pos = 0
    for j in range(8):
        for g in range(NG):
            pos += 1
            dve_pos[("ev", 3 + g, j)] = pos     # evict of m1 chunk (m=3+g, j)
    for gc, eng in DIFF_ENG.items():
        if eng == "dve":
            pos += 1
            dve_pos[("diff",) + gc] = pos
    DVE_PRE = pos                               # evicts+diffs before m2 evicts
    for kind, x in PE_SCHED:
        if kind != "m2":
            continue
        pos += 1
        # even: stt evict from psum; odd: bf16 residual add from tmp
        dve_pos[("m2e", x) if M2E_DVE[x] else ("m2o", x)] = pos
    N_PL_MS = 14                                # memset count on Pool (below)
    pl_pos = {}
    pos = N_PL_MS
    for gc, eng in DIFF_ENG.items():
        if eng == "pl":
            pos += 1
            pl_pos[("diff",) + gc] = pos

    def diff_wait(g, c):
        if DIFF_ENG[(g, c)] == "dve":
            return ("dve", dve_pos[("diff", g, c)])
        return ("pl", pl_pos[("diff", g, c)])

    # ACT program positions: 24 main m1 evicts (j-major, m inner), then the
    # PE_SCHED-ordered conv evicts + odd m2-tile evicts
    act_pos = {}
    pos = 0
    for j in range(8):
        for m in range(3):
            pos += 1
            act_pos[("ev", m, j)] = pos
    for kind, x in PE_SCHED:
        if kind == "cv":
            pos += 1
            act_pos[("cv", x)] = pos
        elif not M2E_DVE[x]:
            pos += 1
            act_pos[("m2e", x)] = pos

    def m1_evict_wait(q):
        """(sem_name, thr) for 'm1 chunk q's psum bank has been evicted'."""
        j, mi = M1_CHUNKS[q]
        m = M_ORDER[mi]
        if m >= 3:
            return ("dve", dve_pos[("ev", m, j)])
        return ("act", act_pos[("ev", m, j)])

    # PE completion positions on s_pe (interleaved conv blocks + m2 tiles)
    pe_pos = {}
    pos = N_M1
    for kind, x in PE_SCHED:
        pos += 1
        pe_pos[(kind, x)] = pos

    def main_data_thr(g, c, tc):
        """ACT threshold: h1 planes t<=2tc+1 of (g,c) evicted."""
        j = c * 4 + min(tc + 1, 3)
        return act_pos[("ev", g, j)]

    M2_THR = []
    for i in range(NTIL2):
        p_hi = (min(128 * (i + 1), NPIX) - 1) // 196
        c_hi, t_hi = divmod(p_hi, T)
        blocks = c_hi * 4 + t_hi // 2       # completed (c,tc) blocks before
        M2_THR.append(act_pos[("cv", blocks * 6 + 5)])

    from contextlib import ExitStack
    _sems = ExitStack()
    xk = [_sems.enter_context(nc.semaphore(f"s_xk{i}")) for i in range(8)]
    ot = [_sems.enter_context(nc.semaphore(f"s_ot{i}")) for i in range(8)]
    s_xt = [_sems.enter_context(nc.semaphore(f"s_xt{i}")) for i in range(4)]
    with (
        _sems,
        nc.Block() as block,
        nc.semaphore("s_ld") as s_ld,
        nc.semaphore("s_w1a") as s_w1a,
        nc.semaphore("s_w1b") as s_w1b,
        nc.semaphore("s_dgo") as s_dgo,
        nc.semaphore("s_dgm") as s_dgm,
        nc.semaphore("s_w2") as s_w2,
        nc.semaphore("s_idt") as s_idt,
        nc.semaphore("s_pe") as s_pe,
        nc.semaphore("s_act") as s_act,
        nc.semaphore("s_dve") as s_dve,
        nc.semaphore("s_pl") as s_pl,
        nc.semaphore("s_cls") as s_cls,
        nc.semaphore("s_dbg") as s_dbg,
    ):
        # ================= SP: all DMA (serial, need-ordered) =================
        @block.sync
        def _(sync):
            w1v = w1_sb[:].rearrange("p (q m) -> p q m", q=KC)    # q=(pr,s)
            w1d = w1c[:].rearrange("p (q m) -> p q m", q=KC)
            sync.dma_start(out=w1v[:, :, 0:128], in_=w1d[:, :, 0:128]
                           ).then_inc(s_w1a, 16)
            xtv = xT_sb[:].rearrange("p (k n) -> p k n", k=KC)
            xtd = xT[:].rearrange("(k p) n -> p k n", p=128)
            sync.dma_start(out=xtv[:, :, 0:784],
                           in_=xtd[:, :, 0:784]).then_inc(s_xt[0], 16)
            sync.dma_start(out=w1v[:, :, 128:768], in_=w1d[:, :, 128:768]
                           ).then_inc(s_w1b, 16)
            sync.dma_start(out=b1_sb[:], in_=b1c[:]).then_inc(s_ld, 16)
            sync.dma_start(out=cb_sb[:], in_=cbc[:]).then_inc(s_ld, 16)
            for qq in (1, 2, 3):
                sync.dma_start(out=xtv[:, :, qq * 784:(qq + 1) * 784],
                               in_=xtd[:, :, qq * 784:(qq + 1) * 784]
                               ).then_inc(s_xt[qq], 16)
            doff = NPR_MAIN * NG * 256
            sync.dma_start(out=diag_sb[:, doff:], in_=diag[:, doff:]
                           ).then_inc(s_dgo, 16)
            for g in range(NG):      # main diag, grouped by g (host layout)
                lo, hi = g * NPR_MAIN * 256, (g + 1) * NPR_MAIN * 256
                sync.dma_start(out=diag_sb[:, lo:hi], in_=diag[:, lo:hi]
                               ).then_inc(s_dgm, 16)
            sync.dma_start(out=idt_sb[:], in_=idt[:]).then_inc(s_idt, 16)
            sync.dma_start(out=w2_sb[:], in_=w2c[:]).then_inc(s_w2, 16)
            sync.dma_start(out=out[NPIX:OUT_ROWS, :], in_=xcls[:]).then_inc(s_cls, 16)
            if debug:
                sync.wait_ge(s_act, act_pos[("ev", 2, 7)])
                sync.wait_ge(s_dve, DVE_PRE)
                sync.dma_start(out=dbg_h1[:], in_=h1p[:]).then_inc(s_dbg, 16)
                sync.dma_start(out=dbg_df[:], in_=dfp[:]).then_inc(s_dbg, 16)
                sync.wait_ge(s_act, act_pos[("cv", N_CONV - 1)])
                sync.dma_start(out=dbg_cv[:], in_=cvo[:]).then_inc(s_dbg, 16)

            def load_pair(pj):
                if pj < 12:
                    j = 2 * pj
                    sync.dma_start(
                        out=xtk[:, j * C:(j + 2) * C].rearrange("p (b c) -> p b c", b=2),
                        in_=xtok[j * 128:(j + 2) * 128, :].rearrange("(b r) c -> r b c", b=2),
                    ).then_inc(xk[pj % 8], 16)
                else:
                    sync.dma_start(out=xtk[:64, bass.ts(24, C)],
                                   in_=xtok[24 * 128:NPIX, :]).then_inc(xk[12 % 8], 16)

            # xtok pair-loads interleaved with out stores: pairs arrive two
            # blocks ahead of the tiles that read them
            seen_pairs = set()

            def pairs_for(b):
                want = sorted({i // 2 for i in TILES_BY_BLOCK[b]} - seen_pairs)
                for pj in want:
                    seen_pairs.add(pj)
                    load_pair(pj)

            pairs_for(0)
            pairs_for(1)
            done_b = -1
            for kind, x in PE_SCHED:
                if kind != "m2":
                    continue
                i = x
                b = _tile_block(i)
                if b > done_b:                  # prefetch two blocks ahead
                    done_b = b
                    if b + 2 < 8:
                        pairs_for(b + 2)
                rows = min(128, NPIX - i * 128)
                if M2E_DVE[i]:
                    sync.wait_ge(s_dve, dve_pos[("m2e", i)])
                else:
                    sync.wait_ge(s_act, act_pos[("m2e", i)])
                sync.dma_start(out=out[i * 128:i * 128 + rows, :],
                               in_=ost[:rows, bass.ts(i % 8, C)]
                               ).then_inc(ot[i % 8], 16)
            if debug:
                sync.wait_ge(s_dbg, 48)

        # ================= Pool: halo/guard memsets, then m2-evict share ======
        @block.gpsimd
        def _(gpsimd):
            gpsimd.memset(warm[:], 0.0).then_inc(s_pl, 1)   # PE warmup source
            ms = [
                # gp halos: row 0 of each plane + cols 0/15 of the 14 data rows
                bass.AP(gp, 0, [[GP_EXT, 128], [NPL, 48], [1, 16]]),
                bass.AP(gp, 16, [[GP_EXT, 128], [NPL, 48], [16, 14]]),
                bass.AP(gp, 31, [[GP_EXT, 128], [NPL, 48], [16, 14]]),
                # (trailing halo row of each (g,c) row is the next row's
                # plane-0 row-0, zeroed above; diff output inherits zeros)
                # dfp: front/back OOB guards + plane-0 of each (g,c) row
                bass.AP(dfp, 0, [[DF_EXT, 128], [1, GF]]),
                bass.AP(dfp, GF + GPAD, [[DF_EXT, 128], [1, GB]]),
                bass.AP(dfp, GF, [[DF_EXT, 128], [GROW, 6], [1, NPL]]),
                # h1p: front/back guards, t-guard planes (tpad 0/9), halos
                bass.AP(h1p, 0, [[H1_EXT, 128], [1, GF]]),
                bass.AP(h1p, GF + H1PAD, [[H1_EXT, 128], [1, GB]]),
                bass.AP(h1p, GF, [[H1_EXT, 128], [H1ROW, 6], [1, NPL]]),
                bass.AP(h1p, GF + 9 * NPL, [[H1_EXT, 128], [H1ROW, 6], [1, NPL]]),
                bass.AP(h1p, GF, [[H1_EXT, 128], [NPL, 60], [1, 16]]),
                bass.AP(h1p, GF + 16, [[H1_EXT, 128], [NPL, 60], [16, 14]]),
                bass.AP(h1p, GF + 31, [[H1_EXT, 128], [NPL, 60], [16, 14]]),
            ]
            assert len(ms) == N_PL_MS - 1, len(ms)   # +1 warm memset
            for m in ms:
                gpsimd.memset(m, 0.0).then_inc(s_pl, 1)
            # frame-diff share: each waits the gp evicts it reads (t planes
            # of (g,c) are complete once band 3 (c=0) / 7 (c=1) evict g lands)
            for (g, c), eng in DIFF_ENG.items():
                if eng != "pl":
                    continue
                gpsimd.wait_ge(s_dve, dve_pos[("ev", 3 + g, 4 * c + 3)])
                a = g_plane(g, c, 1)
                b = g_plane(g, c, 0)
                d = df_plane(g, c, 1)
                gpsimd.tensor_tensor(
                    dfp[:, d:d + 7 * NPL],
                    gp[:, a:a + 7 * NPL], gp[:, b:b + 7 * NPL],
                    op=AOT.subtract).then_inc(s_pl, 1)

        # ================= PE =================
        @block.tensor
        def _(tensor):
            # p-state warmup: ~4.3us of throwaway matmuls during the initial
            # DMA wait so real work starts at full clock
            tensor.wait_ge(s_pl, 1)
            for _ in range(12):
                tensor.matmul(ps[:, 0:512],
                              bass.AP(warm, 0, [[512, 128], [1, 128]]),
                              bass.AP(warm, 0, [[512, 128], [1, 512]]),
                              start=True, stop=True, skip_group_check=True)
            tensor.wait_ge(s_w1a, 16)
            tensor.wait_ge(s_xt[0], 16)
            # ---- matmul1 (DR), j-major, banks 0..7 rotating ----
            for q, (j, mi) in enumerate(M1_CHUNKS):
                bank = q % 8
                if q == 1:
                    tensor.wait_ge(s_w1b, 16)
                if j in (2, 4, 6) and mi == 0:
                    tensor.wait_ge(s_xt[j // 2], 16)
                if q >= 8:
                    sem, thr = m1_evict_wait(q - 8)
                    tensor.wait_ge(s_dve if sem == "dve" else s_act, thr)
                pv = ps[:, bank * 512: bank * 512 + M1_CH]
                for pr in range(KC // 2):
                    lhsT = sv3(w1_sb, W_EXT, pr * 2 * C + mi * 128,
                               [(C, 2), (1, 128)])
                    rhs = sv3(xT_sb, XT_EXT, (pr * 2) * NPIX + j * M1_CH,
                              [(NPIX, 2), (1, M1_CH)])
                    mm = tensor.matmul(pv, lhsT, rhs, perf_mode=DR,
                                       start=(pr == 0), stop=(pr == KC // 2 - 1))
                mm.then_inc(s_pe, 1)
            # ---- conv (banks 0..3) interleaved with m2 tiles (banks 4..7) ----
            tensor.wait_ge(s_dgo, 16)
            tensor.wait_ge(s_pl, N_PL_MS)
            dgm_seen = 0
            w2_seen = False
            for kind, x in PE_SCHED:
                if kind == "cv":
                    qc = x
                    br, g, c, tc = CONV_CHUNKS[qc]
                    bank = qc % 4
                    if br and 16 * (g + 1) > dgm_seen:
                        dgm_seen = 16 * (g + 1)
                        tensor.wait_ge(s_dgm, dgm_seen)
                    if qc < 4:
                        sem, thr = m1_evict_wait(40 + qc)
                        tensor.wait_ge(s_dve if sem == "dve" else s_act, thr)
                    else:
                        tensor.wait_ge(s_act, act_pos[("cv", qc - 4)])
                    if br:
                        tensor.wait_ge(s_act, main_data_thr(g, c, tc))
                    else:
                        sem, thr = diff_wait(g, c)
                        tensor.wait_ge(s_dve if sem == "dve" else s_pl, thr)
                    pairs = MAIN_PAIRS if br else OFF_PAIRS
                    pv = ps[:, bank * 512: bank * 512 + 480]
                    for ip, (tA, tB) in enumerate(pairs):
                        dtA, dhA, dwA = tA
                        if br:
                            offA = h1_plane(g, c, 2 * tc + 1 + dtA) + dhA * 16 + dwA
                            buf, ext = h1p, H1_EXT
                        else:
                            offA = df_plane(g, c, 2 * tc + dtA) + dhA * 16 + dwA
                            buf, ext = dfp, DF_EXT
                        if tB is None:
                            sstep = 16
                        else:
                            dtB, dhB, dwB = tB
                            sstep = (dtB - dtA) * NPL + (dhB - dhA) * 16
                        plane = (g * NPR_MAIN + ip) if br else (NPR_MAIN * NG + ip * NG + g)
                        lhsT = sv3(diag_sb, DG_EXT, plane * 256,
                                   [(128, 2), (1, 128)])
                        rhs = sv3(buf, ext, offA, [(sstep, 2), (1, 480)])
                        mm = tensor.matmul(pv, lhsT, rhs, perf_mode=DR,
                                           start=(ip == 0), stop=(ip == len(pairs) - 1),
                                           skip_group_check=True)
                    mm.then_inc(s_pe, 1)
                else:
                    i = x
                    if not w2_seen:
                        tensor.wait_ge(s_w2, 16)
                        tensor.wait_ge(s_idt, 16)
                        w2_seen = True
                    rows = min(128, NPIX - i * 128)
                    odd = i % 2 == 1
                    tensor.wait_ge(s_act, M2_THR[i])
                    if i < 2:
                        for qq in (44 + 2 * i, 45 + 2 * i):
                            sem, thr = m1_evict_wait(qq)
                            tensor.wait_ge(s_dve if sem == "dve" else s_act, thr)
                    elif M2E_DVE[i - 2]:
                        tensor.wait_ge(s_dve, dve_pos[("m2e", i - 2)])
                    else:
                        tensor.wait_ge(s_act, act_pos[("m2e", i - 2)])
                    if odd:
                        pj = i // 2
                        tensor.wait_ge(xk[pj % 8], 16 * (pj // 8 + 1))
                    pb = (4 + 2 * (i % 2)) * 512
                    pv = ps[:rows, pb:pb + 768]
                    for pr in range(KC // 2):
                        last = (pr == KC // 2 - 1) and not odd
                        lhsT = sv3(cvo, CV_EXT, (pr * 2) * NPIX + i * 128,
                                   [(NPIX, 2), (1, rows)])
                        tensor.matmul(pv[:, 0:512], lhsT,
                                      sv3(w2_sb, W_EXT, pr * 2 * C, [(C, 2), (1, 512)]),
                                      perf_mode=DR,
                                      start=(pr == 0), stop=last,
                                      skip_group_check=True)
                        mm1 = tensor.matmul(pv[:, 512:768], lhsT,
                                            sv3(w2_sb, W_EXT, pr * 2 * C + 512,
                                                [(C, 2), (1, 256)]),
                                            perf_mode=DR,
                                            start=(pr == 0), stop=last,
                                            skip_group_check=True)
                    if odd:
                        # accumulate 4096 * x residual so ACT can evict with
                        # a pure scaled copy
                        ilh = bass.AP(idt_sb, 0, [[128, 128], [1, 128]])
                        tensor.matmul(pv[:, 0:512], ilh,
                                      sv3(xtk, NTIL2 * C, i * C, [(1, 512)]),
                                      start=False, stop=True, skip_group_check=True)
                        mm1 = tensor.matmul(pv[:, 512:768], ilh,
                                            sv3(xtk, NTIL2 * C, i * C + 512, [(1, 256)]),
                                            start=False, stop=True,
                                            skip_group_check=True)
                    mm1.then_inc(s_pe, 1)

        # ================= ACT: main m1 evicts + all conv evicts =============
        @block.scalar
        def _(scalar):
            scalar.wait_ge(s_ld, 32)
            for j in range(8):
                c, t0 = divmod(2 * j, T)
                for m in range(3):
                    q = CHUNK_IDX[(j, M_POS[m])]   # main m's chunk in band j
                    bank = q % 8
                    scalar.wait_ge(s_pe, q + 1)
                    src = sv3(ps, 4096, bank * 512, [(196, 2), (14, 14), (1, 14)])
                    dst = sv3(h1p, H1_EXT,
                              h1_plane(m, c, t0 + 1) + 17,
                              [(NPL, 2), (16, 14), (1, 14)])
                    scalar.activation(dst, src, AFT.Identity,
                                      bias=b1_sb[:, m:m + 1]).then_inc(s_act, 1)
            for kind, x in PE_SCHED:
                if kind == "cv":
                    qc = x
                    br, g, c, tc = CONV_CHUNKS[qc]
                    bank = qc % 4
                    scalar.wait_ge(s_pe, pe_pos[("cv", qc)])
                    grp = g if br else 3 + g
                    src = sv3(ps, 4096, bank * 512 + 17,
                              [(NPL, 2), (16, 14), (1, 14)])
                    dst = sv3(cvo, CV_EXT,
                              grp * NPIX + c * NPIX_CLIP + 2 * tc * 196,
                              [(196, 2), (14, 14), (1, 14)])
                    scalar.activation(dst, src, AFT.Identity,
                                      bias=cb_sb[:, grp:grp + 1]).then_inc(s_act, 1)
                elif not M2E_DVE[x]:
                    # odd m2-tile psum evict: scaled copy to bf16 staging
                    # (DVE adds the residual at 2x)
                    i = x
                    rows = min(128, NPIX - i * 128)
                    scalar.wait_ge(s_pe, pe_pos[("m2", i)])
                    if i >= 4:
                        scalar.wait_ge(s_dve, dve_pos[("m2o", i - 4)])
                    scalar.activation(
                        tmp[:rows, bass.ts((i // 2) % 2, C)],
                        ps[:rows, 6 * 512:6 * 512 + 768],
                        AFT.Identity, scale=1.0 / (CVS ** 3)).then_inc(s_act, 1)

        # ================= DVE: off m1 evicts + diffs + m2-evict share =======
        @block.vector
        def _(vector):
            for j in range(8):
                c, t0 = divmod(2 * j, T)
                for g in range(NG):
                    q = CHUNK_IDX[(j, g)]      # chunk (j, mi=g) => m=3+g
                    bank = q % 8
                    vector.wait_ge(s_pe, q + 1)
                    src = sv3(ps, 4096, bank * 512, [(196, 2), (14, 14), (1, 14)])
                    dst = sv3(gp, GP_EXT, g_plane(g, c, t0) + 17,
                              [(NPL, 2), (16, 14), (1, 14)])
                    vector.tensor_copy(dst, src).then_inc(s_dve, 1)
            for (g, c), eng in DIFF_ENG.items():      # DVE's diff share
                if eng != "dve":
                    continue
                vector.wait_ge(s_pl, 4)               # gp halos zeroed
                a = g_plane(g, c, 1)
                b = g_plane(g, c, 0)
                d = df_plane(g, c, 1)
                vector.tensor_tensor(
                    dfp[:, d:d + 7 * NPL],
                    gp[:, a:a + 7 * NPL], gp[:, b:b + 7 * NPL],
                    op=AOT.subtract).then_inc(s_dve, 1)
            # ---- m2 evict + residual: even tiles stt from psum, odd tiles
            # bf16 add of ACT's scaled copy (2x DVE rate) ----
            for kind, x in PE_SCHED:
                if kind != "m2":
                    continue
                i = x
                rows = min(128, NPIX - i * 128)
                pj = i // 2
                vector.wait_ge(xk[pj % 8], 16 * (pj // 8 + 1))
                if i >= 8:
                    jj = i - 8
                    vector.wait_ge(ot[jj % 8], 16 * (jj // 8 + 1))
                if M2E_DVE[i]:
                    vector.wait_ge(s_pe, pe_pos[("m2", i)])
                    vector.scalar_tensor_tensor(
                        ost[:rows, bass.ts(i % 8, C)],
                        ps[:rows, 4 * 512:4 * 512 + 768],
                        1.0 / (CVS ** 3),
                        xtk[:rows, bass.ts(i, C)],
                        op0=AOT.mult, op1=AOT.add).then_inc(s_dve, 1)
                else:
                    vector.wait_ge(s_act, act_pos[("m2e", i)])
                    vector.tensor_tensor(
                        ost[:rows, bass.ts(i % 8, C)],
                        tmp[:rows, bass.ts((i // 2) % 2, C)],
                        xtk[:rows, bass.ts(i, C)],
                        op=AOT.add).then_inc(s_dve, 1)

    return nc


# ---------------- host side ----------------
_NC_CACHE = {}


def _get_nc():
    if "nc" not in _NC_CACHE:
        _NC_CACHE["nc"] = build()
    return _NC_CACHE["nc"]


def _dr_pack(W):
    """[768(k), M] -> per-partition DR layout [128(ki), pair, s, M] flattened."""
    M = W.shape[1]
    out = np.zeros((128, KC // 2, 2, M), np.float32)
    for pr in range(KC // 2):
        for s in range(2):
            out[:, pr, s, :] = W[pr * 256 + s * 128: pr * 256 + (s + 1) * 128, :]
    return out.reshape(128, KC // 2 * 2 * M)


def _prep_weights(w1, b1, cw, cb, w2, b2, ow1, ob1, ocw, ocb, ow2, ob2):
    w1cat = np.hstack([w1, ow1]) * CVS          # [768, 768], col m-blocks
    perm = np.concatenate([np.arange(m * 128, (m + 1) * 128) for m in M_ORDER])
    w1c = _dr_pack(w1cat[:, perm]).astype(F8NP)
    w2c = _dr_pack(np.vstack([w2, ow2]) * CVS).astype(F8NP)
    # diag DR pairs: [128(ki), pr_tot, s, 128(m)] with diagonal per s
    diag = np.zeros((128, NPR_TOT, 2, 128), np.float32)
    eye = np.eye(128, dtype=bool)

    def tapw(w_, tp, main):
        dt, dh, dw = tp
        if main:
            return w_[:, 0, dt + 1, dh + 1, dw + 1]
        return w_[:, 0, 0, dh + 1, dw + 1]

    for br, (pairs, w_) in enumerate([(MAIN_PAIRS, cw), (OFF_PAIRS, ocw)]):
        for ip, (tA, tB) in enumerate(pairs):
            for g in range(NG):
                # main planes grouped by g (contiguous per-g DMA pieces)
                pi = (g * NPR_MAIN + ip) if br == 0 else (NPR_MAIN * NG + ip * NG + g)
                vA = tapw(w_, tA, br == 0) * CVS
                diag[:, pi, 0, :][eye] = vA[g * 128:(g + 1) * 128]
                if tB is not None:
                    vB = tapw(w_, tB, br == 0) * CVS
                    diag[:, pi, 1, :][eye] = vB[g * 128:(g + 1) * 128]
    b1cv = np.ascontiguousarray(
        (np.concatenate([b1, ob1]) * CVS).reshape(KC, 128).T).astype(np.float32)
    # off-branch ob1 folded through the (linear) dwconv into its evict bias:
    # conv(diff + ob1) = conv(diff) + ob1 * sum(ocw taps)
    ocb_f = ocb + ob1 * ocw.sum(axis=(1, 2, 3, 4))
    cbcv = np.ascontiguousarray(
        (np.concatenate([cb, ocb_f]) * CVS * CVS).reshape(KC, 128).T).astype(np.float32)
    bias2 = (b2 + ob2).astype(np.float32)
    idt = (np.eye(128, dtype=np.float32) * (CVS ** 3)).astype(BF)
    return dict(w1c=w1c, w2c=w2c,
                diag=diag.reshape(128, NPR_TOT * 2 * 128).astype(F8NP),
                b1c=b1cv, cbc=cbcv, idt=idt), bias2


def kernel(**inputs):
    x = np.asarray(inputs["x"], dtype=np.float32)
    Tv = int(np.asarray(inputs["T"]))
    assert Tv == T and x.shape == (128, 197, C)
    wd, bias2 = _prep_weights(
        *[np.asarray(inputs[k], dtype=np.float32) for k in
          ("w1", "b1", "cw", "cb", "w2", "b2", "ow1", "ob1", "ocw", "ocb", "ow2", "ob2")])

    in_maps = []
    for core in range(8):
        xs = x[core * 16:(core + 1) * 16]
        xpat = np.ascontiguousarray(xs[:, 1:, :]).reshape(NPIX, C)
        m = dict(wd)
        m["xT"] = np.ascontiguousarray(xpat.T).astype(F8NP)
        m["xtok"] = (xpat + bias2).astype(BF)
        m["xcls"] = np.ascontiguousarray(xs[:, 0, :]).astype(BF)
        in_maps.append(m)

    nc = _get_nc()
    res = run_bass_kernel_spmd(nc, in_maps, core_ids=list(range(8)))

    full = np.empty((128, 197, C), np.float32)
    for core in range(8):
        o = np.asarray(res.results[core]["out"]).astype(np.float32)
        full[core * 16:(core + 1) * 16, 0, :] = o[NPIX:NPIX + 16]
        full[core * 16:(core + 1) * 16, 1:, :] = o[:NPIX].reshape(16, 196, C)
    return full


# revision 78
# speedup vs baseline: 1.1111x; 1.0331x over previous
"""TRN2 Bass kernel for nn_Adapter (dense_cnn): ViT adapter with two branches
  main:   h1 = xs@w1+b1 ; y = dwconv3d_3x3x3(h1)+cb ; y@w2+b2
  offset: g = xs@ow1    ; d = tdiff(g) ; oc = dwconv_1x3x3(d)+bias' ; oc@ow2
  out = x with patch tokens += main + offset   (CLS rows pass through)

Data-parallel over 8 NeuronCores: 2 clips (16 frames) per core; adapter
weights replicated. Per-core kernel (raw bass, explicit semaphores):
  - fp8-e4m3 DoubleRow matmuls for m1 / depthwise conv (diagonal lhsT) / m2
  - 240-stride padded planes (15 rows x 16 cols); adjacent planes share the
    zero halo row, saving 1/16 of all PE conv streaming
  - m1 is j-major so PE starts on the first 512 columns of x^T; w1/diag are
    DMA'd in need-order pieces
  - evict work is spread across three engines: DVE evicts off-branch m1
    planes then computes wide per-(g,c) frame diffs; ACT evicts main m1
    planes + all conv outputs (two planes per op); m2 evict+residual pairs
    alternate DVE / GPSIMD
  - GPSIMD also zero-fills every halo/guard via memsets (no zeros DMA)
  - weights scaled x16/stage to keep fp8 out of subnormals; the final evict
    multiplies by 1/16^3 and adds the bf16 residual tokens
  - bf16 token stream + bf16 output (cast to f32 on host)

Self-contained: hardcodes shapes for x:[128,197,768], T=8 (asserts).
"""
import numpy as np
import ml_dtypes

import concourse.bass as bass
import concourse.mybir as mybir
from concourse.bass_utils import run_bass_kernel_spmd

F32 = mybir.dt.float32
BF16 = mybir.dt.bfloat16
F8 = mybir.dt.float8e4
AOT = mybir.AluOpType
AFT = mybir.ActivationFunctionType
DR = mybir.MatmulPerfMode.DoubleRow
BF = ml_dtypes.bfloat16
F8NP = ml_dtypes.float8_e4m3

# ---- problem constants (per core) ----
C = 768
CA = 384
T = 8
NPL = 240                 # padded plane stride: 15 rows x 16 cols
CLIPS = 2
NPIX_CLIP = T * 14 * 14
NPIX = CLIPS * NPIX_CLIP
KC = C // 128
NG = CA // 128
GROW = T * NPL            # one (g,c) row of unpadded-t planes (gp/diffp)
H1ROW = (T + 2) * NPL     # one (g,c) row incl t-guard planes (h1p)
H1PAD = NG * CLIPS * H1ROW
GPAD = NG * CLIPS * GROW
GF, GB = 32, 304          # front/back OOB guards for conv rhs over-reads
NTIL2 = (NPIX + 127) // 128
M1_CH = 392
OUT_ROWS = NPIX + 16
CVS = 16.0   # weight up-scale per stage; /CVS**3 folded into final evict

M_ORDER = [3, 4, 5, 0, 1, 2]          # m-block order (offs first); w1c is
                                      # stored column-permuted to match

# tap (dt, dh, dw) lists grouped by dw so DR pairs share dw (step % 16 == 0)
def _pairs(taps):
    by_dw = {}
    for tp in taps:
        by_dw.setdefault(tp[2], []).append(tp)
    prs = []
    for dw in sorted(by_dw):
        grp = by_dw[dw]
        for i in range(0, len(grp) - 1, 2):
            prs.append((grp[i], grp[i + 1]))
        if len(grp) % 2:
            prs.append((grp[-1], None))
    return prs

MAIN_TAPS = [(kd - 1, kh - 1, kw - 1)
             for kd in range(3) for kh in range(3) for kw in range(3)]
OFF_TAPS = [(0, kh - 1, kw - 1) for kh in range(3) for kw in range(3)]
MAIN_PAIRS = _pairs(MAIN_TAPS)   # 15 (12 pairs + 3 singles)
OFF_PAIRS = _pairs(OFF_TAPS)     # 6 (3 pairs + 3 singles)
NPR_MAIN = len(MAIN_PAIRS)
NPR_OFF = len(OFF_PAIRS)
NPR_TOT = (NPR_MAIN + NPR_OFF) * NG   # 63

# j-major; band 7 runs mains first so DVE's last off-evicts aren't needed
# by the earliest conv chunks' psum-bank reuse
M1_CHUNKS = ([(j, mi) for j in range(7) for mi in range(6)]
             + [(7, mi) for mi in (3, 4, 5, 0, 1, 2)])
CHUNK_IDX = {ch: q for q, ch in enumerate(M1_CHUNKS)}
M_POS = {m: mi for mi, m in enumerate(M_ORDER)}
CONV_CHUNKS = [(br, g, c, tc)
               for c in range(2) for tc in range(4)
               for br in (1, 0) for g in range(NG)]    # mains first per block
# diff(g,c) -> engine: Pool does most (it idles after memsets); DVE keeps
# (2,0) appended after its evict stream
DIFF_ENG = {(0, 0): "pl", (1, 0): "pl", (2, 0): "dve",
            (0, 1): "pl", (1, 1): "pl", (2, 1): "pl"}
N_M1 = len(M1_CHUNKS)      # 48
N_CONV = len(CONV_CHUNKS)  # 48

# m2 evict tile -> engine: even tiles DVE (stt w/ residual); odd tiles get
# the residual via a 4096*I bf16 matmul on PE, then a pure scaled ACT copy.
M2E_DVE = {i: i % 2 == 0 for i in range(25)}

# m2 tile i is unlocked once conv block (c,tc) covering its last token is
# evicted; PE interleaves each block's conv chunks with its unlocked tiles
def _tile_block(i):
    p_hi = (min(128 * (i + 1), NPIX) - 1) // 196
    c_hi, t_hi = divmod(p_hi, T)
    return c_hi * 4 + t_hi // 2

TILES_BY_BLOCK = [[] for _ in range(8)]
for _i in range(NTIL2):
    TILES_BY_BLOCK[_tile_block(_i)].append(_i)

# PE order: conv block b+1 runs while ACT evicts block b; block b's tiles
# are spread between block b+1's chunks (after chunks 1/3/5) so the 2-slot
# m2 psum rotation never outruns the evict engines
PE_SCHED = []
for _b in range(8):
    _tiles = TILES_BY_BLOCK[_b - 1] if _b >= 1 else []
    for _k in range(6):
        PE_SCHED.append(("cv", _b * 6 + _k))
        if _k % 2 == 1 and _tiles:
            PE_SCHED.append(("m2", _tiles.pop(0)))
    PE_SCHED += [("m2", _i) for _i in _tiles]
PE_SCHED += [("m2", _i) for _i in TILES_BY_BLOCK[7]]
TILES_BY_BLOCK = [[] for _ in range(8)]          # rebuild (popped above)
for _i in range(NTIL2):
    TILES_BY_BLOCK[_tile_block(_i)].append(_i)


def build(debug=False):
    nc = bass.Bass()
    xT = nc.declare_dram_parameter("xT", [C, NPIX], F8, isOutput=False)
    xtok = nc.declare_dram_parameter("xtok", [NPIX, C], BF16, isOutput=False)
    xcls = nc.declare_dram_parameter("xcls", [16, C], BF16, isOutput=False)
    w1c = nc.declare_dram_parameter("w1c", [128, KC // 2 * 2 * C], F8, isOutput=False)
    w2c = nc.declare_dram_parameter("w2c", [128, KC // 2 * 2 * C], F8, isOutput=False)
    diag = nc.declare_dram_parameter("diag", [128, NPR_TOT * 2 * 128], F8, isOutput=False)
    b1c = nc.declare_dram_parameter("b1c", [128, KC], F32, isOutput=False)
    cbc = nc.declare_dram_parameter("cbc", [128, KC], F32, isOutput=False)
    out = nc.declare_dram_parameter("out", [OUT_ROWS, C], BF16, isOutput=True)
    if debug:
        dbg_h1 = nc.declare_dram_parameter("dbg_h1", [128, GF + H1PAD + GB], F8, isOutput=True)
        dbg_df = nc.declare_dram_parameter("dbg_df", [128, GF + GPAD + GB], F8, isOutput=True)
        dbg_cv = nc.declare_dram_parameter("dbg_cv", [128, KC * NPIX], F8, isOutput=True)

    xT_sb = nc.alloc_sbuf_tensor([128, KC * NPIX], F8)
    w1_sb = nc.alloc_sbuf_tensor([128, KC // 2 * 2 * C], F8)   # [pair][s][mblk]
    w2_sb = nc.alloc_sbuf_tensor([128, KC // 2 * 2 * C], F8)
    diag_sb = nc.alloc_sbuf_tensor([128, NPR_TOT * 2 * 128], F8)  # [pr][s][m]
    b1_sb = nc.alloc_sbuf_tensor([128, KC], F32)
    cb_sb = nc.alloc_sbuf_tensor([128, KC], F32)
    h1p = nc.alloc_sbuf_tensor([128, GF + H1PAD + GB], F8)
    gp = nc.alloc_sbuf_tensor([128, GPAD], F8)
    dfp = nc.alloc_sbuf_tensor([128, GF + GPAD + GB], F8)
    cvo = nc.alloc_sbuf_tensor([128, KC * NPIX], F8)
    xtk = nc.alloc_sbuf_tensor([128, NTIL2 * C], BF16)
    ost = nc.alloc_sbuf_tensor([128, 8 * C], BF16)
    tmp = nc.alloc_sbuf_tensor([128, 2 * C], BF16)   # odd-tile scaled psum
    warm = nc.alloc_sbuf_tensor([128, 512], F8)
    ps = nc.alloc_psum_tensor([128, 4096], F32)

    def h1_plane(g, c, tpad):
        return GF + (g * CLIPS + c) * H1ROW + tpad * NPL

    def g_plane(g, c, t):
        return (g * CLIPS + c) * GROW + t * NPL

    def df_plane(g, c, t):
        return GF + (g * CLIPS + c) * GROW + t * NPL

    def sv3(buf, ext, offset, dims):
        """3D free view [part + dims] of an sbuf tensor via explicit AP."""
        return bass.AP(buf, offset, [[ext, 128]] + [list(d) for d in dims])

    XT_EXT = KC * NPIX
    W_EXT = KC // 2 * 2 * C
    DG_EXT = NPR_TOT * 2 * 128
    H1_EXT = GF + H1PAD + GB
    GP_EXT = GPAD
    DF_EXT = GF + GPAD + GB
    CV_EXT = KC * NPIX

    # ---------- static evict/producer schedules ----------
    # DVE program positions (1-based sem thresholds after inc):
    #   per j-band: 3 off-evict ops (g=0,1,2); after bands 3 and 7: 3 diffs.
    dve_pos = {}
    pos = 0
    for j in range(8):
        for g in range(NG):
            pos += 1
            dve_pos[("ev", 3 + g, j)] = pos     # evict of m1 chunk (m=3+g, j)
    for gc, eng in DIFF_ENG.items():
        if eng == "dve":
            pos += 1
            dve_pos[("diff",) + gc] = pos
    DVE_PRE = pos                               # evicts+diffs before m2 evicts
    for kind, x in PE_SCHED:
        if kind != "m2":
            continue
        pos += 1
        # even: stt evict from psum; odd: bf16 residual add from tmp
        dve_pos[("m2e", x) if M2E_DVE[x] else ("m2o", x)] = pos
    N_PL_MS = 14                                # memset count on Pool (below)
    pl_pos = {}
    pos = N_PL_MS
    for gc, eng in DIFF_ENG.items():
        if eng == "pl":
            pos += 1
            pl_pos[("diff",) + gc] = pos

    def diff_wait(g, c):
        if DIFF_ENG[(g, c)] == "dve":
            return ("dve", dve_pos[("diff", g, c)])
        return ("pl", pl_pos[("diff", g, c)])

    # ACT program positions: 24 main m1 evicts (j-major, m inner), then the
    # PE_SCHED-ordered conv evicts + odd m2-tile evicts
    act_pos = {}
    pos = 0
    for j in range(8):
        for m in range(3):
            pos += 1
            act_pos[("ev", m, j)] = pos
    for kind, x in PE_SCHED:
        if kind == "cv":
            pos += 1
            act_pos[("cv", x)] = pos
        elif not M2E_DVE[x]:
            pos += 1
            act_pos[("m2e", x)] = pos

    def m1_evict_wait(q):
        """(sem_name, thr) for 'm1 chunk q's psum bank has been evicted'."""
        j, mi = M1_CHUNKS[q]
        m = M_ORDER[mi]
        if m >= 3:
            return ("dve", dve_pos[("ev", m, j)])
        return ("act", act_pos[("ev", m, j)])

    # PE completion positions on s_pe (interleaved conv blocks + m2 tiles)
    pe_pos = {}
    pos = N_M1
    for kind, x in PE_SCHED:
        pos += 1
        pe_pos[(kind, x)] = pos

    # m2 tile psum slots: {4,5}/{6,7} alternating; the last block's four
    # tiles fan out over all four bank-pairs (conv banks are free by then)
    PB = {i: (4 + 2 * (i % 2)) * 512 for i in range(NTIL2)}
    b7 = TILES_BY_BLOCK[7]
    PB[b7[0]], PB[b7[1]], PB[b7[2]], PB[b7[3]] = 0, 1024, 3072, 2048
    TILE_BW = {                     # custom bank-free waits for those tiles
        b7[0]: ("act", ("cv", 45)),
        b7[1]: ("act", ("cv", 47)),
        b7[2]: ("act", ("m2e", 19)),
        b7[3]: ("dve", ("m2e", 20)),
    }

    def main_data_thr(g, c, tc):
        """ACT threshold: h1 planes t<=2tc+1 of (g,c) evicted."""
        j = c * 4 + min(tc + 1, 3)
        return act_pos[("ev", g, j)]

    M2_THR = []
    for i in range(NTIL2):
        p_hi = (min(128 * (i + 1), NPIX) - 1) // 196
        c_hi, t_hi = divmod(p_hi, T)
        blocks = c_hi * 4 + t_hi // 2       # completed (c,tc) blocks before
        M2_THR.append(act_pos[("cv", blocks * 6 + 5)])

    from contextlib import ExitStack
    _sems = ExitStack()
    xk = [_sems.enter_context(nc.semaphore(f"s_xk{i}")) for i in range(8)]
    ot = [_sems.enter_context(nc.semaphore(f"s_ot{i}")) for i in range(8)]
    s_xt = [_sems.enter_context(nc.semaphore(f"s_xt{i}")) for i in range(4)]
    with (
        _sems,
        nc.Block() as block,
        nc.semaphore("s_ld") as s_ld,
        nc.semaphore("s_w1a") as s_w1a,
        nc.semaphore("s_w1b") as s_w1b,
        nc.semaphore("s_dgo") as s_dgo,
        nc.semaphore("s_dgm") as s_dgm,
        nc.semaphore("s_w2") as s_w2,
        nc.semaphore("s_pe") as s_pe,
        nc.semaphore("s_act") as s_act,
        nc.semaphore("s_dve") as s_dve,
        nc.semaphore("s_pl") as s_pl,
        nc.semaphore("s_cls") as s_cls,
        nc.semaphore("s_dbg") as s_dbg,
    ):
        # ================= SP: all DMA (serial, need-ordered) =================
        @block.sync
        def _(sync):
            w1v = w1_sb[:].rearrange("p (q m) -> p q m", q=KC)    # q=(pr,s)
            w1d = w1c[:].rearrange("p (q m) -> p q m", q=KC)
            sync.dma_start(out=w1v[:, :, 0:128], in_=w1d[:, :, 0:128]
                           ).then_inc(s_w1a, 16)
            xtv = xT_sb[:].rearrange("p (k n) -> p k n", k=KC)
            xtd = xT[:].rearrange("(k p) n -> p k n", p=128)
            sync.dma_start(out=xtv[:, :, 0:784],
                           in_=xtd[:, :, 0:784]).then_inc(s_xt[0], 16)
            sync.dma_start(out=w1v[:, :, 128:768], in_=w1d[:, :, 128:768]
                           ).then_inc(s_w1b, 16)
            sync.dma_start(out=b1_sb[:], in_=b1c[:]).then_inc(s_ld, 16)
            sync.dma_start(out=cb_sb[:], in_=cbc[:]).then_inc(s_ld, 16)
            for qq in (1, 2, 3):
                sync.dma_start(out=xtv[:, :, qq * 784:(qq + 1) * 784],
                               in_=xtd[:, :, qq * 784:(qq + 1) * 784]
                               ).then_inc(s_xt[qq], 16)
            doff = NPR_MAIN * NG * 256
            sync.dma_start(out=diag_sb[:, doff:], in_=diag[:, doff:]
                           ).then_inc(s_dgo, 16)
            for g in range(NG):      # main diag, grouped by g (host layout)
                lo, hi = g * NPR_MAIN * 256, (g + 1) * NPR_MAIN * 256
                sync.dma_start(out=diag_sb[:, lo:hi], in_=diag[:, lo:hi]
                               ).then_inc(s_dgm, 16)
            sync.dma_start(out=w2_sb[:], in_=w2c[:]).then_inc(s_w2, 16)
            sync.dma_start(out=out[NPIX:OUT_ROWS, :], in_=xcls[:]).then_inc(s_cls, 16)
            if debug:
                sync.wait_ge(s_act, act_pos[("ev", 2, 7)])
                sync.wait_ge(s_dve, DVE_PRE)
                sync.dma_start(out=dbg_h1[:], in_=h1p[:]).then_inc(s_dbg, 16)
                sync.dma_start(out=dbg_df[:], in_=dfp[:]).then_inc(s_dbg, 16)
                sync.wait_ge(s_act, act_pos[("cv", N_CONV - 1)])
                sync.dma_start(out=dbg_cv[:], in_=cvo[:]).then_inc(s_dbg, 16)

            def load_pair(pj):
                if pj < 12:
                    j = 2 * pj
                    sync.dma_start(
                        out=xtk[:, j * C:(j + 2) * C].rearrange("p (b c) -> p b c", b=2),
                        in_=xtok[j * 128:(j + 2) * 128, :].rearrange("(b r) c -> r b c", b=2),
                    ).then_inc(xk[pj % 8], 16)
                else:
                    sync.dma_start(out=xtk[:64, bass.ts(24, C)],
                                   in_=xtok[24 * 128:NPIX, :]).then_inc(xk[12 % 8], 16)

            # xtok pair-loads interleaved with out stores: pairs arrive two
            # blocks ahead of the tiles that read them
            seen_pairs = set()

            def pairs_for(b):
                want = sorted({i // 2 for i in TILES_BY_BLOCK[b]} - seen_pairs)
                for pj in want:
                    seen_pairs.add(pj)
                    load_pair(pj)

            pairs_for(0)
            pairs_for(1)
            done_b = -1
            for kind, x in PE_SCHED:
                if kind != "m2":
                    continue
                i = x
                b = _tile_block(i)
                if b > done_b:                  # prefetch two blocks ahead
                    done_b = b
                    if b + 2 < 8:
                        pairs_for(b + 2)
                rows = min(128, NPIX - i * 128)
                sync.wait_ge(s_dve, dve_pos[("m2e" if M2E_DVE[i] else "m2o", i)])
                sync.dma_start(out=out[i * 128:i * 128 + rows, :],
                               in_=ost[:rows, bass.ts(i % 8, C)]
                               ).then_inc(ot[i % 8], 16)
            if debug:
                sync.wait_ge(s_dbg, 48)

        # ================= Pool: halo/guard memsets, then m2-evict share ======
        @block.gpsimd
        def _(gpsimd):
            gpsimd.memset(warm[:], 0.0).then_inc(s_pl, 1)   # PE warmup source
            ms = [
                # gp halos: row 0 of each plane + cols 0/15 of the 14 data rows
                bass.AP(gp, 0, [[GP_EXT, 128], [NPL, 48], [1, 16]]),
                bass.AP(gp, 16, [[GP_EXT, 128], [NPL, 48], [16, 14]]),
                bass.AP(gp, 31, [[GP_EXT, 128], [NPL, 48], [16, 14]]),
                # (trailing halo row of each (g,c) row is the next row's
                # plane-0 row-0, zeroed above; diff output inherits zeros)
                # dfp: front/back OOB guards + plane-0 of each (g,c) row
                bass.AP(dfp, 0, [[DF_EXT, 128], [1, GF]]),
                bass.AP(dfp, GF + GPAD, [[DF_EXT, 128], [1, GB]]),
                bass.AP(dfp, GF, [[DF_EXT, 128], [GROW, 6], [1, NPL]]),
                # h1p: front/back guards, t-guard planes (tpad 0/9), halos
                bass.AP(h1p, 0, [[H1_EXT, 128], [1, GF]]),
                bass.AP(h1p, GF + H1PAD, [[H1_EXT, 128], [1, GB]]),
                bass.AP(h1p, GF, [[H1_EXT, 128], [H1ROW, 6], [1, NPL]]),
                bass.AP(h1p, GF + 9 * NPL, [[H1_EXT, 128], [H1ROW, 6], [1, NPL]]),
                bass.AP(h1p, GF, [[H1_EXT, 128], [NPL, 60], [1, 16]]),
                bass.AP(h1p, GF + 16, [[H1_EXT, 128], [NPL, 60], [16, 14]]),
                bass.AP(h1p, GF + 31, [[H1_EXT, 128], [NPL, 60], [16, 14]]),
            ]
            assert len(ms) == N_PL_MS - 1, len(ms)   # +1 warm memset
            for m in ms:
                gpsimd.memset(m, 0.0).then_inc(s_pl, 1)
            # frame-diff share: each waits the gp evicts it reads (t planes
            # of (g,c) are complete once band 3 (c=0) / 7 (c=1) evict g lands)
            for (g, c), eng in DIFF_ENG.items():
                if eng != "pl":
                    continue
                gpsimd.wait_ge(s_dve, dve_pos[("ev", 3 + g, 4 * c + 3)])
                a = g_plane(g, c, 1)
                b = g_plane(g, c, 0)
                d = df_plane(g, c, 1)
                gpsimd.tensor_tensor(
                    dfp[:, d:d + 7 * NPL],
                    gp[:, a:a + 7 * NPL], gp[:, b:b + 7 * NPL],
                    op=AOT.subtract).then_inc(s_pl, 1)

        # ================= PE =================
        @block.tensor
        def _(tensor):
            # p-state warmup: ~4.3us of throwaway matmuls during the initial
            # DMA wait so real work starts at full clock
            tensor.wait_ge(s_pl, 1)
            for _ in range(12):
                tensor.matmul(ps[:, 0:512],
                              bass.AP(warm, 0, [[512, 128], [1, 128]]),
                              bass.AP(warm, 0, [[512, 128], [1, 512]]),
                              start=True, stop=True, skip_group_check=True)
            tensor.wait_ge(s_w1a, 16)
            tensor.wait_ge(s_xt[0], 16)
            # ---- matmul1 (DR), j-major, banks 0..7 rotating ----
            for q, (j, mi) in enumerate(M1_CHUNKS):
                bank = q % 8
                if q == 1:
                    tensor.wait_ge(s_w1b, 16)
                if j in (2, 4, 6) and mi == 0:
                    tensor.wait_ge(s_xt[j // 2], 16)
                if q >= 8:
                    sem, thr = m1_evict_wait(q - 8)
                    tensor.wait_ge(s_dve if sem == "dve" else s_act, thr)
                pv = ps[:, bank * 512: bank * 512 + M1_CH]
                for pr in range(KC // 2):
                    lhsT = sv3(w1_sb, W_EXT, pr * 2 * C + mi * 128,
                               [(C, 2), (1, 128)])
                    rhs = sv3(xT_sb, XT_EXT, (pr * 2) * NPIX + j * M1_CH,
                              [(NPIX, 2), (1, M1_CH)])
                    mm = tensor.matmul(pv, lhsT, rhs, perf_mode=DR,
                                       start=(pr == 0), stop=(pr == KC // 2 - 1))
                mm.then_inc(s_pe, 1)
            # ---- conv (banks 0..3) interleaved with m2 tiles (banks 4..7) ----
            tensor.wait_ge(s_dgo, 16)
            tensor.wait_ge(s_pl, N_PL_MS)
            dgm_seen = 0
            w2_seen = False
            for kind, x in PE_SCHED:
                if kind == "cv":
                    qc = x
                    br, g, c, tc = CONV_CHUNKS[qc]
                    bank = qc % 4
                    if br and 16 * (g + 1) > dgm_seen:
                        dgm_seen = 16 * (g + 1)
                        tensor.wait_ge(s_dgm, dgm_seen)
                    if qc < 4:
                        sem, thr = m1_evict_wait(40 + qc)
                        tensor.wait_ge(s_dve if sem == "dve" else s_act, thr)
                    else:
                        tensor.wait_ge(s_act, act_pos[("cv", qc - 4)])
                    if br:
                        tensor.wait_ge(s_act, main_data_thr(g, c, tc))
                    else:
                        sem, thr = diff_wait(g, c)
                        tensor.wait_ge(s_dve if sem == "dve" else s_pl, thr)
                    pairs = MAIN_PAIRS if br else OFF_PAIRS
                    pv = ps[:, bank * 512: bank * 512 + 480]
                    for ip, (tA, tB) in enumerate(pairs):
                        dtA, dhA, dwA = tA
                        if br:
                            offA = h1_plane(g, c, 2 * tc + 1 + dtA) + dhA * 16 + dwA
                            buf, ext = h1p, H1_EXT
                        else:
                            offA = df_plane(g, c, 2 * tc + dtA) + dhA * 16 + dwA
                            buf, ext = dfp, DF_EXT
                        if tB is None:
                            sstep = 16
                        else:
                            dtB, dhB, dwB = tB
                            sstep = (dtB - dtA) * NPL + (dhB - dhA) * 16
                        plane = (g * NPR_MAIN + ip) if br else (NPR_MAIN * NG + ip * NG + g)
                        lhsT = sv3(diag_sb, DG_EXT, plane * 256,
                                   [(128, 2), (1, 128)])
                        rhs = sv3(buf, ext, offA, [(sstep, 2), (1, 480)])
                        mm = tensor.matmul(pv, lhsT, rhs, perf_mode=DR,
                                           start=(ip == 0), stop=(ip == len(pairs) - 1),
                                           skip_group_check=True)
                    mm.then_inc(s_pe, 1)
                else:
                    i = x
                    if not w2_seen:
                        tensor.wait_ge(s_w2, 16)
                        w2_seen = True
                    rows = min(128, NPIX - i * 128)
                    odd = i % 2 == 1
                    tensor.wait_ge(s_act, M2_THR[i])
                    if i < 2:
                        for qq in (44 + 2 * i, 45 + 2 * i):
                            sem, thr = m1_evict_wait(qq)
                            tensor.wait_ge(s_dve if sem == "dve" else s_act, thr)
                    elif M2E_DVE[i - 2]:
                        tensor.wait_ge(s_dve, dve_pos[("m2e", i - 2)])
                    else:
                        tensor.wait_ge(s_act, act_pos[("m2e", i - 2)])
                    pb = (4 + 2 * (i % 2)) * 512
                    pv = ps[:rows, pb:pb + 768]
                    for pr in range(KC // 2):
                        last = pr == KC // 2 - 1
                        lhsT = sv3(cvo, CV_EXT, (pr * 2) * NPIX + i * 128,
                                   [(NPIX, 2), (1, rows)])
                        tensor.matmul(pv[:, 0:512], lhsT,
                                      sv3(w2_sb, W_EXT, pr * 2 * C, [(C, 2), (1, 512)]),
                                      perf_mode=DR,
                                      start=(pr == 0), stop=last,
                                      skip_group_check=True)
                        mm1 = tensor.matmul(pv[:, 512:768], lhsT,
                                            sv3(w2_sb, W_EXT, pr * 2 * C + 512,
                                                [(C, 2), (1, 256)]),
                                            perf_mode=DR,
                                            start=(pr == 0), stop=last,
                                            skip_group_check=True)
                    mm1.then_inc(s_pe, 1)

        # ================= ACT: main m1 evicts + all conv evicts =============
        @block.scalar
        def _(scalar):
            scalar.wait_ge(s_ld, 32)
            for j in range(8):
                c, t0 = divmod(2 * j, T)
                for m in range(3):
                    q = CHUNK_IDX[(j, M_POS[m])]   # main m's chunk in band j
                    bank = q % 8
                    scalar.wait_ge(s_pe, q + 1)
                    src = sv3(ps, 4096, bank * 512, [(196, 2), (14, 14), (1, 14)])
                    dst = sv3(h1p, H1_EXT,
                              h1_plane(m, c, t0 + 1) + 17,
                              [(NPL, 2), (16, 14), (1, 14)])
                    scalar.activation(dst, src, AFT.Identity,
                                      bias=b1_sb[:, m:m + 1]).then_inc(s_act, 1)
            for kind, x in PE_SCHED:
                if kind == "cv":
                    qc = x
                    br, g, c, tc = CONV_CHUNKS[qc]
                    bank = qc % 4
                    scalar.wait_ge(s_pe, pe_pos[("cv", qc)])
                    grp = g if br else 3 + g
                    src = sv3(ps, 4096, bank * 512 + 17,
                              [(NPL, 2), (16, 14), (1, 14)])
                    dst = sv3(cvo, CV_EXT,
                              grp * NPIX + c * NPIX_CLIP + 2 * tc * 196,
                              [(196, 2), (14, 14), (1, 14)])
                    scalar.activation(dst, src, AFT.Identity,
                                      bias=cb_sb[:, grp:grp + 1]).then_inc(s_act, 1)
                elif not M2E_DVE[x]:
                    # odd m2-tile psum evict: scaled copy to bf16 staging
                    # (DVE adds the residual at 2x)
                    i = x
                    rows = min(128, NPIX - i * 128)
                    scalar.wait_ge(s_pe, pe_pos[("m2", i)])
                    if i >= 4:
                        scalar.wait_ge(s_dve, dve_pos[("m2o", i - 4)])
                    scalar.activation(
                        tmp[:rows, bass.ts((i // 2) % 2, C)],
                        ps[:rows, 6 * 512:6 * 512 + 768],
                        AFT.Identity, scale=1.0 / (CVS ** 3)).then_inc(s_act, 1)

        # ================= DVE: off m1 evicts + diffs + m2-evict share =======
        @block.vector
        def _(vector):
            for j in range(8):
                c, t0 = divmod(2 * j, T)
                for g in range(NG):
                    q = CHUNK_IDX[(j, g)]      # chunk (j, mi=g) => m=3+g
                    bank = q % 8
                    vector.wait_ge(s_pe, q + 1)
                    src = sv3(ps, 4096, bank * 512, [(196, 2), (14, 14), (1, 14)])
                    dst = sv3(gp, GP_EXT, g_plane(g, c, t0) + 17,
                              [(NPL, 2), (16, 14), (1, 14)])
                    vector.tensor_copy(dst, src).then_inc(s_dve, 1)
            for (g, c), eng in DIFF_ENG.items():      # DVE's diff share
                if eng != "dve":
                    continue
                vector.wait_ge(s_pl, 4)               # gp halos zeroed
                a = g_plane(g, c, 1)
                b = g_plane(g, c, 0)
                d = df_plane(g, c, 1)
                vector.tensor_tensor(
                    dfp[:, d:d + 7 * NPL],
                    gp[:, a:a + 7 * NPL], gp[:, b:b + 7 * NPL],
                    op=AOT.subtract).then_inc(s_dve, 1)
            # ---- m2 evict + residual: even tiles stt from psum, odd tiles
            # bf16 add of ACT's scaled copy (2x DVE rate) ----
            for kind, x in PE_SCHED:
                if kind != "m2":
                    continue
                i = x
                rows = min(128, NPIX - i * 128)
                pj = i // 2
                vector.wait_ge(xk[pj % 8], 16 * (pj // 8 + 1))
                if i >= 8:
                    jj = i - 8
                    vector.wait_ge(ot[jj % 8], 16 * (jj // 8 + 1))
                if M2E_DVE[i]:
                    vector.wait_ge(s_pe, pe_pos[("m2", i)])
                    vector.scalar_tensor_tensor(
                        ost[:rows, bass.ts(i % 8, C)],
                        ps[:rows, 4 * 512:4 * 512 + 768],
                        1.0 / (CVS ** 3),
                        xtk[:rows, bass.ts(i, C)],
                        op0=AOT.mult, op1=AOT.add).then_inc(s_dve, 1)
                else:
                    vector.wait_ge(s_act, act_pos[("m2e", i)])
                    vector.tensor_tensor(
                        ost[:rows, bass.ts(i % 8, C)],
                        tmp[:rows, bass.ts((i // 2) % 2, C)],
                        xtk[:rows, bass.ts(i, C)],
                        op=AOT.add).then_inc(s_dve, 1)

    return nc


# ---------------- host side ----------------
_NC_CACHE = {}


def _get_nc():
    if "nc" not in _NC_CACHE:
        _NC_CACHE["nc"] = build()
    return _NC_CACHE["nc"]


def _dr_pack(W):
    """[768(k), M] -> per-partition DR layout [128(ki), pair, s, M] flattened."""
    M = W.shape[1]
    out = np.zeros((128, KC // 2, 2, M), np.float32)
    for pr in range(KC // 2):
        for s in range(2):
            out[:, pr, s, :] = W[pr * 256 + s * 128: pr * 256 + (s + 1) * 128, :]
    return out.reshape(128, KC // 2 * 2 * M)


def _prep_weights(w1, b1, cw, cb, w2, b2, ow1, ob1, ocw, ocb, ow2, ob2):
    w1cat = np.hstack([w1, ow1]) * CVS          # [768, 768], col m-blocks
    perm = np.concatenate([np.arange(m * 128, (m + 1) * 128) for m in M_ORDER])
    w1c = _dr_pack(w1cat[:, perm]).astype(F8NP)
    w2c = _dr_pack(np.vstack([w2, ow2]) * CVS).astype(F8NP)
    # diag DR pairs: [128(ki), pr_tot, s, 128(m)] with diagonal per s
    diag = np.zeros((128, NPR_TOT, 2, 128), np.float32)
    eye = np.eye(128, dtype=bool)

    def tapw(w_, tp, main):
        dt, dh, dw = tp
        if main:
            return w_[:, 0, dt + 1, dh + 1, dw + 1]
        return w_[:, 0, 0, dh + 1, dw + 1]

    for br, (pairs, w_) in enumerate([(MAIN_PAIRS, cw), (OFF_PAIRS, ocw)]):
        for ip, (tA, tB) in enumerate(pairs):
            for g in range(NG):
                # main planes grouped by g (contiguous per-g DMA pieces)
                pi = (g * NPR_MAIN + ip) if br == 0 else (NPR_MAIN * NG + ip * NG + g)
                vA = tapw(w_, tA, br == 0) * CVS
                diag[:, pi, 0, :][eye] = vA[g * 128:(g + 1) * 128]
                if tB is not None:
                    vB = tapw(w_, tB, br == 0) * CVS
                    diag[:, pi, 1, :][eye] = vB[g * 128:(g + 1) * 128]
    b1cv = np.ascontiguousarray(
        (np.concatenate([b1, ob1]) * CVS).reshape(KC, 128).T).astype(np.float32)
    # off-branch ob1 folded through the (linear) dwconv into its evict bias:
    # conv(diff + ob1) = conv(diff) + ob1 * sum(ocw taps)
    ocb_f = ocb + ob1 * ocw.sum(axis=(1, 2, 3, 4))
    cbcv = np.ascontiguousarray(
        (np.concatenate([cb, ocb_f]) * CVS * CVS).reshape(KC, 128).T).astype(np.float32)
    bias2 = (b2 + ob2).astype(np.float32)
    return dict(w1c=w1c, w2c=w2c,
                diag=diag.reshape(128, NPR_TOT * 2 * 128).astype(F8NP),
                b1c=b1cv, cbc=cbcv), bias2


def kernel(**inputs):
    x = np.asarray(inputs["x"], dtype=np.float32)
    Tv = int(np.asarray(inputs["T"]))
    assert Tv == T and x.shape == (128, 197, C)
    wd, bias2 = _prep_weights(
        *[np.asarray(inputs[k], dtype=np.float32) for k in
          ("w1", "b1", "cw", "cb", "w2", "b2", "ow1", "ob1", "ocw", "ocb", "ow2", "ob2")])

    in_maps = []
    for core in range(8):
        xs = x[core * 16:(core + 1) * 16]
        xpat = np.ascontiguousarray(xs[:, 1:, :]).reshape(NPIX, C)
        m = dict(wd)
        m["xT"] = np.ascontiguousarray(xpat.T).astype(F8NP)
        m["xtok"] = (xpat + bias2).astype(BF)
        m["xcls"] = np.ascontiguousarray(xs[:, 0, :]).astype(BF)
        in_maps.append(m)

    nc = _get_nc()
    res = run_bass_kernel_spmd(nc, in_maps, core_ids=list(range(8)))

    full = np.empty((128, 197, C), np.float32)
    for core in range(8):
        o = np.asarray(res.results[core]["out"]).astype(np.float32)
        full[core * 16:(core + 1) * 16, 0, :] = o[NPIX:NPIX + 16]
        full[core * 16:(core + 1) * 16, 1:, :] = o[:NPIX].reshape(16, 196, C)
    return full


# revision 83
# speedup vs baseline: 1.1168x; 1.0052x over previous
"""TRN2 Bass kernel for nn_Adapter (dense_cnn): ViT adapter with two branches
  main:   h1 = xs@w1+b1 ; y = dwconv3d_3x3x3(h1)+cb ; y@w2+b2
  offset: g = xs@ow1    ; d = tdiff(g) ; oc = dwconv_1x3x3(d)+bias' ; oc@ow2
  out = x with patch tokens += main + offset   (CLS rows pass through)

Data-parallel over 8 NeuronCores: 2 clips (16 frames) per core; adapter
weights replicated. Per-core kernel (raw bass, explicit semaphores):
  - fp8-e4m3 DoubleRow matmuls for m1 / depthwise conv (diagonal lhsT) / m2
  - 240-stride padded planes (15 rows x 16 cols); adjacent planes share the
    zero halo row, saving 1/16 of all PE conv streaming
  - m1 is j-major so PE starts on the first 512 columns of x^T; w1/diag are
    DMA'd in need-order pieces
  - evict work is spread across three engines: DVE evicts off-branch m1
    planes then computes wide per-(g,c) frame diffs; ACT evicts main m1
    planes + all conv outputs (two planes per op); m2 evict+residual pairs
    alternate DVE / GPSIMD
  - GPSIMD also zero-fills every halo/guard via memsets (no zeros DMA)
  - weights scaled x16/stage to keep fp8 out of subnormals; the final evict
    multiplies by 1/16^3 and adds the bf16 residual tokens
  - bf16 token stream + bf16 output (cast to f32 on host)

Self-contained: hardcodes shapes for x:[128,197,768], T=8 (asserts).
"""
import numpy as np
import ml_dtypes

import concourse.bass as bass
import concourse.mybir as mybir
from concourse.bass_utils import run_bass_kernel_spmd

F32 = mybir.dt.float32
BF16 = mybir.dt.bfloat16
F8 = mybir.dt.float8e4
AOT = mybir.AluOpType
AFT = mybir.ActivationFunctionType
DR = mybir.MatmulPerfMode.DoubleRow
BF = ml_dtypes.bfloat16
F8NP = ml_dtypes.float8_e4m3

# ---- problem constants (per core) ----
C = 768
CA = 384
T = 8
NPL = 240                 # padded plane stride: 15 rows x 16 cols
CLIPS = 2
NPIX_CLIP = T * 14 * 14
NPIX = CLIPS * NPIX_CLIP
KC = C // 128
NG = CA // 128
GROW = T * NPL            # one (g,c) row of unpadded-t planes (gp/diffp)
H1ROW = (T + 2) * NPL     # one (g,c) row incl t-guard planes (h1p)
H1PAD = NG * CLIPS * H1ROW
GPAD = NG * CLIPS * GROW
GF, GB = 32, 304          # front/back OOB guards for conv rhs over-reads
NTIL2 = (NPIX + 127) // 128
M1_CH = 392
OUT_ROWS = NPIX + 16
CVS = 16.0   # weight up-scale per stage; /CVS**3 folded into final evict

M_ORDER = [3, 4, 5, 0, 1, 2]          # m-block order (offs first); w1c is
                                      # stored column-permuted to match

# tap (dt, dh, dw) lists grouped by dw so DR pairs share dw (step % 16 == 0)
def _pairs(taps):
    by_dw = {}
    for tp in taps:
        by_dw.setdefault(tp[2], []).append(tp)
    prs = []
    for dw in sorted(by_dw):
        grp = by_dw[dw]
        for i in range(0, len(grp) - 1, 2):
            prs.append((grp[i], grp[i + 1]))
        if len(grp) % 2:
            prs.append((grp[-1], None))
    return prs

MAIN_TAPS = [(kd - 1, kh - 1, kw - 1)
             for kd in range(3) for kh in range(3) for kw in range(3)]
OFF_TAPS = [(0, kh - 1, kw - 1) for kh in range(3) for kw in range(3)]
MAIN_PAIRS = _pairs(MAIN_TAPS)   # 15 (12 pairs + 3 singles)
OFF_PAIRS = _pairs(OFF_TAPS)     # 6 (3 pairs + 3 singles)
NPR_MAIN = len(MAIN_PAIRS)
NPR_OFF = len(OFF_PAIRS)
NPR_TOT = (NPR_MAIN + NPR_OFF) * NG   # 63

# j-major; band 7 runs mains first so DVE's last off-evicts aren't needed
# by the earliest conv chunks' psum-bank reuse
M1_CHUNKS = ([(j, mi) for j in range(7) for mi in range(6)]
             + [(7, mi) for mi in (3, 4, 5, 0, 1, 2)])
CHUNK_IDX = {ch: q for q, ch in enumerate(M1_CHUNKS)}
M_POS = {m: mi for mi, m in enumerate(M_ORDER)}
CONV_CHUNKS = [(br, g, c, tc)
               for c in range(2) for tc in range(4)
               for br in (1, 0) for g in range(NG)]    # mains first per block
# diff(g,c) -> engine: Pool does most (it idles after memsets); DVE keeps
# (2,0) appended after its evict stream
DIFF_ENG = {(0, 0): "pl", (1, 0): "pl", (2, 0): "dve",
            (0, 1): "pl", (1, 1): "pl", (2, 1): "pl"}
N_M1 = len(M1_CHUNKS)      # 48
N_CONV = len(CONV_CHUNKS)  # 48

# m2 evict tile -> engine: even tiles DVE (stt w/ residual); odd tiles get
# the residual via a 4096*I bf16 matmul on PE, then a pure scaled ACT copy.
M2E_DVE = {i: i % 2 == 0 for i in range(25)}

# m2 tile i is unlocked once conv block (c,tc) covering its last token is
# evicted; PE interleaves each block's conv chunks with its unlocked tiles
def _tile_block(i):
    p_hi = (min(128 * (i + 1), NPIX) - 1) // 196
    c_hi, t_hi = divmod(p_hi, T)
    return c_hi * 4 + t_hi // 2

TILES_BY_BLOCK = [[] for _ in range(8)]
for _i in range(NTIL2):
    TILES_BY_BLOCK[_tile_block(_i)].append(_i)

# PE order: conv block b+1 runs while ACT evicts block b; block b's tiles
# are spread between block b+1's chunks (after chunks 1/3/5) so the 2-slot
# m2 psum rotation never outruns the evict engines
PE_SCHED = []
for _b in range(8):
    _tiles = TILES_BY_BLOCK[_b - 1] if _b >= 1 else []
    for _k in range(6):
        PE_SCHED.append(("cv", _b * 6 + _k))
        if _k % 2 == 1 and _tiles:
            PE_SCHED.append(("m2", _tiles.pop(0)))
    PE_SCHED += [("m2", _i) for _i in _tiles]
PE_SCHED += [("m2", _i) for _i in TILES_BY_BLOCK[7]]
TILES_BY_BLOCK = [[] for _ in range(8)]          # rebuild (popped above)
for _i in range(NTIL2):
    TILES_BY_BLOCK[_tile_block(_i)].append(_i)


def build(debug=False):
    nc = bass.Bass()
    xT = nc.declare_dram_parameter("xT", [C, NPIX], F8, isOutput=False)
    xtok = nc.declare_dram_parameter("xtok", [NPIX, C], BF16, isOutput=False)
    xcls = nc.declare_dram_parameter("xcls", [16, C], BF16, isOutput=False)
    head = nc.declare_dram_parameter("head", [128, KC * 128 + KC * 784], F8, isOutput=False)
    w1c = nc.declare_dram_parameter("w1c", [128, KC // 2 * 2 * C], F8, isOutput=False)
    w2c = nc.declare_dram_parameter("w2c", [128, KC // 2 * 2 * C], F8, isOutput=False)
    diag = nc.declare_dram_parameter("diag", [128, NPR_TOT * 2 * 128], F8, isOutput=False)
    b1c = nc.declare_dram_parameter("b1c", [128, KC], F32, isOutput=False)
    cbc = nc.declare_dram_parameter("cbc", [128, KC], F32, isOutput=False)
    out = nc.declare_dram_parameter("out", [OUT_ROWS, C], BF16, isOutput=True)
    if debug:
        dbg_h1 = nc.declare_dram_parameter("dbg_h1", [128, GF + H1PAD + GB], F8, isOutput=True)
        dbg_df = nc.declare_dram_parameter("dbg_df", [128, GF + GPAD + GB], F8, isOutput=True)
        dbg_cv = nc.declare_dram_parameter("dbg_cv", [128, KC * NPIX], F8, isOutput=True)

    xT_sb = nc.alloc_sbuf_tensor([128, KC * NPIX], F8)
    head_sb = nc.alloc_sbuf_tensor([128, KC * 128 + KC * 784], F8)  # w1 mblk0 | xT q0
    w1_sb = nc.alloc_sbuf_tensor([128, KC // 2 * 2 * C], F8)   # [pair][s][mblk]
    w2_sb = nc.alloc_sbuf_tensor([128, KC // 2 * 2 * C], F8)
    diag_sb = nc.alloc_sbuf_tensor([128, NPR_TOT * 2 * 128], F8)  # [pr][s][m]
    b1_sb = nc.alloc_sbuf_tensor([128, KC], F32)
    cb_sb = nc.alloc_sbuf_tensor([128, KC], F32)
    h1p = nc.alloc_sbuf_tensor([128, GF + H1PAD + GB], F8)
    gp = nc.alloc_sbuf_tensor([128, GPAD], F8)
    dfp = nc.alloc_sbuf_tensor([128, GF + GPAD + GB], F8)
    cvo = nc.alloc_sbuf_tensor([128, KC * NPIX], F8)
    xtk = nc.alloc_sbuf_tensor([128, NTIL2 * C], BF16)
    ost = nc.alloc_sbuf_tensor([128, 8 * C], BF16)
    tmp = nc.alloc_sbuf_tensor([128, 2 * C], BF16)   # odd-tile scaled psum
    warm = nc.alloc_sbuf_tensor([128, 512], F8)
    ps = nc.alloc_psum_tensor([128, 4096], F32)

    def h1_plane(g, c, tpad):
        return GF + (g * CLIPS + c) * H1ROW + tpad * NPL

    def g_plane(g, c, t):
        return (g * CLIPS + c) * GROW + t * NPL

    def df_plane(g, c, t):
        return GF + (g * CLIPS + c) * GROW + t * NPL

    def sv3(buf, ext, offset, dims):
        """3D free view [part + dims] of an sbuf tensor via explicit AP."""
        return bass.AP(buf, offset, [[ext, 128]] + [list(d) for d in dims])

    XT_EXT = KC * NPIX
    W_EXT = KC // 2 * 2 * C
    DG_EXT = NPR_TOT * 2 * 128
    H1_EXT = GF + H1PAD + GB
    GP_EXT = GPAD
    DF_EXT = GF + GPAD + GB
    CV_EXT = KC * NPIX

    # ---------- static evict/producer schedules ----------
    # DVE program positions (1-based sem thresholds after inc):
    #   per j-band: 3 off-evict ops (g=0,1,2); after bands 3 and 7: 3 diffs.
    dve_pos = {}
    pos = 0
    for j in range(8):
        for g in range(NG):
            pos += 1
            dve_pos[("ev", 3 + g, j)] = pos     # evict of m1 chunk (m=3+g, j)
    for gc, eng in DIFF_ENG.items():
        if eng == "dve":
            pos += 1
            dve_pos[("diff",) + gc] = pos
    DVE_PRE = pos                               # evicts+diffs before m2 evicts
    for kind, x in PE_SCHED:
        if kind != "m2":
            continue
        pos += 1
        # even: stt evict from psum; odd: bf16 residual add from tmp
        dve_pos[("m2e", x) if M2E_DVE[x] else ("m2o", x)] = pos
    N_PL_MS = 14                                # memset count on Pool (below)
    pl_pos = {}
    pos = N_PL_MS
    for gc, eng in DIFF_ENG.items():
        if eng == "pl":
            pos += 1
            pl_pos[("diff",) + gc] = pos

    def diff_wait(g, c):
        if DIFF_ENG[(g, c)] == "dve":
            return ("dve", dve_pos[("diff", g, c)])
        return ("pl", pl_pos[("diff", g, c)])

    # ACT program positions: 24 main m1 evicts (j-major, m inner), then the
    # PE_SCHED-ordered conv evicts + odd m2-tile evicts
    act_pos = {}
    pos = 0
    for j in range(8):
        for m in range(3):
            pos += 1
            act_pos[("ev", m, j)] = pos
    for kind, x in PE_SCHED:
        if kind == "cv":
            pos += 1
            act_pos[("cv", x)] = pos
        elif not M2E_DVE[x]:
            pos += 1
            act_pos[("m2e", x)] = pos

    def m1_evict_wait(q):
        """(sem_name, thr) for 'm1 chunk q's psum bank has been evicted'."""
        j, mi = M1_CHUNKS[q]
        m = M_ORDER[mi]
        if m >= 3:
            return ("dve", dve_pos[("ev", m, j)])
        return ("act", act_pos[("ev", m, j)])

    # PE completion positions on s_pe (interleaved conv blocks + m2 tiles)
    pe_pos = {}
    pos = N_M1
    for kind, x in PE_SCHED:
        pos += 1
        pe_pos[(kind, x)] = pos

    # m2 tile psum slots: {4,5}/{6,7} alternating; the last block's four
    # tiles fan out over all four bank-pairs (conv banks are free by then)
    PB = {i: (4 + 2 * (i % 2)) * 512 for i in range(NTIL2)}
    b7 = TILES_BY_BLOCK[7]
    PB[b7[0]], PB[b7[1]], PB[b7[2]], PB[b7[3]] = 0, 1024, 3072, 2048
    TILE_BW = {                     # custom bank-free waits for those tiles
        b7[0]: ("act", ("cv", 45)),
        b7[1]: ("act", ("cv", 47)),
        b7[2]: ("act", ("m2e", 19)),
        b7[3]: ("dve", ("m2e", 20)),
    }

    def main_data_thr(g, c, tc):
        """ACT threshold: h1 planes t<=2tc+1 of (g,c) evicted."""
        j = c * 4 + min(tc + 1, 3)
        return act_pos[("ev", g, j)]

    M2_THR = []
    for i in range(NTIL2):
        p_hi = (min(128 * (i + 1), NPIX) - 1) // 196
        c_hi, t_hi = divmod(p_hi, T)
        blocks = c_hi * 4 + t_hi // 2       # completed (c,tc) blocks before
        M2_THR.append(act_pos[("cv", blocks * 6 + 5)])

    from contextlib import ExitStack
    _sems = ExitStack()
    xk = [_sems.enter_context(nc.semaphore(f"s_xk{i}")) for i in range(8)]
    ot = [_sems.enter_context(nc.semaphore(f"s_ot{i}")) for i in range(8)]
    s_xt = [_sems.enter_context(nc.semaphore(f"s_xt{i}")) for i in range(4)]
    with (
        _sems,
        nc.Block(no_gpsimd_drain=True) as block,
        nc.semaphore("s_ld") as s_ld,
        nc.semaphore("s_w1a") as s_w1a,
        nc.semaphore("s_w1b") as s_w1b,
        nc.semaphore("s_dgo") as s_dgo,
        nc.semaphore("s_dgm") as s_dgm,
        nc.semaphore("s_w2") as s_w2,
        nc.semaphore("s_pe") as s_pe,
        nc.semaphore("s_act") as s_act,
        nc.semaphore("s_dve") as s_dve,
        nc.semaphore("s_pl") as s_pl,
        nc.semaphore("s_cls") as s_cls,
        nc.semaphore("s_dbg") as s_dbg,
    ):
        # ================= SP: all DMA (serial, need-ordered) =================
        @block.sync
        def _(sync):
            w1v = w1_sb[:].rearrange("p (q m) -> p q m", q=KC)    # q=(pr,s)
            w1d = w1c[:].rearrange("p (q m) -> p q m", q=KC)
            sync.dma_start(out=head_sb[:], in_=head[:]).then_inc(s_w1a, 16)
            xtv = xT_sb[:].rearrange("p (k n) -> p k n", k=KC)
            xtd = xT[:].rearrange("(k p) n -> p k n", p=128)
            sync.dma_start(out=w1v[:, :, 128:768], in_=w1d[:, :, 128:768]
                           ).then_inc(s_w1b, 16)
            sync.dma_start(out=b1_sb[:], in_=b1c[:]).then_inc(s_ld, 16)
            sync.dma_start(out=cb_sb[:], in_=cbc[:]).then_inc(s_ld, 16)
            for qq in (1, 2, 3):
                sync.dma_start(out=xtv[:, :, qq * 784:(qq + 1) * 784],
                               in_=xtd[:, :, qq * 784:(qq + 1) * 784]
                               ).then_inc(s_xt[qq], 16)
            doff = NPR_MAIN * NG * 256
            sync.dma_start(out=diag_sb[:, doff:], in_=diag[:, doff:]
                           ).then_inc(s_dgo, 16)
            for g in range(NG):      # main diag, grouped by g (host layout)
                lo, hi = g * NPR_MAIN * 256, (g + 1) * NPR_MAIN * 256
                sync.dma_start(out=diag_sb[:, lo:hi], in_=diag[:, lo:hi]
                               ).then_inc(s_dgm, 16)
            sync.dma_start(out=w2_sb[:], in_=w2c[:]).then_inc(s_w2, 16)
            sync.dma_start(out=out[NPIX:OUT_ROWS, :], in_=xcls[:]).then_inc(s_cls, 16)
            if debug:
                sync.wait_ge(s_act, act_pos[("ev", 2, 7)])
                sync.wait_ge(s_dve, DVE_PRE)
                sync.dma_start(out=dbg_h1[:], in_=h1p[:]).then_inc(s_dbg, 16)
                sync.dma_start(out=dbg_df[:], in_=dfp[:]).then_inc(s_dbg, 16)
                sync.wait_ge(s_act, act_pos[("cv", N_CONV - 1)])
                sync.dma_start(out=dbg_cv[:], in_=cvo[:]).then_inc(s_dbg, 16)

            def load_pair(pj):
                if pj < 12:
                    j = 2 * pj
                    sync.dma_start(
                        out=xtk[:, j * C:(j + 2) * C].rearrange("p (b c) -> p b c", b=2),
                        in_=xtok[j * 128:(j + 2) * 128, :].rearrange("(b r) c -> r b c", b=2),
                    ).then_inc(xk[pj % 8], 16)
                else:
                    sync.dma_start(out=xtk[:64, bass.ts(24, C)],
                                   in_=xtok[24 * 128:NPIX, :]).then_inc(xk[12 % 8], 16)

            # xtok pair-loads interleaved with out stores: pairs arrive two
            # blocks ahead of the tiles that read them
            seen_pairs = set()

            def pairs_for(b):
                want = sorted({i // 2 for i in TILES_BY_BLOCK[b]} - seen_pairs)
                for pj in want:
                    seen_pairs.add(pj)
                    load_pair(pj)

            pairs_for(0)
            pairs_for(1)
            done_b = -1
            for kind, x in PE_SCHED:
                if kind != "m2":
                    continue
                i = x
                b = _tile_block(i)
                if b > done_b:                  # prefetch two blocks ahead
                    done_b = b
                    if b + 2 < 8:
                        pairs_for(b + 2)
                rows = min(128, NPIX - i * 128)
                sync.wait_ge(s_dve, dve_pos[("m2e" if M2E_DVE[i] else "m2o", i)])
                sync.dma_start(out=out[i * 128:i * 128 + rows, :],
                               in_=ost[:rows, bass.ts(i % 8, C)]
                               ).then_inc(ot[i % 8], 16)
            if debug:
                sync.wait_ge(s_dbg, 48)

        # ================= Pool: halo/guard memsets, then m2-evict share ======
        @block.gpsimd
        def _(gpsimd):
            gpsimd.memset(warm[:], 0.0).then_inc(s_pl, 1)   # PE warmup source
            ms = [
                # gp halos: row 0 of each plane + cols 0/15 of the 14 data rows
                bass.AP(gp, 0, [[GP_EXT, 128], [NPL, 48], [1, 16]]),
                bass.AP(gp, 16, [[GP_EXT, 128], [NPL, 48], [16, 14]]),
                bass.AP(gp, 31, [[GP_EXT, 128], [NPL, 48], [16, 14]]),
                # (trailing halo row of each (g,c) row is the next row's
                # plane-0 row-0, zeroed above; diff output inherits zeros)
                # dfp: front/back OOB guards + plane-0 of each (g,c) row
                bass.AP(dfp, 0, [[DF_EXT, 128], [1, GF]]),
                bass.AP(dfp, GF + GPAD, [[DF_EXT, 128], [1, GB]]),
                bass.AP(dfp, GF, [[DF_EXT, 128], [GROW, 6], [1, NPL]]),
                # h1p: front/back guards, t-guard planes (tpad 0/9), halos
                bass.AP(h1p, 0, [[H1_EXT, 128], [1, GF]]),
                bass.AP(h1p, GF + H1PAD, [[H1_EXT, 128], [1, GB]]),
                bass.AP(h1p, GF, [[H1_EXT, 128], [H1ROW, 6], [1, NPL]]),
                bass.AP(h1p, GF + 9 * NPL, [[H1_EXT, 128], [H1ROW, 6], [1, NPL]]),
                bass.AP(h1p, GF, [[H1_EXT, 128], [NPL, 60], [1, 16]]),
                bass.AP(h1p, GF + 16, [[H1_EXT, 128], [NPL, 60], [16, 14]]),
                bass.AP(h1p, GF + 31, [[H1_EXT, 128], [NPL, 60], [16, 14]]),
            ]
            assert len(ms) == N_PL_MS - 1, len(ms)   # +1 warm memset
            for m in ms:
                gpsimd.memset(m, 0.0).then_inc(s_pl, 1)
            # frame-diff share: each waits the gp evicts it reads (t planes
            # of (g,c) are complete once band 3 (c=0) / 7 (c=1) evict g lands)
            for (g, c), eng in DIFF_ENG.items():
                if eng != "pl":
                    continue
                gpsimd.wait_ge(s_dve, dve_pos[("ev", 3 + g, 4 * c + 3)])
                a = g_plane(g, c, 1)
                b = g_plane(g, c, 0)
                d = df_plane(g, c, 1)
                gpsimd.tensor_tensor(
                    dfp[:, d:d + 7 * NPL],
                    gp[:, a:a + 7 * NPL], gp[:, b:b + 7 * NPL],
                    op=AOT.subtract).then_inc(s_pl, 1)

        # ================= PE =================
        @block.tensor
        def _(tensor):
            # p-state warmup: ~4.3us of throwaway matmuls during the initial
            # DMA wait so real work starts at full clock
            tensor.wait_ge(s_pl, 1)
            for _ in range(9):
                tensor.matmul(ps[:, 0:512],
                              bass.AP(warm, 0, [[512, 128], [1, 128]]),
                              bass.AP(warm, 0, [[512, 128], [1, 512]]),
                              start=True, stop=True, skip_group_check=True)
            tensor.wait_ge(s_w1a, 16)
            # ---- matmul1 (DR), j-major, banks 0..7 rotating ----
            for q, (j, mi) in enumerate(M1_CHUNKS):
                bank = q % 8
                if q == 1:
                    tensor.wait_ge(s_w1b, 16)
                if j in (2, 4, 6) and mi == 0:
                    tensor.wait_ge(s_xt[j // 2], 16)
                if q >= 8:
                    sem, thr = m1_evict_wait(q - 8)
                    tensor.wait_ge(s_dve if sem == "dve" else s_act, thr)
                pv = ps[:, bank * 512: bank * 512 + M1_CH]
                HD_EXT = KC * 128 + KC * 784
                for pr in range(KC // 2):
                    if mi == 0:
                        lhsT = sv3(head_sb, HD_EXT, pr * 256, [(128, 2), (1, 128)])
                    else:
                        lhsT = sv3(w1_sb, W_EXT, pr * 2 * C + mi * 128,
                                   [(C, 2), (1, 128)])
                    if j < 2:
                        rhs = sv3(head_sb, HD_EXT,
                                  KC * 128 + (pr * 2) * 784 + j * M1_CH,
                                  [(784, 2), (1, M1_CH)])
                    else:
                        rhs = sv3(xT_sb, XT_EXT, (pr * 2) * NPIX + j * M1_CH,
                                  [(NPIX, 2), (1, M1_CH)])
                    mm = tensor.matmul(pv, lhsT, rhs, perf_mode=DR,
                                       start=(pr == 0), stop=(pr == KC // 2 - 1))
                mm.then_inc(s_pe, 1)
            # ---- conv (banks 0..3) interleaved with m2 tiles (banks 4..7) ----
            tensor.wait_ge(s_dgo, 16)
            tensor.wait_ge(s_pl, N_PL_MS)
            dgm_seen = 0
            w2_seen = False
            for kind, x in PE_SCHED:
                if kind == "cv":
                    qc = x
                    br, g, c, tc = CONV_CHUNKS[qc]
                    bank = qc % 4
                    if br and 16 * (g + 1) > dgm_seen:
                        dgm_seen = 16 * (g + 1)
                        tensor.wait_ge(s_dgm, dgm_seen)
                    if qc < 4:
                        sem, thr = m1_evict_wait(40 + qc)
                        tensor.wait_ge(s_dve if sem == "dve" else s_act, thr)
                    else:
                        tensor.wait_ge(s_act, act_pos[("cv", qc - 4)])
                    if br:
                        tensor.wait_ge(s_act, main_data_thr(g, c, tc))
                    else:
                        sem, thr = diff_wait(g, c)
                        tensor.wait_ge(s_dve if sem == "dve" else s_pl, thr)
                    pairs = MAIN_PAIRS if br else OFF_PAIRS
                    pv = ps[:, bank * 512: bank * 512 + 480]
                    for ip, (tA, tB) in enumerate(pairs):
                        dtA, dhA, dwA = tA
                        if br:
                            offA = h1_plane(g, c, 2 * tc + 1 + dtA) + dhA * 16 + dwA
                            buf, ext = h1p, H1_EXT
                        else:
                            offA = df_plane(g, c, 2 * tc + dtA) + dhA * 16 + dwA
                            buf, ext = dfp, DF_EXT
                        if tB is None:
                            sstep = 16
                        else:
                            dtB, dhB, dwB = tB
                            sstep = (dtB - dtA) * NPL + (dhB - dhA) * 16
                        plane = (g * NPR_MAIN + ip) if br else (NPR_MAIN * NG + ip * NG + g)
                        lhsT = sv3(diag_sb, DG_EXT, plane * 256,
                                   [(128, 2), (1, 128)])
                        rhs = sv3(buf, ext, offA, [(sstep, 2), (1, 480)])
                        mm = tensor.matmul(pv, lhsT, rhs, perf_mode=DR,
                                           start=(ip == 0), stop=(ip == len(pairs) - 1),
                                           skip_group_check=True)
                    mm.then_inc(s_pe, 1)
                else:
                    i = x
                    if not w2_seen:
                        tensor.wait_ge(s_w2, 16)
                        w2_seen = True
                    rows = min(128, NPIX - i * 128)
                    odd = i % 2 == 1
                    tensor.wait_ge(s_act, M2_THR[i])
                    if i in TILE_BW:
                        sem, key = TILE_BW[i]
                        if sem == "act":
                            tensor.wait_ge(s_act, act_pos[key])
                        else:
                            tensor.wait_ge(s_dve, dve_pos[key])
                    elif i < 2:
                        for qq in (44 + 2 * i, 45 + 2 * i):
                            sem, thr = m1_evict_wait(qq)
                            tensor.wait_ge(s_dve if sem == "dve" else s_act, thr)
                    elif M2E_DVE[i - 2]:
                        tensor.wait_ge(s_dve, dve_pos[("m2e", i - 2)])
                    else:
                        tensor.wait_ge(s_act, act_pos[("m2e", i - 2)])
                    pv = ps[:rows, PB[i]:PB[i] + 768]
                    for pr in range(KC // 2):
                        last = pr == KC // 2 - 1
                        lhsT = sv3(cvo, CV_EXT, (pr * 2) * NPIX + i * 128,
                                   [(NPIX, 2), (1, rows)])
                        tensor.matmul(pv[:, 0:512], lhsT,
                                      sv3(w2_sb, W_EXT, pr * 2 * C, [(C, 2), (1, 512)]),
                                      perf_mode=DR,
                                      start=(pr == 0), stop=last,
                                      skip_group_check=True)
                        mm1 = tensor.matmul(pv[:, 512:768], lhsT,
                                            sv3(w2_sb, W_EXT, pr * 2 * C + 512,
                                                [(C, 2), (1, 256)]),
                                            perf_mode=DR,
                                            start=(pr == 0), stop=last,
                                            skip_group_check=True)
                    mm1.then_inc(s_pe, 1)

        # ================= ACT: main m1 evicts + all conv evicts =============
        @block.scalar
        def _(scalar):
            scalar.wait_ge(s_ld, 32)
            for j in range(8):
                c, t0 = divmod(2 * j, T)
                for m in range(3):
                    q = CHUNK_IDX[(j, M_POS[m])]   # main m's chunk in band j
                    bank = q % 8
                    scalar.wait_ge(s_pe, q + 1)
                    src = sv3(ps, 4096, bank * 512, [(196, 2), (14, 14), (1, 14)])
                    dst = sv3(h1p, H1_EXT,
                              h1_plane(m, c, t0 + 1) + 17,
                              [(NPL, 2), (16, 14), (1, 14)])
                    scalar.activation(dst, src, AFT.Identity,
                                      bias=b1_sb[:, m:m + 1]).then_inc(s_act, 1)
            for kind, x in PE_SCHED:
                if kind == "cv":
                    qc = x
                    br, g, c, tc = CONV_CHUNKS[qc]
                    bank = qc % 4
                    scalar.wait_ge(s_pe, pe_pos[("cv", qc)])
                    grp = g if br else 3 + g
                    src = sv3(ps, 4096, bank * 512 + 17,
                              [(NPL, 2), (16, 14), (1, 14)])
                    dst = sv3(cvo, CV_EXT,
                              grp * NPIX + c * NPIX_CLIP + 2 * tc * 196,
                              [(196, 2), (14, 14), (1, 14)])
                    scalar.activation(dst, src, AFT.Identity,
                                      bias=cb_sb[:, grp:grp + 1]).then_inc(s_act, 1)
                elif not M2E_DVE[x]:
                    # odd m2-tile psum evict: scaled copy to bf16 staging
                    # (DVE adds the residual at 2x)
                    i = x
                    rows = min(128, NPIX - i * 128)
                    scalar.wait_ge(s_pe, pe_pos[("m2", i)])
                    if i >= 4:
                        scalar.wait_ge(s_dve, dve_pos[("m2o", i - 4)])
                    scalar.activation(
                        tmp[:rows, bass.ts((i // 2) % 2, C)],
                        ps[:rows, PB[i]:PB[i] + 768],
                        AFT.Identity, scale=1.0 / (CVS ** 3)).then_inc(s_act, 1)

        # ================= DVE: off m1 evicts + diffs + m2-evict share =======
        @block.vector
        def _(vector):
            for j in range(8):
                c, t0 = divmod(2 * j, T)
                for g in range(NG):
                    q = CHUNK_IDX[(j, g)]      # chunk (j, mi=g) => m=3+g
                    bank = q % 8
                    vector.wait_ge(s_pe, q + 1)
                    src = sv3(ps, 4096, bank * 512, [(196, 2), (14, 14), (1, 14)])
                    dst = sv3(gp, GP_EXT, g_plane(g, c, t0) + 17,
                              [(NPL, 2), (16, 14), (1, 14)])
                    vector.tensor_copy(dst, src).then_inc(s_dve, 1)
            for (g, c), eng in DIFF_ENG.items():      # DVE's diff share
                if eng != "dve":
                    continue
                vector.wait_ge(s_pl, 4)               # gp halos zeroed
                a = g_plane(g, c, 1)
                b = g_plane(g, c, 0)
                d = df_plane(g, c, 1)
                vector.tensor_tensor(
                    dfp[:, d:d + 7 * NPL],
                    gp[:, a:a + 7 * NPL], gp[:, b:b + 7 * NPL],
                    op=AOT.subtract).then_inc(s_dve, 1)
            # ---- m2 evict + residual: even tiles stt from psum, odd tiles
            # bf16 add of ACT's scaled copy (2x DVE rate) ----
            for kind, x in PE_SCHED:
                if kind != "m2":
                    continue
                i = x
                rows = min(128, NPIX - i * 128)
                pj = i // 2
                vector.wait_ge(xk[pj % 8], 16 * (pj // 8 + 1))
                if i >= 8:
                    jj = i - 8
                    vector.wait_ge(ot[jj % 8], 16 * (jj // 8 + 1))
                if M2E_DVE[i]:
                    vector.wait_ge(s_pe, pe_pos[("m2", i)])
                    vector.scalar_tensor_tensor(
                        ost[:rows, bass.ts(i % 8, C)],
                        ps[:rows, PB[i]:PB[i] + 768],
                        1.0 / (CVS ** 3),
                        xtk[:rows, bass.ts(i, C)],
                        op0=AOT.mult, op1=AOT.add).then_inc(s_dve, 1)
                else:
                    vector.wait_ge(s_act, act_pos[("m2e", i)])
                    vector.tensor_tensor(
                        ost[:rows, bass.ts(i % 8, C)],
                        tmp[:rows, bass.ts((i // 2) % 2, C)],
                        xtk[:rows, bass.ts(i, C)],
                        op=AOT.add).then_inc(s_dve, 1)

    return nc


# ---------------- host side ----------------
_NC_CACHE = {}


def _get_nc():
    if "nc" not in _NC_CACHE:
        _NC_CACHE["nc"] = build()
    return _NC_CACHE["nc"]


def _dr_pack(W):
    """[768(k), M] -> per-partition DR layout [128(ki), pair, s, M] flattened."""
    M = W.shape[1]
    out = np.zeros((128, KC // 2, 2, M), np.float32)
    for pr in range(KC // 2):
        for s in range(2):
            out[:, pr, s, :] = W[pr * 256 + s * 128: pr * 256 + (s + 1) * 128, :]
    return out.reshape(128, KC // 2 * 2 * M)


def _prep_weights(w1, b1, cw, cb, w2, b2, ow1, ob1, ocw, ocb, ow2, ob2):
    w1cat = np.hstack([w1, ow1]) * CVS          # [768, 768], col m-blocks
    perm = np.concatenate([np.arange(m * 128, (m + 1) * 128) for m in M_ORDER])
    w1c = _dr_pack(w1cat[:, perm]).astype(F8NP)
    w2c = _dr_pack(np.vstack([w2, ow2]) * CVS).astype(F8NP)
    # diag DR pairs: [128(ki), pr_tot, s, 128(m)] with diagonal per s
    diag = np.zeros((128, NPR_TOT, 2, 128), np.float32)
    eye = np.eye(128, dtype=bool)

    def tapw(w_, tp, main):
        dt, dh, dw = tp
        if main:
            return w_[:, 0, dt + 1, dh + 1, dw + 1]
        return w_[:, 0, 0, dh + 1, dw + 1]

    for br, (pairs, w_) in enumerate([(MAIN_PAIRS, cw), (OFF_PAIRS, ocw)]):
        for ip, (tA, tB) in enumerate(pairs):
            for g in range(NG):
                # main planes grouped by g (contiguous per-g DMA pieces)
                pi = (g * NPR_MAIN + ip) if br == 0 else (NPR_MAIN * NG + ip * NG + g)
                vA = tapw(w_, tA, br == 0) * CVS
                diag[:, pi, 0, :][eye] = vA[g * 128:(g + 1) * 128]
                if tB is not None:
                    vB = tapw(w_, tB, br == 0) * CVS
                    diag[:, pi, 1, :][eye] = vB[g * 128:(g + 1) * 128]
    b1cv = np.ascontiguousarray(
        (np.concatenate([b1, ob1]) * CVS).reshape(KC, 128).T).astype(np.float32)
    # off-branch ob1 folded through the (linear) dwconv into its evict bias:
    # conv(diff + ob1) = conv(diff) + ob1 * sum(ocw taps)
    ocb_f = ocb + ob1 * ocw.sum(axis=(1, 2, 3, 4))
    cbcv = np.ascontiguousarray(
        (np.concatenate([cb, ocb_f]) * CVS * CVS).reshape(KC, 128).T).astype(np.float32)
    bias2 = (b2 + ob2).astype(np.float32)
    return dict(w1c=w1c, w2c=w2c,
                diag=diag.reshape(128, NPR_TOT * 2 * 128).astype(F8NP),
                b1c=b1cv, cbc=cbcv), bias2


def kernel(**inputs):
    x = np.asarray(inputs["x"], dtype=np.float32)
    Tv = int(np.asarray(inputs["T"]))
    assert Tv == T and x.shape == (128, 197, C)
    wd, bias2 = _prep_weights(
        *[np.asarray(inputs[k], dtype=np.float32) for k in
          ("w1", "b1", "cw", "cb", "w2", "b2", "ow1", "ob1", "ocw", "ocb", "ow2", "ob2")])

    in_maps = [_core_map(wd, bias2, x[core * 16:(core + 1) * 16])
               for core in range(8)]

    nc = _get_nc()
    res = run_bass_kernel_spmd(nc, in_maps, core_ids=list(range(8)))

    full = np.empty((128, 197, C), np.float32)
    for core in range(8):
        o = np.asarray(res.results[core]["out"]).astype(np.float32)
        full[core * 16:(core + 1) * 16, 0, :] = o[NPIX:NPIX + 16]
        full[core * 16:(core + 1) * 16, 1:, :] = o[:NPIX].reshape(16, 196, C)
    return full


# revision 85
# speedup vs baseline: 1.1239x; 1.0063x over previous
"""TRN2 Bass kernel for nn_Adapter (dense_cnn): ViT adapter with two branches
  main:   h1 = xs@w1+b1 ; y = dwconv3d_3x3x3(h1)+cb ; y@w2+b2
  offset: g = xs@ow1    ; d = tdiff(g) ; oc = dwconv_1x3x3(d)+bias' ; oc@ow2
  out = x with patch tokens += main + offset   (CLS rows pass through)

Data-parallel over 8 NeuronCores: 2 clips (16 frames) per core; adapter
weights replicated. Per-core kernel (raw bass, explicit semaphores):
  - fp8-e4m3 DoubleRow matmuls for m1 / depthwise conv (diagonal lhsT) / m2
  - 240-stride padded planes (15 rows x 16 cols); adjacent planes share the
    zero halo row, saving 1/16 of all PE conv streaming
  - m1 is j-major so PE starts on the first 512 columns of x^T; w1/diag are
    DMA'd in need-order pieces
  - evict work is spread across three engines: DVE evicts off-branch m1
    planes then computes wide per-(g,c) frame diffs; ACT evicts main m1
    planes + all conv outputs (two planes per op); m2 evict+residual pairs
    alternate DVE / GPSIMD
  - GPSIMD also zero-fills every halo/guard via memsets (no zeros DMA)
  - weights scaled x16/stage to keep fp8 out of subnormals; the final evict
    multiplies by 1/16^3 and adds the bf16 residual tokens
  - bf16 token stream + bf16 output (cast to f32 on host)

Self-contained: hardcodes shapes for x:[128,197,768], T=8 (asserts).
"""
import numpy as np
import ml_dtypes

import concourse.bass as bass
import concourse.mybir as mybir
from concourse.bass_utils import run_bass_kernel_spmd

F32 = mybir.dt.float32
BF16 = mybir.dt.bfloat16
F8 = mybir.dt.float8e4
AOT = mybir.AluOpType
AFT = mybir.ActivationFunctionType
DR = mybir.MatmulPerfMode.DoubleRow
BF = ml_dtypes.bfloat16
F8NP = ml_dtypes.float8_e4m3

# ---- problem constants (per core) ----
C = 768
CA = 384
T = 8
NPL = 240                 # padded plane stride: 15 rows x 16 cols
CLIPS = 2
NPIX_CLIP = T * 14 * 14
NPIX = CLIPS * NPIX_CLIP
KC = C // 128
NG = CA // 128
GROW = T * NPL            # one (g,c) row of unpadded-t planes (gp/diffp)
H1ROW = (T + 2) * NPL     # one (g,c) row incl t-guard planes (h1p)
H1PAD = NG * CLIPS * H1ROW
GPAD = NG * CLIPS * GROW
GF, GB = 32, 304          # front/back OOB guards for conv rhs over-reads
NTIL2 = (NPIX + 127) // 128
M1_CH = 392
OUT_ROWS = NPIX + 16
CVS = 16.0   # weight up-scale per stage; /CVS**3 folded into final evict

M_ORDER = [3, 4, 5, 0, 1, 2]          # m-block order (offs first); w1c is
                                      # stored column-permuted to match

# tap (dt, dh, dw) lists grouped by dw so DR pairs share dw (step % 16 == 0)
def _pairs(taps):
    by_dw = {}
    for tp in taps:
        by_dw.setdefault(tp[2], []).append(tp)
    prs = []
    for dw in sorted(by_dw):
        grp = by_dw[dw]
        for i in range(0, len(grp) - 1, 2):
            prs.append((grp[i], grp[i + 1]))
        if len(grp) % 2:
            prs.append((grp[-1], None))
    return prs

MAIN_TAPS = [(kd - 1, kh - 1, kw - 1)
             for kd in range(3) for kh in range(3) for kw in range(3)]
OFF_TAPS = [(0, kh - 1, kw - 1) for kh in range(3) for kw in range(3)]
MAIN_PAIRS = _pairs(MAIN_TAPS)   # 15 (12 pairs + 3 singles)
OFF_PAIRS = _pairs(OFF_TAPS)     # 6 (3 pairs + 3 singles)
NPR_MAIN = len(MAIN_PAIRS)
NPR_OFF = len(OFF_PAIRS)
NPR_TOT = (NPR_MAIN + NPR_OFF) * NG   # 63

# j-major; band 7 runs mains first so DVE's last off-evicts aren't needed
# by the earliest conv chunks' psum-bank reuse
M1_CHUNKS = ([(j, mi) for j in range(7) for mi in range(6)]
             + [(7, mi) for mi in (3, 4, 5, 0, 1, 2)])
CHUNK_IDX = {ch: q for q, ch in enumerate(M1_CHUNKS)}
M_POS = {m: mi for mi, m in enumerate(M_ORDER)}
CONV_CHUNKS = [(br, g, c, tc)
               for c in range(2) for tc in range(4)
               for br in (1, 0) for g in range(NG)]    # mains first per block
# diff(g,c) -> engine: Pool does most (it idles after memsets); DVE keeps
# (2,0) appended after its evict stream
DIFF_ENG = {(0, 0): "pl", (1, 0): "pl", (2, 0): "dve",
            (0, 1): "pl", (1, 1): "pl", (2, 1): "pl"}
N_M1 = len(M1_CHUNKS)      # 48
N_CONV = len(CONV_CHUNKS)  # 48

# m2 evict tile -> engine: even tiles DVE (stt w/ residual); odd tiles get
# the residual via a 4096*I bf16 matmul on PE, then a pure scaled ACT copy.
M2E_DVE = {i: i % 2 == 0 for i in range(25)}

# m2 tile i is unlocked once conv block (c,tc) covering its last token is
# evicted; PE interleaves each block's conv chunks with its unlocked tiles
def _tile_block(i):
    p_hi = (min(128 * (i + 1), NPIX) - 1) // 196
    c_hi, t_hi = divmod(p_hi, T)
    return c_hi * 4 + t_hi // 2

TILES_BY_BLOCK = [[] for _ in range(8)]
for _i in range(NTIL2):
    TILES_BY_BLOCK[_tile_block(_i)].append(_i)

# PE order: conv block b+1 runs while ACT evicts block b; block b's tiles
# are spread between block b+1's chunks (after chunks 1/3/5) so the 2-slot
# m2 psum rotation never outruns the evict engines
PE_SCHED = []
for _b in range(8):
    _tiles = TILES_BY_BLOCK[_b - 1] if _b >= 1 else []
    for _k in range(6):
        PE_SCHED.append(("cv", _b * 6 + _k))
        if _k % 2 == 1 and _tiles:
            PE_SCHED.append(("m2", _tiles.pop(0)))
    PE_SCHED += [("m2", _i) for _i in _tiles]
PE_SCHED += [("m2", _i) for _i in TILES_BY_BLOCK[7]]
TILES_BY_BLOCK = [[] for _ in range(8)]          # rebuild (popped above)
for _i in range(NTIL2):
    TILES_BY_BLOCK[_tile_block(_i)].append(_i)


def build(debug=False):
    nc = bass.Bass()
    xT = nc.declare_dram_parameter("xT", [C, NPIX], F8, isOutput=False)
    xtok = nc.declare_dram_parameter("xtok", [NPIX, C], BF16, isOutput=False)
    xcls = nc.declare_dram_parameter("xcls", [16, C], BF16, isOutput=False)
    head = nc.declare_dram_parameter("head", [128, KC * 128 + KC * 784], F8, isOutput=False)
    w1c = nc.declare_dram_parameter("w1c", [128, KC // 2 * 2 * C], F8, isOutput=False)
    w2c = nc.declare_dram_parameter("w2c", [128, KC // 2 * 2 * C], F8, isOutput=False)
    diag = nc.declare_dram_parameter("diag", [128, NPR_TOT * 2 * 128], F8, isOutput=False)
    b1c = nc.declare_dram_parameter("b1c", [128, KC], F32, isOutput=False)
    cbc = nc.declare_dram_parameter("cbc", [128, KC], F32, isOutput=False)
    out = nc.declare_dram_parameter("out", [OUT_ROWS, C], BF16, isOutput=True)
    if debug:
        dbg_h1 = nc.declare_dram_parameter("dbg_h1", [128, GF + H1PAD + GB], F8, isOutput=True)
        dbg_df = nc.declare_dram_parameter("dbg_df", [128, GF + GPAD + GB], F8, isOutput=True)
        dbg_cv = nc.declare_dram_parameter("dbg_cv", [128, KC * NPIX], F8, isOutput=True)

    xT_sb = nc.alloc_sbuf_tensor([128, KC * NPIX], F8)
    head_sb = nc.alloc_sbuf_tensor([128, KC * 128 + KC * 784], F8)  # w1 mblk0 | xT q0
    w1_sb = nc.alloc_sbuf_tensor([128, KC // 2 * 2 * C], F8)   # [pair][s][mblk]
    w2_sb = nc.alloc_sbuf_tensor([128, KC // 2 * 2 * C], F8)
    diag_sb = nc.alloc_sbuf_tensor([128, NPR_TOT * 2 * 128], F8)  # [pr][s][m]
    b1_sb = nc.alloc_sbuf_tensor([128, KC], F32)
    cb_sb = nc.alloc_sbuf_tensor([128, KC], F32)
    h1p = nc.alloc_sbuf_tensor([128, GF + H1PAD + GB], F8)
    gp = nc.alloc_sbuf_tensor([128, GPAD], F8)
    dfp = nc.alloc_sbuf_tensor([128, GF + GPAD + GB], F8)
    cvo = nc.alloc_sbuf_tensor([128, KC * NPIX], F8)
    xtk = nc.alloc_sbuf_tensor([128, NTIL2 * C], BF16)
    ost = nc.alloc_sbuf_tensor([128, 8 * C], BF16)
    tmp = nc.alloc_sbuf_tensor([128, 2 * C], BF16)   # odd-tile scaled psum
    warm = nc.alloc_sbuf_tensor([128, 512], F8)
    ps = nc.alloc_psum_tensor([128, 4096], F32)

    def h1_plane(g, c, tpad):
        return GF + (g * CLIPS + c) * H1ROW + tpad * NPL

    def g_plane(g, c, t):
        return (g * CLIPS + c) * GROW + t * NPL

    def df_plane(g, c, t):
        return GF + (g * CLIPS + c) * GROW + t * NPL

    def sv3(buf, ext, offset, dims):
        """3D free view [part + dims] of an sbuf tensor via explicit AP."""
        return bass.AP(buf, offset, [[ext, 128]] + [list(d) for d in dims])

    XT_EXT = KC * NPIX
    W_EXT = KC // 2 * 2 * C
    DG_EXT = NPR_TOT * 2 * 128
    H1_EXT = GF + H1PAD + GB
    GP_EXT = GPAD
    DF_EXT = GF + GPAD + GB
    CV_EXT = KC * NPIX

    # ---------- static evict/producer schedules ----------
    # DVE program positions (1-based sem thresholds after inc):
    #   per j-band: 3 off-evict ops (g=0,1,2); after bands 3 and 7: 3 diffs.
    dve_pos = {}
    pos = 0
    for j in range(8):
        for g in range(NG):
            pos += 1
            dve_pos[("ev", 3 + g, j)] = pos     # evict of m1 chunk (m=3+g, j)
    for gc, eng in DIFF_ENG.items():
        if eng == "dve":
            pos += 1
            dve_pos[("diff",) + gc] = pos
    DVE_PRE = pos                               # evicts+diffs before m2 evicts
    for kind, x in PE_SCHED:
        if kind != "m2":
            continue
        pos += 1
        # even: stt evict from psum; odd: bf16 residual add from tmp
        dve_pos[("m2e", x) if M2E_DVE[x] else ("m2o", x)] = pos
    N_PL_MS = 13                                # memset count on Pool (below)
    pl_pos = {}
    pos = N_PL_MS
    for gc, eng in DIFF_ENG.items():
        if eng == "pl":
            pos += 1
            pl_pos[("diff",) + gc] = pos

    def diff_wait(g, c):
        if DIFF_ENG[(g, c)] == "dve":
            return ("dve", dve_pos[("diff", g, c)])
        return ("pl", pl_pos[("diff", g, c)])

    # ACT program positions: 24 main m1 evicts (j-major, m inner), then the
    # PE_SCHED-ordered conv evicts + odd m2-tile evicts
    act_pos = {}
    pos = 0
    for j in range(8):
        for m in range(3):
            pos += 1
            act_pos[("ev", m, j)] = pos
    for kind, x in PE_SCHED:
        if kind == "cv":
            pos += 1
            act_pos[("cv", x)] = pos
        elif not M2E_DVE[x]:
            pos += 1
            act_pos[("m2e", x)] = pos

    def m1_evict_wait(q):
        """(sem_name, thr) for 'm1 chunk q's psum bank has been evicted'."""
        j, mi = M1_CHUNKS[q]
        m = M_ORDER[mi]
        if m >= 3:
            return ("dve", dve_pos[("ev", m, j)])
        return ("act", act_pos[("ev", m, j)])

    # PE completion positions on s_pe (interleaved conv blocks + m2 tiles)
    pe_pos = {}
    pos = N_M1
    for kind, x in PE_SCHED:
        pos += 1
        pe_pos[(kind, x)] = pos

    # m2 tile psum slots: {4,5}/{6,7} alternating; the last block's four
    # tiles fan out over all four bank-pairs (conv banks are free by then)
    PB = {i: (4 + 2 * (i % 2)) * 512 for i in range(NTIL2)}
    b7 = TILES_BY_BLOCK[7]
    PB[b7[0]], PB[b7[1]], PB[b7[2]], PB[b7[3]] = 0, 1024, 3072, 2048
    TILE_BW = {                     # custom bank-free waits for those tiles
        b7[0]: ("act", ("cv", 45)),
        b7[1]: ("act", ("cv", 47)),
        b7[2]: ("act", ("m2e", 19)),
        b7[3]: ("dve", ("m2e", 20)),
    }

    def main_data_thr(g, c, tc):
        """ACT threshold: h1 planes t<=2tc+1 of (g,c) evicted."""
        j = c * 4 + min(tc + 1, 3)
        return act_pos[("ev", g, j)]

    M2_THR = []
    for i in range(NTIL2):
        p_hi = (min(128 * (i + 1), NPIX) - 1) // 196
        c_hi, t_hi = divmod(p_hi, T)
        blocks = c_hi * 4 + t_hi // 2       # completed (c,tc) blocks before
        M2_THR.append(act_pos[("cv", blocks * 6 + 5)])

    from contextlib import ExitStack
    _sems = ExitStack()
    xk = [_sems.enter_context(nc.semaphore(f"s_xk{i}")) for i in range(8)]
    ot = [_sems.enter_context(nc.semaphore(f"s_ot{i}")) for i in range(8)]
    s_xt = [_sems.enter_context(nc.semaphore(f"s_xt{i}")) for i in range(4)]
    with (
        _sems,
        nc.Block(no_gpsimd_drain=True) as block,
        nc.semaphore("s_ld") as s_ld,
        nc.semaphore("s_w1a") as s_w1a,
        nc.semaphore("s_w1b") as s_w1b,
        nc.semaphore("s_dgo") as s_dgo,
        nc.semaphore("s_dgm") as s_dgm,
        nc.semaphore("s_w2") as s_w2,
        nc.semaphore("s_pe") as s_pe,
        nc.semaphore("s_act") as s_act,
        nc.semaphore("s_dve") as s_dve,
        nc.semaphore("s_pl") as s_pl,
        nc.semaphore("s_cls") as s_cls,
        nc.semaphore("s_dbg") as s_dbg,
    ):
        # ================= SP: all DMA (serial, need-ordered) =================
        @block.sync
        def _(sync):
            w1v = w1_sb[:].rearrange("p (q m) -> p q m", q=KC)    # q=(pr,s)
            w1d = w1c[:].rearrange("p (q m) -> p q m", q=KC)
            sync.dma_start(out=head_sb[:], in_=head[:]).then_inc(s_w1a, 16)
            xtv = xT_sb[:].rearrange("p (k n) -> p k n", k=KC)
            xtd = xT[:].rearrange("(k p) n -> p k n", p=128)
            sync.dma_start(out=w1v[:, :, 128:768], in_=w1d[:, :, 128:768]
                           ).then_inc(s_w1b, 16)
            sync.dma_start(out=b1_sb[:], in_=b1c[:]).then_inc(s_ld, 16)
            sync.dma_start(out=cb_sb[:], in_=cbc[:]).then_inc(s_ld, 16)
            for qq in (1, 2, 3):
                sync.dma_start(out=xtv[:, :, qq * 784:(qq + 1) * 784],
                               in_=xtd[:, :, qq * 784:(qq + 1) * 784]
                               ).then_inc(s_xt[qq], 16)
            doff = NPR_MAIN * NG * 256
            sync.dma_start(out=diag_sb[:, doff:], in_=diag[:, doff:]
                           ).then_inc(s_dgo, 16)
            for g in range(NG):      # main diag, grouped by g (host layout)
                lo, hi = g * NPR_MAIN * 256, (g + 1) * NPR_MAIN * 256
                sync.dma_start(out=diag_sb[:, lo:hi], in_=diag[:, lo:hi]
                               ).then_inc(s_dgm, 16)
            sync.dma_start(out=w2_sb[:], in_=w2c[:]).then_inc(s_w2, 16)
            sync.dma_start(out=out[NPIX:OUT_ROWS, :], in_=xcls[:]).then_inc(s_cls, 16)
            if debug:
                sync.wait_ge(s_act, act_pos[("ev", 2, 7)])
                sync.wait_ge(s_dve, DVE_PRE)
                sync.dma_start(out=dbg_h1[:], in_=h1p[:]).then_inc(s_dbg, 16)
                sync.dma_start(out=dbg_df[:], in_=dfp[:]).then_inc(s_dbg, 16)
                sync.wait_ge(s_act, act_pos[("cv", N_CONV - 1)])
                sync.dma_start(out=dbg_cv[:], in_=cvo[:]).then_inc(s_dbg, 16)

            def load_pair(pj):
                if pj < 12:
                    j = 2 * pj
                    sync.dma_start(
                        out=xtk[:, j * C:(j + 2) * C].rearrange("p (b c) -> p b c", b=2),
                        in_=xtok[j * 128:(j + 2) * 128, :].rearrange("(b r) c -> r b c", b=2),
                    ).then_inc(xk[pj % 8], 16)
                else:
                    sync.dma_start(out=xtk[:64, bass.ts(24, C)],
                                   in_=xtok[24 * 128:NPIX, :]).then_inc(xk[12 % 8], 16)

            # xtok pair-loads interleaved with out stores: pairs arrive two
            # blocks ahead of the tiles that read them
            seen_pairs = set()

            def pairs_for(b):
                want = sorted({i // 2 for i in TILES_BY_BLOCK[b]} - seen_pairs)
                for pj in want:
                    seen_pairs.add(pj)
                    load_pair(pj)

            pairs_for(0)
            pairs_for(1)
            done_b = -1
            for kind, x in PE_SCHED:
                if kind != "m2":
                    continue
                i = x
                b = _tile_block(i)
                if b > done_b:                  # prefetch two blocks ahead
                    done_b = b
                    if b + 2 < 8:
                        pairs_for(b + 2)
                rows = min(128, NPIX - i * 128)
                sync.wait_ge(s_dve, dve_pos[("m2e" if M2E_DVE[i] else "m2o", i)])
                sync.dma_start(out=out[i * 128:i * 128 + rows, :],
                               in_=ost[:rows, bass.ts(i % 8, C)]
                               ).then_inc(ot[i % 8], 16)
            if debug:
                sync.wait_ge(s_dbg, 48)

        # ================= Pool: halo/guard memsets, then m2-evict share ======
        @block.gpsimd
        def _(gpsimd):
            ms = [
                # gp halos: row 0 of each plane + cols 0/15 of the 14 data rows
                bass.AP(gp, 0, [[GP_EXT, 128], [NPL, 48], [1, 16]]),
                bass.AP(gp, 16, [[GP_EXT, 128], [NPL, 48], [16, 14]]),
                bass.AP(gp, 31, [[GP_EXT, 128], [NPL, 48], [16, 14]]),
                # (trailing halo row of each (g,c) row is the next row's
                # plane-0 row-0, zeroed above; diff output inherits zeros)
                # dfp: front/back OOB guards + plane-0 of each (g,c) row
                bass.AP(dfp, 0, [[DF_EXT, 128], [1, GF]]),
                bass.AP(dfp, GF + GPAD, [[DF_EXT, 128], [1, GB]]),
                bass.AP(dfp, GF, [[DF_EXT, 128], [GROW, 6], [1, NPL]]),
                # h1p: front/back guards, t-guard planes (tpad 0/9), halos
                bass.AP(h1p, 0, [[H1_EXT, 128], [1, GF]]),
                bass.AP(h1p, GF + H1PAD, [[H1_EXT, 128], [1, GB]]),
                bass.AP(h1p, GF, [[H1_EXT, 128], [H1ROW, 6], [1, NPL]]),
                bass.AP(h1p, GF + 9 * NPL, [[H1_EXT, 128], [H1ROW, 6], [1, NPL]]),
                bass.AP(h1p, GF, [[H1_EXT, 128], [NPL, 60], [1, 16]]),
                bass.AP(h1p, GF + 16, [[H1_EXT, 128], [NPL, 60], [16, 14]]),
                bass.AP(h1p, GF + 31, [[H1_EXT, 128], [NPL, 60], [16, 14]]),
            ]
            assert len(ms) == N_PL_MS, len(ms)
            for m in ms:
                gpsimd.memset(m, 0.0).then_inc(s_pl, 1)
            # frame-diff share: each waits the gp evicts it reads (t planes
            # of (g,c) are complete once band 3 (c=0) / 7 (c=1) evict g lands)
            for (g, c), eng in DIFF_ENG.items():
                if eng != "pl":
                    continue
                gpsimd.wait_ge(s_dve, dve_pos[("ev", 3 + g, 4 * c + 3)])
                a = g_plane(g, c, 1)
                b = g_plane(g, c, 0)
                d = df_plane(g, c, 1)
                gpsimd.tensor_tensor(
                    dfp[:, d:d + 7 * NPL],
                    gp[:, a:a + 7 * NPL], gp[:, b:b + 7 * NPL],
                    op=AOT.subtract).then_inc(s_pl, 1)

        # ================= PE =================
        @block.tensor
        def _(tensor):
            # p-state warmup: throwaway matmuls during the initial DMA wait
            # so real work starts at full clock (warm is never written; the
            # interp zero-fills SBUF)
            for _ in range(11):
                tensor.matmul(ps[:, 0:512],
                              bass.AP(warm, 0, [[512, 128], [1, 128]]),
                              bass.AP(warm, 0, [[512, 128], [1, 512]]),
                              start=True, stop=True, skip_group_check=True)
            tensor.wait_ge(s_w1a, 16)
            # ---- matmul1 (DR), j-major, banks 0..7 rotating ----
            for q, (j, mi) in enumerate(M1_CHUNKS):
                bank = q % 8
                if q == 1:
                    tensor.wait_ge(s_w1b, 16)
                if j in (2, 4, 6) and mi == 0:
                    tensor.wait_ge(s_xt[j // 2], 16)
                if q >= 8:
                    sem, thr = m1_evict_wait(q - 8)
                    tensor.wait_ge(s_dve if sem == "dve" else s_act, thr)
                pv = ps[:, bank * 512: bank * 512 + M1_CH]
                HD_EXT = KC * 128 + KC * 784
                for pr in range(KC // 2):
                    if mi == 0:
                        lhsT = sv3(head_sb, HD_EXT, pr * 256, [(128, 2), (1, 128)])
                    else:
                        lhsT = sv3(w1_sb, W_EXT, pr * 2 * C + mi * 128,
                                   [(C, 2), (1, 128)])
                    if j < 2:
                        rhs = sv3(head_sb, HD_EXT,
                                  KC * 128 + (pr * 2) * 784 + j * M1_CH,
                                  [(784, 2), (1, M1_CH)])
                    else:
                        rhs = sv3(xT_sb, XT_EXT, (pr * 2) * NPIX + j * M1_CH,
                                  [(NPIX, 2), (1, M1_CH)])
                    mm = tensor.matmul(pv, lhsT, rhs, perf_mode=DR,
                                       start=(pr == 0), stop=(pr == KC // 2 - 1))
                mm.then_inc(s_pe, 1)
            # ---- conv (banks 0..3) interleaved with m2 tiles (banks 4..7) ----
            tensor.wait_ge(s_dgo, 16)
            tensor.wait_ge(s_pl, N_PL_MS)
            dgm_seen = 0
            w2_seen = False
            for kind, x in PE_SCHED:
                if kind == "cv":
                    qc = x
                    br, g, c, tc = CONV_CHUNKS[qc]
                    bank = qc % 4
                    if br and 16 * (g + 1) > dgm_seen:
                        dgm_seen = 16 * (g + 1)
                        tensor.wait_ge(s_dgm, dgm_seen)
                    if qc < 4:
                        sem, thr = m1_evict_wait(40 + qc)
                        tensor.wait_ge(s_dve if sem == "dve" else s_act, thr)
                    else:
                        tensor.wait_ge(s_act, act_pos[("cv", qc - 4)])
                    if br:
                        tensor.wait_ge(s_act, main_data_thr(g, c, tc))
                    else:
                        sem, thr = diff_wait(g, c)
                        tensor.wait_ge(s_dve if sem == "dve" else s_pl, thr)
                    pairs = MAIN_PAIRS if br else OFF_PAIRS
                    pv = ps[:, bank * 512: bank * 512 + 480]
                    for ip, (tA, tB) in enumerate(pairs):
                        dtA, dhA, dwA = tA
                        if br:
                            offA = h1_plane(g, c, 2 * tc + 1 + dtA) + dhA * 16 + dwA
                            buf, ext = h1p, H1_EXT
                        else:
                            offA = df_plane(g, c, 2 * tc + dtA) + dhA * 16 + dwA
                            buf, ext = dfp, DF_EXT
                        if tB is None:
                            sstep = 16
                        else:
                            dtB, dhB, dwB = tB
                            sstep = (dtB - dtA) * NPL + (dhB - dhA) * 16
                        plane = (g * NPR_MAIN + ip) if br else (NPR_MAIN * NG + ip * NG + g)
                        lhsT = sv3(diag_sb, DG_EXT, plane * 256,
                                   [(128, 2), (1, 128)])
                        rhs = sv3(buf, ext, offA, [(sstep, 2), (1, 480)])
                        mm = tensor.matmul(pv, lhsT, rhs, perf_mode=DR,
                                           start=(ip == 0), stop=(ip == len(pairs) - 1),
                                           skip_group_check=True)
                    mm.then_inc(s_pe, 1)
                else:
                    i = x
                    if not w2_seen:
                        tensor.wait_ge(s_w2, 16)
                        w2_seen = True
                    rows = min(128, NPIX - i * 128)
                    odd = i % 2 == 1
                    tensor.wait_ge(s_act, M2_THR[i])
                    if i in TILE_BW:
                        sem, key = TILE_BW[i]
                        if sem == "act":
                            tensor.wait_ge(s_act, act_pos[key])
                        else:
                            tensor.wait_ge(s_dve, dve_pos[key])
                    elif i < 2:
                        for qq in (44 + 2 * i, 45 + 2 * i):
                            sem, thr = m1_evict_wait(qq)
                            tensor.wait_ge(s_dve if sem == "dve" else s_act, thr)
                    elif M2E_DVE[i - 2]:
                        tensor.wait_ge(s_dve, dve_pos[("m2e", i - 2)])
                    else:
                        tensor.wait_ge(s_act, act_pos[("m2e", i - 2)])
                    pv = ps[:rows, PB[i]:PB[i] + 768]
                    for pr in range(KC // 2):
                        last = pr == KC // 2 - 1
                        lhsT = sv3(cvo, CV_EXT, (pr * 2) * NPIX + i * 128,
                                   [(NPIX, 2), (1, rows)])
                        tensor.matmul(pv[:, 0:512], lhsT,
                                      sv3(w2_sb, W_EXT, pr * 2 * C, [(C, 2), (1, 512)]),
                                      perf_mode=DR,
                                      start=(pr == 0), stop=last,
                                      skip_group_check=True)
                        mm1 = tensor.matmul(pv[:, 512:768], lhsT,
                                            sv3(w2_sb, W_EXT, pr * 2 * C + 512,
                                                [(C, 2), (1, 256)]),
                                            perf_mode=DR,
                                            start=(pr == 0), stop=last,
                                            skip_group_check=True)
                    mm1.then_inc(s_pe, 1)

        # ================= ACT: main m1 evicts + all conv evicts =============
        @block.scalar
        def _(scalar):
            scalar.wait_ge(s_ld, 32)
            for j in range(8):
                c, t0 = divmod(2 * j, T)
                for m in range(3):
                    q = CHUNK_IDX[(j, M_POS[m])]   # main m's chunk in band j
                    bank = q % 8
                    scalar.wait_ge(s_pe, q + 1)
                    src = sv3(ps, 4096, bank * 512, [(196, 2), (14, 14), (1, 14)])
                    dst = sv3(h1p, H1_EXT,
                              h1_plane(m, c, t0 + 1) + 17,
                              [(NPL, 2), (16, 14), (1, 14)])
                    scalar.activation(dst, src, AFT.Identity,
                                      bias=b1_sb[:, m:m + 1]).then_inc(s_act, 1)
            for kind, x in PE_SCHED:
                if kind == "cv":
                    qc = x
                    br, g, c, tc = CONV_CHUNKS[qc]
                    bank = qc % 4
                    scalar.wait_ge(s_pe, pe_pos[("cv", qc)])
                    grp = g if br else 3 + g
                    src = sv3(ps, 4096, bank * 512 + 17,
                              [(NPL, 2), (16, 14), (1, 14)])
                    dst = sv3(cvo, CV_EXT,
                              grp * NPIX + c * NPIX_CLIP + 2 * tc * 196,
                              [(196, 2), (14, 14), (1, 14)])
                    scalar.activation(dst, src, AFT.Identity,
                                      bias=cb_sb[:, grp:grp + 1]).then_inc(s_act, 1)
                elif not M2E_DVE[x]:
                    # odd m2-tile psum evict: scaled copy to bf16 staging
                    # (DVE adds the residual at 2x)
                    i = x
                    rows = min(128, NPIX - i * 128)
                    scalar.wait_ge(s_pe, pe_pos[("m2", i)])
                    if i >= 4:
                        scalar.wait_ge(s_dve, dve_pos[("m2o", i - 4)])
                    scalar.activation(
                        tmp[:rows, bass.ts((i // 2) % 2, C)],
                        ps[:rows, PB[i]:PB[i] + 768],
                        AFT.Identity, scale=1.0 / (CVS ** 3)).then_inc(s_act, 1)

        # ================= DVE: off m1 evicts + diffs + m2-evict share =======
        @block.vector
        def _(vector):
            for j in range(8):
                c, t0 = divmod(2 * j, T)
                for g in range(NG):
                    q = CHUNK_IDX[(j, g)]      # chunk (j, mi=g) => m=3+g
                    bank = q % 8
                    vector.wait_ge(s_pe, q + 1)
                    src = sv3(ps, 4096, bank * 512, [(196, 2), (14, 14), (1, 14)])
                    dst = sv3(gp, GP_EXT, g_plane(g, c, t0) + 17,
                              [(NPL, 2), (16, 14), (1, 14)])
                    vector.tensor_copy(dst, src).then_inc(s_dve, 1)
            for (g, c), eng in DIFF_ENG.items():      # DVE's diff share
                if eng != "dve":
                    continue
                vector.wait_ge(s_pl, 3)               # gp halos zeroed
                a = g_plane(g, c, 1)
                b = g_plane(g, c, 0)
                d = df_plane(g, c, 1)
                vector.tensor_tensor(
                    dfp[:, d:d + 7 * NPL],
                    gp[:, a:a + 7 * NPL], gp[:, b:b + 7 * NPL],
                    op=AOT.subtract).then_inc(s_dve, 1)
            # ---- m2 evict + residual: even tiles stt from psum, odd tiles
            # bf16 add of ACT's scaled copy (2x DVE rate) ----
            for kind, x in PE_SCHED:
                if kind != "m2":
                    continue
                i = x
                rows = min(128, NPIX - i * 128)
                pj = i // 2
                vector.wait_ge(xk[pj % 8], 16 * (pj // 8 + 1))
                if i >= 8:
                    jj = i - 8
                    vector.wait_ge(ot[jj % 8], 16 * (jj // 8 + 1))
                if M2E_DVE[i]:
                    vector.wait_ge(s_pe, pe_pos[("m2", i)])
                    vector.scalar_tensor_tensor(
                        ost[:rows, bass.ts(i % 8, C)],
                        ps[:rows, PB[i]:PB[i] + 768],
                        1.0 / (CVS ** 3),
                        xtk[:rows, bass.ts(i, C)],
                        op0=AOT.mult, op1=AOT.add).then_inc(s_dve, 1)
                else:
                    vector.wait_ge(s_act, act_pos[("m2e", i)])
                    vector.tensor_tensor(
                        ost[:rows, bass.ts(i % 8, C)],
                        tmp[:rows, bass.ts((i // 2) % 2, C)],
                        xtk[:rows, bass.ts(i, C)],
                        op=AOT.add).then_inc(s_dve, 1)

    return nc


# ---------------- host side ----------------
_NC_CACHE = {}


def _get_nc():
    if "nc" not in _NC_CACHE:
        _NC_CACHE["nc"] = build()
    return _NC_CACHE["nc"]


def _dr_pack(W):
    """[768(k), M] -> per-partition DR layout [128(ki), pair, s, M] flattened."""
    M = W.shape[1]
    out = np.zeros((128, KC // 2, 2, M), np.float32)
    for pr in range(KC // 2):
        for s in range(2):
            out[:, pr, s, :] = W[pr * 256 + s * 128: pr * 256 + (s + 1) * 128, :]
    return out.reshape(128, KC // 2 * 2 * M)


def _prep_weights(w1, b1, cw, cb, w2, b2, ow1, ob1, ocw, ocb, ow2, ob2):
    w1cat = np.hstack([w1, ow1]) * CVS          # [768, 768], col m-blocks
    perm = np.concatenate([np.arange(m * 128, (m + 1) * 128) for m in M_ORDER])
    w1c = _dr_pack(w1cat[:, perm]).astype(F8NP)
    w2c = _dr_pack(np.vstack([w2, ow2]) * CVS).astype(F8NP)
    # diag DR pairs: [128(ki), pr_tot, s, 128(m)] with diagonal per s
    diag = np.zeros((128, NPR_TOT, 2, 128), np.float32)
    eye = np.eye(128, dtype=bool)

    def tapw(w_, tp, main):
        dt, dh, dw = tp
        if main:
            return w_[:, 0, dt + 1, dh + 1, dw + 1]
        return w_[:, 0, 0, dh + 1, dw + 1]

    for br, (pairs, w_) in enumerate([(MAIN_PAIRS, cw), (OFF_PAIRS, ocw)]):
        for ip, (tA, tB) in enumerate(pairs):
            for g in range(NG):
                # main planes grouped by g (contiguous per-g DMA pieces)
                pi = (g * NPR_MAIN + ip) if br == 0 else (NPR_MAIN * NG + ip * NG + g)
                vA = tapw(w_, tA, br == 0) * CVS
                diag[:, pi, 0, :][eye] = vA[g * 128:(g + 1) * 128]
                if tB is not None:
                    vB = tapw(w_, tB, br == 0) * CVS
                    diag[:, pi, 1, :][eye] = vB[g * 128:(g + 1) * 128]
    b1cv = np.ascontiguousarray(
        (np.concatenate([b1, ob1]) * CVS).reshape(KC, 128).T).astype(np.float32)
    # off-branch ob1 folded through the (linear) dwconv into its evict bias:
    # conv(diff + ob1) = conv(diff) + ob1 * sum(ocw taps)
    ocb_f = ocb + ob1 * ocw.sum(axis=(1, 2, 3, 4))
    cbcv = np.ascontiguousarray(
        (np.concatenate([cb, ocb_f]) * CVS * CVS).reshape(KC, 128).T).astype(np.float32)
    bias2 = (b2 + ob2).astype(np.float32)
    return dict(w1c=w1c, w2c=w2c,
                diag=diag.reshape(128, NPR_TOT * 2 * 128).astype(F8NP),
                b1c=b1cv, cbc=cbcv), bias2


def _core_map(wd, bias2, xs):
    """Per-core input map from shared weights + this core's 16 BT rows."""
    xpat = np.ascontiguousarray(xs[:, 1:, :]).reshape(NPIX, C)
    m = dict(wd)
    xT8 = np.ascontiguousarray(xpat.T).astype(F8NP)
    m["xT"] = xT8
    hd = np.empty((128, KC * 128 + KC * 784), F8NP)
    for q in range(KC):
        hd[:, q * 128:(q + 1) * 128] = wd["w1c"][:, q * 768:q * 768 + 128]
    for k in range(KC):
        hd[:, KC * 128 + k * 784: KC * 128 + (k + 1) * 784] = \
            xT8[k * 128:(k + 1) * 128, 0:784]
    m["head"] = hd
    m["xtok"] = (xpat + bias2).astype(BF)
    m["xcls"] = np.ascontiguousarray(xs[:, 0, :]).astype(BF)
    return m


def kernel(**inputs):
    x = np.asarray(inputs["x"], dtype=np.float32)
    Tv = int(np.asarray(inputs["T"]))
    assert Tv == T and x.shape == (128, 197, C)
    wd, bias2 = _prep_weights(
        *[np.asarray(inputs[k], dtype=np.float32) for k in
          ("w1", "b1", "cw", "cb", "w2", "b2", "ow1", "ob1", "ocw", "ocb", "ow2", "ob2")])

    in_maps = [_core_map(wd, bias2, x[core * 16:(core + 1) * 16])
               for core in range(8)]

    nc = _get_nc()
    res = run_bass_kernel_spmd(nc, in_maps, core_ids=list(range(8)))

    full = np.empty((128, 197, C), np.float32)
    for core in range(8):
        o = np.asarray(res.results[core]["out"]).astype(np.float32)
        full[core * 16:(core + 1) * 16, 0, :] = o[NPIX:NPIX + 16]
        full[core * 16:(core + 1) * 16, 1:, :] = o[:NPIX].reshape(16, 196, C)
    return full


# revision 89
# speedup vs baseline: 1.1296x; 1.0051x over previous
"""TRN2 Bass kernel for nn_Adapter (dense_cnn): ViT adapter with two branches
  main:   h1 = xs@w1+b1 ; y = dwconv3d_3x3x3(h1)+cb ; y@w2+b2
  offset: g = xs@ow1    ; d = tdiff(g) ; oc = dwconv_1x3x3(d)+bias' ; oc@ow2
  out = x with patch tokens += main + offset   (CLS rows pass through)

Data-parallel over 8 NeuronCores: 2 clips (16 frames) per core; adapter
weights replicated. Per-core kernel (raw bass, explicit semaphores):
  - fp8-e4m3 DoubleRow matmuls for m1 / depthwise conv (diagonal lhsT) / m2
  - 240-stride padded planes (15 rows x 16 cols); adjacent planes share the
    zero halo row, saving 1/16 of all PE conv streaming
  - m1 is j-major; the first w1 block + first x^T quarter ship as one
    contiguous "head" DMA so PE starts ~4.9us in, with p-state warmup
    matmuls during the DMA wait
  - evict work is spread across all three non-PE engines: DVE evicts
    off-branch m1 planes (gp), ACT evicts main m1 planes + all conv outputs
    (two planes per op); the frame diffs run wide (7 planes/op) mostly on
    GPSIMD, which also zero-fills every halo/guard via memsets
  - m2 is folded into the conv phase: each (c,tc) block's output tiles run
    interleaved between the next block's conv chunks (conv on psum banks
    0-3, tiles alternating bank pairs {4,5}/{6,7}); even tiles evict via
    DVE stt (+residual), odd tiles via ACT scaled-copy + DVE bf16 add
  - weights scaled x16/stage to keep fp8 out of subnormals; the final evict
    multiplies by 1/16^3 and adds the bf16 residual tokens
  - bf16 token stream + bf16 output (cast to f32 on host)

Self-contained: hardcodes shapes for x:[128,197,768], T=8 (asserts).
"""
import numpy as np
import ml_dtypes

import concourse.bass as bass
import concourse.mybir as mybir
from concourse.bass_utils import run_bass_kernel_spmd

F32 = mybir.dt.float32
BF16 = mybir.dt.bfloat16
F8 = mybir.dt.float8e4
AOT = mybir.AluOpType
AFT = mybir.ActivationFunctionType
DR = mybir.MatmulPerfMode.DoubleRow
BF = ml_dtypes.bfloat16
F8NP = ml_dtypes.float8_e4m3

# ---- problem constants (per core) ----
C = 768
CA = 384
T = 8
NPL = 240                 # padded plane stride: 15 rows x 16 cols
CLIPS = 2
NPIX_CLIP = T * 14 * 14
NPIX = CLIPS * NPIX_CLIP
KC = C // 128
NG = CA // 128
GROW = T * NPL            # one (g,c) row of unpadded-t planes (gp/diffp)
H1ROW = (T + 2) * NPL     # one (g,c) row incl t-guard planes (h1p)
H1PAD = NG * CLIPS * H1ROW
GPAD = NG * CLIPS * GROW
GF, GB = 32, 304          # front/back OOB guards for conv rhs over-reads
NTIL2 = (NPIX + 127) // 128
M1_CH = 392
OUT_ROWS = NPIX + 16
CVS = 16.0   # weight up-scale per stage; /CVS**3 folded into final evict

M_ORDER = [3, 4, 5, 0, 1, 2]          # m-block order (offs first); w1c is
                                      # stored column-permuted to match

# tap (dt, dh, dw) lists grouped by dw so DR pairs share dw (step % 16 == 0)
def _pairs(taps):
    by_dw = {}
    for tp in taps:
        by_dw.setdefault(tp[2], []).append(tp)
    prs = []
    for dw in sorted(by_dw):
        grp = by_dw[dw]
        for i in range(0, len(grp) - 1, 2):
            prs.append((grp[i], grp[i + 1]))
        if len(grp) % 2:
            prs.append((grp[-1], None))
    return prs

MAIN_TAPS = [(kd - 1, kh - 1, kw - 1)
             for kd in range(3) for kh in range(3) for kw in range(3)]
OFF_TAPS = [(0, kh - 1, kw - 1) for kh in range(3) for kw in range(3)]
MAIN_PAIRS = _pairs(MAIN_TAPS)   # 15 (12 pairs + 3 singles)
OFF_PAIRS = _pairs(OFF_TAPS)     # 6 (3 pairs + 3 singles)
NPR_MAIN = len(MAIN_PAIRS)
NPR_OFF = len(OFF_PAIRS)
NPR_TOT = (NPR_MAIN + NPR_OFF) * NG   # 63

# j-major; band 7 runs mains first so DVE's last off-evicts aren't needed
# by the earliest conv chunks' psum-bank reuse
M1_CHUNKS = ([(j, mi) for j in range(7) for mi in range(6)]
             + [(7, mi) for mi in (3, 4, 5, 0, 1, 2)])
CHUNK_IDX = {ch: q for q, ch in enumerate(M1_CHUNKS)}
M_POS = {m: mi for mi, m in enumerate(M_ORDER)}
CONV_CHUNKS = [(br, g, c, tc)
               for c in range(2) for tc in range(4)
               for br in (1, 0) for g in range(NG)]    # mains first per block
# diff(g,c) -> engine: Pool does most (it idles after memsets); DVE keeps
# (2,0) appended after its evict stream
DIFF_ENG = {(0, 0): "pl", (1, 0): "pl", (2, 0): "dve",
            (0, 1): "pl", (1, 1): "pl", (2, 1): "pl"}
N_M1 = len(M1_CHUNKS)      # 48
N_CONV = len(CONV_CHUNKS)  # 48

# m2 evict tile -> engine: even tiles DVE (stt w/ residual); odd tiles go
# ACT (scaled copy to bf16 staging) + DVE (2x residual add). Tile 21 runs
# DVE-direct so the end-of-kernel chain interleaves engines better.
M2E_DVE = {i: i % 2 == 0 for i in range(25)}
M2E_DVE[21] = True

# m2 tile i is unlocked once conv block (c,tc) covering its last token is
# evicted; PE interleaves each block's conv chunks with its unlocked tiles
def _tile_block(i):
    p_hi = (min(128 * (i + 1), NPIX) - 1) // 196
    c_hi, t_hi = divmod(p_hi, T)
    return c_hi * 4 + t_hi // 2

TILES_BY_BLOCK = [[] for _ in range(8)]
for _i in range(NTIL2):
    TILES_BY_BLOCK[_tile_block(_i)].append(_i)

# PE order: conv block b+1 runs while ACT evicts block b; block b's tiles
# are spread between block b+1's chunks (after chunks 1/3/5) so the 2-slot
# m2 psum rotation never outruns the evict engines
PE_SCHED = []
for _b in range(8):
    _tiles = TILES_BY_BLOCK[_b - 1] if _b >= 1 else []
    for _k in range(6):
        PE_SCHED.append(("cv", _b * 6 + _k))
        if _k % 2 == 1 and _tiles:
            PE_SCHED.append(("m2", _tiles.pop(0)))
    PE_SCHED += [("m2", _i) for _i in _tiles]
PE_SCHED += [("m2", _i) for _i in TILES_BY_BLOCK[7]]
TILES_BY_BLOCK = [[] for _ in range(8)]          # rebuild (popped above)
for _i in range(NTIL2):
    TILES_BY_BLOCK[_tile_block(_i)].append(_i)


def build(debug=False):
    nc = bass.Bass()
    xT = nc.declare_dram_parameter("xT", [C, NPIX], F8, isOutput=False)
    xtok = nc.declare_dram_parameter("xtok", [NPIX, C], BF16, isOutput=False)
    xcls = nc.declare_dram_parameter("xcls", [16, C], BF16, isOutput=False)
    head = nc.declare_dram_parameter("head", [128, KC * 128 + KC * 784], F8, isOutput=False)
    w1c = nc.declare_dram_parameter("w1c", [128, KC // 2 * 2 * C], F8, isOutput=False)
    w2c = nc.declare_dram_parameter("w2c", [128, KC // 2 * 2 * C], F8, isOutput=False)
    diag = nc.declare_dram_parameter("diag", [128, NPR_TOT * 2 * 128], F8, isOutput=False)
    b1c = nc.declare_dram_parameter("b1c", [128, KC], F32, isOutput=False)
    cbc = nc.declare_dram_parameter("cbc", [128, KC], F32, isOutput=False)
    out = nc.declare_dram_parameter("out", [OUT_ROWS, C], BF16, isOutput=True)
    if debug:
        dbg_h1 = nc.declare_dram_parameter("dbg_h1", [128, GF + H1PAD + GB], F8, isOutput=True)
        dbg_df = nc.declare_dram_parameter("dbg_df", [128, GF + GPAD + GB], F8, isOutput=True)
        dbg_cv = nc.declare_dram_parameter("dbg_cv", [128, KC * NPIX], F8, isOutput=True)

    xT_sb = nc.alloc_sbuf_tensor([128, KC * NPIX], F8)
    head_sb = nc.alloc_sbuf_tensor([128, KC * 128 + KC * 784], F8)  # w1 mblk0 | xT q0
    w1_sb = nc.alloc_sbuf_tensor([128, KC // 2 * 2 * C], F8)   # [pair][s][mblk]
    w2_sb = nc.alloc_sbuf_tensor([128, KC // 2 * 2 * C], F8)
    diag_sb = nc.alloc_sbuf_tensor([128, NPR_TOT * 2 * 128], F8)  # [pr][s][m]
    b1_sb = nc.alloc_sbuf_tensor([128, KC], F32)
    cb_sb = nc.alloc_sbuf_tensor([128, KC], F32)
    h1p = nc.alloc_sbuf_tensor([128, GF + H1PAD + GB], F8)
    gp = nc.alloc_sbuf_tensor([128, GPAD], F8)
    dfp = nc.alloc_sbuf_tensor([128, GF + GPAD + GB], F8)
    cvo = nc.alloc_sbuf_tensor([128, KC * NPIX], F8)
    xtk = nc.alloc_sbuf_tensor([128, NTIL2 * C], BF16)
    ost = nc.alloc_sbuf_tensor([128, 8 * C], BF16)
    tmp = nc.alloc_sbuf_tensor([128, 2 * C], BF16)   # odd-tile scaled psum
    warm = nc.alloc_sbuf_tensor([128, 512], F8)
    ps = nc.alloc_psum_tensor([128, 4096], F32)

    def h1_plane(g, c, tpad):
        return GF + (g * CLIPS + c) * H1ROW + tpad * NPL

    def g_plane(g, c, t):
        return (g * CLIPS + c) * GROW + t * NPL

    def df_plane(g, c, t):
        return GF + (g * CLIPS + c) * GROW + t * NPL

    def sv3(buf, ext, offset, dims):
        """3D free view [part + dims] of an sbuf tensor via explicit AP."""
        return bass.AP(buf, offset, [[ext, 128]] + [list(d) for d in dims])

    XT_EXT = KC * NPIX
    W_EXT = KC // 2 * 2 * C
    DG_EXT = NPR_TOT * 2 * 128
    H1_EXT = GF + H1PAD + GB
    GP_EXT = GPAD
    DF_EXT = GF + GPAD + GB
    CV_EXT = KC * NPIX

    # ---------- static evict/producer schedules ----------
    # DVE program positions (1-based sem thresholds after inc):
    #   per j-band: 3 off-evict ops (g=0,1,2); after bands 3 and 7: 3 diffs.
    dve_pos = {}
    pos = 0
    for j in range(8):
        for g in range(NG):
            pos += 1
            dve_pos[("ev", 3 + g, j)] = pos     # evict of m1 chunk (m=3+g, j)
    for gc, eng in DIFF_ENG.items():
        if eng == "dve":
            pos += 1
            dve_pos[("diff",) + gc] = pos
    DVE_PRE = pos                               # evicts+diffs before m2 evicts
    for kind, x in PE_SCHED:
        if kind != "m2":
            continue
        pos += 1
        # even: stt evict from psum; odd: bf16 residual add from tmp
        dve_pos[("m2e", x) if M2E_DVE[x] else ("m2o", x)] = pos
    N_PL_MS = 13                                # memset count on Pool (below)
    pl_pos = {}
    pos = N_PL_MS
    for gc, eng in DIFF_ENG.items():
        if eng == "pl":
            pos += 1
            pl_pos[("diff",) + gc] = pos

    def diff_wait(g, c):
        if DIFF_ENG[(g, c)] == "dve":
            return ("dve", dve_pos[("diff", g, c)])
        return ("pl", pl_pos[("diff", g, c)])

    # ACT program positions: 24 main m1 evicts (j-major, m inner), then the
    # PE_SCHED-ordered conv evicts + odd m2-tile evicts
    act_pos = {}
    pos = 0
    for j in range(8):
        for m in range(3):
            pos += 1
            act_pos[("ev", m, j)] = pos
    for kind, x in PE_SCHED:
        if kind == "cv":
            pos += 1
            act_pos[("cv", x)] = pos
        elif not M2E_DVE[x]:
            pos += 1
            act_pos[("m2e", x)] = pos

    def m1_evict_wait(q):
        """(sem_name, thr) for 'm1 chunk q's psum bank has been evicted'."""
        j, mi = M1_CHUNKS[q]
        m = M_ORDER[mi]
        if m >= 3:
            return ("dve", dve_pos[("ev", m, j)])
        return ("act", act_pos[("ev", m, j)])

    # PE completion positions on s_pe (interleaved conv blocks + m2 tiles)
    pe_pos = {}
    pos = N_M1
    for kind, x in PE_SCHED:
        pos += 1
        pe_pos[(kind, x)] = pos

    # m2 tile psum slots: {4,5}/{6,7} alternating; the last block's four
    # tiles fan out over all four bank-pairs (conv banks are free by then)
    PB = {i: (4 + 2 * (i % 2)) * 512 for i in range(NTIL2)}
    b7 = TILES_BY_BLOCK[7]
    PB[b7[0]], PB[b7[1]], PB[b7[2]], PB[b7[3]] = 0, 1024, 3072, 2048
    TILE_BW = {                     # custom bank-free waits for those tiles
        b7[0]: ("act", ("cv", 45)),
        b7[1]: ("act", ("cv", 47)),
        b7[2]: ("act", ("m2e", 19)),
        b7[3]: ("dve", ("m2e", 20)),
    }

    def main_data_thr(g, c, tc):
        """ACT threshold: h1 planes t<=2tc+1 of (g,c) evicted."""
        j = c * 4 + min(tc + 1, 3)
        return act_pos[("ev", g, j)]

    M2_THR = []
    for i in range(NTIL2):
        p_hi = (min(128 * (i + 1), NPIX) - 1) // 196
        c_hi, t_hi = divmod(p_hi, T)
        blocks = c_hi * 4 + t_hi // 2       # completed (c,tc) blocks before
        M2_THR.append(act_pos[("cv", blocks * 6 + 5)])

    from contextlib import ExitStack
    _sems = ExitStack()
    xk = [_sems.enter_context(nc.semaphore(f"s_xk{i}")) for i in range(8)]
    ot = [_sems.enter_context(nc.semaphore(f"s_ot{i}")) for i in range(8)]
    s_xt = [_sems.enter_context(nc.semaphore(f"s_xt{i}")) for i in range(4)]
    with (
        _sems,
        nc.Block(no_gpsimd_drain=True) as block,
        nc.semaphore("s_ld") as s_ld,
        nc.semaphore("s_w1a") as s_w1a,
        nc.semaphore("s_w1b") as s_w1b,
        nc.semaphore("s_dgo") as s_dgo,
        nc.semaphore("s_dgm") as s_dgm,
        nc.semaphore("s_w2") as s_w2,
        nc.semaphore("s_pe") as s_pe,
        nc.semaphore("s_act") as s_act,
        nc.semaphore("s_dve") as s_dve,
        nc.semaphore("s_pl") as s_pl,
        nc.semaphore("s_cls") as s_cls,
        nc.semaphore("s_dbg") as s_dbg,
    ):
        # ================= SP: all DMA (serial, need-ordered) =================
        @block.sync
        def _(sync):
            w1v = w1_sb[:].rearrange("p (q m) -> p q m", q=KC)    # q=(pr,s)
            w1d = w1c[:].rearrange("p (q m) -> p q m", q=KC)
            sync.dma_start(out=head_sb[:], in_=head[:]).then_inc(s_w1a, 16)
            xtv = xT_sb[:].rearrange("p (k n) -> p k n", k=KC)
            xtd = xT[:].rearrange("(k p) n -> p k n", p=128)
            sync.dma_start(out=w1v[:, :, 128:768], in_=w1d[:, :, 128:768]
                           ).then_inc(s_w1b, 16)
            sync.dma_start(out=b1_sb[:], in_=b1c[:]).then_inc(s_ld, 16)
            sync.dma_start(out=cb_sb[:], in_=cbc[:]).then_inc(s_ld, 16)
            for qq in (1, 2, 3):
                sync.dma_start(out=xtv[:, :, qq * 784:(qq + 1) * 784],
                               in_=xtd[:, :, qq * 784:(qq + 1) * 784]
                               ).then_inc(s_xt[qq], 16)
            doff = NPR_MAIN * NG * 256
            sync.dma_start(out=diag_sb[:, doff:], in_=diag[:, doff:]
                           ).then_inc(s_dgo, 16)
            for g in range(NG):      # main diag, grouped by g (host layout)
                lo, hi = g * NPR_MAIN * 256, (g + 1) * NPR_MAIN * 256
                sync.dma_start(out=diag_sb[:, lo:hi], in_=diag[:, lo:hi]
                               ).then_inc(s_dgm, 16)
            sync.dma_start(out=w2_sb[:], in_=w2c[:]).then_inc(s_w2, 16)
            sync.dma_start(out=out[NPIX:OUT_ROWS, :], in_=xcls[:]).then_inc(s_cls, 16)
            if debug:
                sync.wait_ge(s_act, act_pos[("ev", 2, 7)])
                sync.wait_ge(s_dve, DVE_PRE)
                sync.dma_start(out=dbg_h1[:], in_=h1p[:]).then_inc(s_dbg, 16)
                sync.dma_start(out=dbg_df[:], in_=dfp[:]).then_inc(s_dbg, 16)
                sync.wait_ge(s_act, act_pos[("cv", N_CONV - 1)])
                sync.dma_start(out=dbg_cv[:], in_=cvo[:]).then_inc(s_dbg, 16)

            def load_pair(pj):
                if pj < 12:
                    j = 2 * pj
                    sync.dma_start(
                        out=xtk[:, j * C:(j + 2) * C].rearrange("p (b c) -> p b c", b=2),
                        in_=xtok[j * 128:(j + 2) * 128, :].rearrange("(b r) c -> r b c", b=2),
                    ).then_inc(xk[pj % 8], 16)
                else:
                    sync.dma_start(out=xtk[:64, bass.ts(24, C)],
                                   in_=xtok[24 * 128:NPIX, :]).then_inc(xk[12 % 8], 16)

            # xtok pair-loads interleaved with out stores: pairs arrive two
            # blocks ahead of the tiles that read them
            seen_pairs = set()

            def pairs_for(b):
                want = sorted({i // 2 for i in TILES_BY_BLOCK[b]} - seen_pairs)
                for pj in want:
                    seen_pairs.add(pj)
                    load_pair(pj)

            pairs_for(0)
            pairs_for(1)
            done_b = -1
            for kind, x in PE_SCHED:
                if kind != "m2":
                    continue
                i = x
                b = _tile_block(i)
                if b > done_b:                  # prefetch two blocks ahead
                    done_b = b
                    if b + 2 < 8:
                        pairs_for(b + 2)
                rows = min(128, NPIX - i * 128)
                sync.wait_ge(s_dve, dve_pos[("m2e" if M2E_DVE[i] else "m2o", i)])
                sync.dma_start(out=out[i * 128:i * 128 + rows, :],
                               in_=ost[:rows, bass.ts(i % 8, C)]
                               ).then_inc(ot[i % 8], 16)
            if debug:
                sync.wait_ge(s_dbg, 48)

        # ================= Pool: halo/guard memsets, then m2-evict share ======
        @block.gpsimd
        def _(gpsimd):
            ms = [
                # gp halos: row 0 of each plane + cols 0/15 of the 14 data rows
                bass.AP(gp, 0, [[GP_EXT, 128], [NPL, 48], [1, 16]]),
                bass.AP(gp, 16, [[GP_EXT, 128], [NPL, 48], [16, 14]]),
                bass.AP(gp, 31, [[GP_EXT, 128], [NPL, 48], [16, 14]]),
                # (trailing halo row of each (g,c) row is the next row's
                # plane-0 row-0, zeroed above; diff output inherits zeros)
                # dfp: front/back OOB guards + plane-0 of each (g,c) row
                bass.AP(dfp, 0, [[DF_EXT, 128], [1, GF]]),
                bass.AP(dfp, GF + GPAD, [[DF_EXT, 128], [1, GB]]),
                bass.AP(dfp, GF, [[DF_EXT, 128], [GROW, 6], [1, NPL]]),
                # h1p: front/back guards, t-guard planes (tpad 0/9), halos
                bass.AP(h1p, 0, [[H1_EXT, 128], [1, GF]]),
                bass.AP(h1p, GF + H1PAD, [[H1_EXT, 128], [1, GB]]),
                bass.AP(h1p, GF, [[H1_EXT, 128], [H1ROW, 6], [1, NPL]]),
                bass.AP(h1p, GF + 9 * NPL, [[H1_EXT, 128], [H1ROW, 6], [1, NPL]]),
                bass.AP(h1p, GF, [[H1_EXT, 128], [NPL, 60], [1, 16]]),
                bass.AP(h1p, GF + 16, [[H1_EXT, 128], [NPL, 60], [16, 14]]),
                bass.AP(h1p, GF + 31, [[H1_EXT, 128], [NPL, 60], [16, 14]]),
            ]
            assert len(ms) == N_PL_MS, len(ms)
            for m in ms:
                gpsimd.memset(m, 0.0).then_inc(s_pl, 1)
            # frame-diff share: each waits the gp evicts it reads (t planes
            # of (g,c) are complete once band 3 (c=0) / 7 (c=1) evict g lands)
            for (g, c), eng in DIFF_ENG.items():
                if eng != "pl":
                    continue
                gpsimd.wait_ge(s_dve, dve_pos[("ev", 3 + g, 4 * c + 3)])
                a = g_plane(g, c, 1)
                b = g_plane(g, c, 0)
                d = df_plane(g, c, 1)
                gpsimd.tensor_tensor(
                    dfp[:, d:d + 7 * NPL],
                    gp[:, a:a + 7 * NPL], gp[:, b:b + 7 * NPL],
                    op=AOT.subtract).then_inc(s_pl, 1)

        # ================= PE =================
        @block.tensor
        def _(tensor):
            # p-state warmup: throwaway matmuls during the initial DMA wait
            # so real work starts at full clock (warm is never written; the
            # interp zero-fills SBUF)
            for _ in range(11):
                tensor.matmul(ps[:, 0:512],
                              bass.AP(warm, 0, [[512, 128], [1, 128]]),
                              bass.AP(warm, 0, [[512, 128], [1, 512]]),
                              start=True, stop=True, skip_group_check=True)
            tensor.wait_ge(s_w1a, 16)
            # ---- matmul1 (DR), j-major, banks 0..7 rotating ----
            for q, (j, mi) in enumerate(M1_CHUNKS):
                bank = q % 8
                if q == 1:
                    tensor.wait_ge(s_w1b, 16)
                if j in (2, 4, 6) and mi == 0:
                    tensor.wait_ge(s_xt[j // 2], 16)
                if q >= 8:
                    sem, thr = m1_evict_wait(q - 8)
                    tensor.wait_ge(s_dve if sem == "dve" else s_act, thr)
                pv = ps[:, bank * 512: bank * 512 + M1_CH]
                HD_EXT = KC * 128 + KC * 784
                for pr in range(KC // 2):
                    if mi == 0:
                        lhsT = sv3(head_sb, HD_EXT, pr * 256, [(128, 2), (1, 128)])
                    else:
                        lhsT = sv3(w1_sb, W_EXT, pr * 2 * C + mi * 128,
                                   [(C, 2), (1, 128)])
                    if j < 2:
                        rhs = sv3(head_sb, HD_EXT,
                                  KC * 128 + (pr * 2) * 784 + j * M1_CH,
                                  [(784, 2), (1, M1_CH)])
                    else:
                        rhs = sv3(xT_sb, XT_EXT, (pr * 2) * NPIX + j * M1_CH,
                                  [(NPIX, 2), (1, M1_CH)])
                    mm = tensor.matmul(pv, lhsT, rhs, perf_mode=DR,
                                       start=(pr == 0), stop=(pr == KC // 2 - 1))
                mm.then_inc(s_pe, 1)
            # ---- conv (banks 0..3) interleaved with m2 tiles (banks 4..7) ----
            tensor.wait_ge(s_dgo, 16)
            tensor.wait_ge(s_pl, N_PL_MS)
            dgm_seen = 0
            w2_seen = False
            for kind, x in PE_SCHED:
                if kind == "cv":
                    qc = x
                    br, g, c, tc = CONV_CHUNKS[qc]
                    bank = qc % 4
                    if br and 16 * (g + 1) > dgm_seen:
                        dgm_seen = 16 * (g + 1)
                        tensor.wait_ge(s_dgm, dgm_seen)
                    if qc < 4:
                        sem, thr = m1_evict_wait(40 + qc)
                        tensor.wait_ge(s_dve if sem == "dve" else s_act, thr)
                    else:
                        tensor.wait_ge(s_act, act_pos[("cv", qc - 4)])
                    if br:
                        tensor.wait_ge(s_act, main_data_thr(g, c, tc))
                    else:
                        sem, thr = diff_wait(g, c)
                        tensor.wait_ge(s_dve if sem == "dve" else s_pl, thr)
                    pairs = MAIN_PAIRS if br else OFF_PAIRS
                    pv = ps[:, bank * 512: bank * 512 + 480]
                    for ip, (tA, tB) in enumerate(pairs):
                        dtA, dhA, dwA = tA
                        if br:
                            offA = h1_plane(g, c, 2 * tc + 1 + dtA) + dhA * 16 + dwA
                            buf, ext = h1p, H1_EXT
                        else:
                            offA = df_plane(g, c, 2 * tc + dtA) + dhA * 16 + dwA
                            buf, ext = dfp, DF_EXT
                        if tB is None:
                            sstep = 16
                        else:
                            dtB, dhB, dwB = tB
                            sstep = (dtB - dtA) * NPL + (dhB - dhA) * 16
                        plane = (g * NPR_MAIN + ip) if br else (NPR_MAIN * NG + ip * NG + g)
                        lhsT = sv3(diag_sb, DG_EXT, plane * 256,
                                   [(128, 2), (1, 128)])
                        rhs = sv3(buf, ext, offA, [(sstep, 2), (1, 480)])
                        mm = tensor.matmul(pv, lhsT, rhs, perf_mode=DR,
                                           start=(ip == 0), stop=(ip == len(pairs) - 1),
                                           skip_group_check=True)
                    mm.then_inc(s_pe, 1)
                else:
                    i = x
                    if not w2_seen:
                        tensor.wait_ge(s_w2, 16)
                        w2_seen = True
                    rows = min(128, NPIX - i * 128)
                    odd = i % 2 == 1
                    tensor.wait_ge(s_act, M2_THR[i])
                    if i in TILE_BW:
                        sem, key = TILE_BW[i]
                        if sem == "act":
                            tensor.wait_ge(s_act, act_pos[key])
                        else:
                            tensor.wait_ge(s_dve, dve_pos[key])
                    elif i < 2:
                        for qq in (44 + 2 * i, 45 + 2 * i):
                            sem, thr = m1_evict_wait(qq)
                            tensor.wait_ge(s_dve if sem == "dve" else s_act, thr)
                    elif M2E_DVE[i - 2]:
                        tensor.wait_ge(s_dve, dve_pos[("m2e", i - 2)])
                    else:
                        tensor.wait_ge(s_act, act_pos[("m2e", i - 2)])
                    pv = ps[:rows, PB[i]:PB[i] + 768]
                    for pr in range(KC // 2):
                        last = pr == KC // 2 - 1
                        lhsT = sv3(cvo, CV_EXT, (pr * 2) * NPIX + i * 128,
                                   [(NPIX, 2), (1, rows)])
                        tensor.matmul(pv[:, 0:512], lhsT,
                                      sv3(w2_sb, W_EXT, pr * 2 * C, [(C, 2), (1, 512)]),
                                      perf_mode=DR,
                                      start=(pr == 0), stop=last,
                                      skip_group_check=True)
                        mm1 = tensor.matmul(pv[:, 512:768], lhsT,
                                            sv3(w2_sb, W_EXT, pr * 2 * C + 512,
                                                [(C, 2), (1, 256)]),
                                            perf_mode=DR,
                                            start=(pr == 0), stop=last,
                                            skip_group_check=True)
                    mm1.then_inc(s_pe, 1)

        # ================= ACT: main m1 evicts + all conv evicts =============
        @block.scalar
        def _(scalar):
            scalar.wait_ge(s_ld, 32)
            for j in range(8):
                c, t0 = divmod(2 * j, T)
                for m in range(3):
                    q = CHUNK_IDX[(j, M_POS[m])]   # main m's chunk in band j
                    bank = q % 8
                    scalar.wait_ge(s_pe, q + 1)
                    src = sv3(ps, 4096, bank * 512, [(196, 2), (14, 14), (1, 14)])
                    dst = sv3(h1p, H1_EXT,
                              h1_plane(m, c, t0 + 1) + 17,
                              [(NPL, 2), (16, 14), (1, 14)])
                    scalar.activation(dst, src, AFT.Identity,
                                      bias=b1_sb[:, m:m + 1]).then_inc(s_act, 1)
            for kind, x in PE_SCHED:
                if kind == "cv":
                    qc = x
                    br, g, c, tc = CONV_CHUNKS[qc]
                    bank = qc % 4
                    scalar.wait_ge(s_pe, pe_pos[("cv", qc)])
                    grp = g if br else 3 + g
                    src = sv3(ps, 4096, bank * 512 + 17,
                              [(NPL, 2), (16, 14), (1, 14)])
                    dst = sv3(cvo, CV_EXT,
                              grp * NPIX + c * NPIX_CLIP + 2 * tc * 196,
                              [(196, 2), (14, 14), (1, 14)])
                    scalar.activation(dst, src, AFT.Identity,
                                      bias=cb_sb[:, grp:grp + 1]).then_inc(s_act, 1)
                elif not M2E_DVE[x]:
                    # odd m2-tile psum evict: scaled copy to bf16 staging
                    # (DVE adds the residual at 2x)
                    i = x
                    rows = min(128, NPIX - i * 128)
                    scalar.wait_ge(s_pe, pe_pos[("m2", i)])
                    if i >= 4:
                        scalar.wait_ge(s_dve, dve_pos[("m2o", i - 4)])
                    scalar.activation(
                        tmp[:rows, bass.ts((i // 2) % 2, C)],
                        ps[:rows, PB[i]:PB[i] + 768],
                        AFT.Identity, scale=1.0 / (CVS ** 3)).then_inc(s_act, 1)

        # ================= DVE: off m1 evicts + diffs + m2-evict share =======
        @block.vector
        def _(vector):
            for j in range(8):
                c, t0 = divmod(2 * j, T)
                for g in range(NG):
                    q = CHUNK_IDX[(j, g)]      # chunk (j, mi=g) => m=3+g
                    bank = q % 8
                    vector.wait_ge(s_pe, q + 1)
                    src = sv3(ps, 4096, bank * 512, [(196, 2), (14, 14), (1, 14)])
                    dst = sv3(gp, GP_EXT, g_plane(g, c, t0) + 17,
                              [(NPL, 2), (16, 14), (1, 14)])
                    vector.tensor_copy(dst, src).then_inc(s_dve, 1)
            for (g, c), eng in DIFF_ENG.items():      # DVE's diff share
                if eng != "dve":
                    continue
                vector.wait_ge(s_pl, 3)               # gp halos zeroed
                a = g_plane(g, c, 1)
                b = g_plane(g, c, 0)
                d = df_plane(g, c, 1)
                vector.tensor_tensor(
                    dfp[:, d:d + 7 * NPL],
                    gp[:, a:a + 7 * NPL], gp[:, b:b + 7 * NPL],
                    op=AOT.subtract).then_inc(s_dve, 1)
            # ---- m2 evict + residual: even tiles stt from psum, odd tiles
            # bf16 add of ACT's scaled copy (2x DVE rate) ----
            for kind, x in PE_SCHED:
                if kind != "m2":
                    continue
                i = x
                rows = min(128, NPIX - i * 128)
                pj = i // 2
                vector.wait_ge(xk[pj % 8], 16 * (pj // 8 + 1))
                if i >= 8:
                    jj = i - 8
                    vector.wait_ge(ot[jj % 8], 16 * (jj // 8 + 1))
                if M2E_DVE[i]:
                    vector.wait_ge(s_pe, pe_pos[("m2", i)])
                    vector.scalar_tensor_tensor(
                        ost[:rows, bass.ts(i % 8, C)],
                        ps[:rows, PB[i]:PB[i] + 768],
                        1.0 / (CVS ** 3),
                        xtk[:rows, bass.ts(i, C)],
                        op0=AOT.mult, op1=AOT.add).then_inc(s_dve, 1)
                else:
                    vector.wait_ge(s_act, act_pos[("m2e", i)])
                    vector.tensor_tensor(
                        ost[:rows, bass.ts(i % 8, C)],
                        tmp[:rows, bass.ts((i // 2) % 2, C)],
                        xtk[:rows, bass.ts(i, C)],
                        op=AOT.add).then_inc(s_dve, 1)

    return nc


# ---------------- host side ----------------
_NC_CACHE = {}


def _get_nc():
    if "nc" not in _NC_CACHE:
        _NC_CACHE["nc"] = build()
    return _NC_CACHE["nc"]


def _dr_pack(W):
    """[768(k), M] -> per-partition DR layout [128(ki), pair, s, M] flattened."""
    M = W.shape[1]
    out = np.zeros((128, KC // 2, 2, M), np.float32)
    for pr in range(KC // 2):
        for s in range(2):
            out[:, pr, s, :] = W[pr * 256 + s * 128: pr * 256 + (s + 1) * 128, :]
    return out.reshape(128, KC // 2 * 2 * M)


def _prep_weights(w1, b1, cw, cb, w2, b2, ow1, ob1, ocw, ocb, ow2, ob2):
    w1cat = np.hstack([w1, ow1]) * CVS          # [768, 768], col m-blocks
    perm = np.concatenate([np.arange(m * 128, (m + 1) * 128) for m in M_ORDER])
    w1c = _dr_pack(w1cat[:, perm]).astype(F8NP)
    w2c = _dr_pack(np.vstack([w2, ow2]) * CVS).astype(F8NP)
    # diag DR pairs: [128(ki), pr_tot, s, 128(m)] with diagonal per s
    diag = np.zeros((128, NPR_TOT, 2, 128), np.float32)
    eye = np.eye(128, dtype=bool)

    def tapw(w_, tp, main):
        dt, dh, dw = tp
        if main:
            return w_[:, 0, dt + 1, dh + 1, dw + 1]
        return w_[:, 0, 0, dh + 1, dw + 1]

    for br, (pairs, w_) in enumerate([(MAIN_PAIRS, cw), (OFF_PAIRS, ocw)]):
        for ip, (tA, tB) in enumerate(pairs):
            for g in range(NG):
                # main planes grouped by g (contiguous per-g DMA pieces)
                pi = (g * NPR_MAIN + ip) if br == 0 else (NPR_MAIN * NG + ip * NG + g)
                vA = tapw(w_, tA, br == 0) * CVS
                diag[:, pi, 0, :][eye] = vA[g * 128:(g + 1) * 128]
                if tB is not None:
                    vB = tapw(w_, tB, br == 0) * CVS
                    diag[:, pi, 1, :][eye] = vB[g * 128:(g + 1) * 128]
    b1cv = np.ascontiguousarray(
        (np.concatenate([b1, ob1]) * CVS).reshape(KC, 128).T).astype(np.float32)
    # off-branch ob1 folded through the (linear) dwconv into its evict bias:
    # conv(diff + ob1) = conv(diff) + ob1 * sum(ocw taps)
    ocb_f = ocb + ob1 * ocw.sum(axis=(1, 2, 3, 4))
    cbcv = np.ascontiguousarray(
        (np.concatenate([cb, ocb_f]) * CVS * CVS).reshape(KC, 128).T).astype(np.float32)
    bias2 = (b2 + ob2).astype(np.float32)
    return dict(w1c=w1c, w2c=w2c,
                diag=diag.reshape(128, NPR_TOT * 2 * 128).astype(F8NP),
                b1c=b1cv, cbc=cbcv), bias2


def _core_map(wd, bias2, xs):
    """Per-core input map from shared weights + this core's 16 BT rows."""
    xpat = np.ascontiguousarray(xs[:, 1:, :]).reshape(NPIX, C)
    m = dict(wd)
    xT8 = np.ascontiguousarray(xpat.T).astype(F8NP)
    m["xT"] = xT8
    hd = np.empty((128, KC * 128 + KC * 784), F8NP)
    for q in range(KC):
        hd[:, q * 128:(q + 1) * 128] = wd["w1c"][:, q * 768:q * 768 + 128]
    for k in range(KC):
        hd[:, KC * 128 + k * 784: KC * 128 + (k + 1) * 784] = \
            xT8[k * 128:(k + 1) * 128, 0:784]
    m["head"] = hd
    m["xtok"] = (xpat + bias2).astype(BF)
    m["xcls"] = np.ascontiguousarray(xs[:, 0, :]).astype(BF)
    return m


def kernel(**inputs):
    x = np.asarray(inputs["x"], dtype=np.float32)
    Tv = int(np.asarray(inputs["T"]))
    assert Tv == T and x.shape == (128, 197, C)
    wd, bias2 = _prep_weights(
        *[np.asarray(inputs[k], dtype=np.float32) for k in
          ("w1", "b1", "cw", "cb", "w2", "b2", "ow1", "ob1", "ocw", "ocb", "ow2", "ob2")])

    in_maps = [_core_map(wd, bias2, x[core * 16:(core + 1) * 16])
               for core in range(8)]

    nc = _get_nc()
    res = run_bass_kernel_spmd(nc, in_maps, core_ids=list(range(8)))

    full = np.empty((128, 197, C), np.float32)
    for core in range(8):
        o = np.asarray(res.results[core]["out"]).astype(np.float32)
        full[core * 16:(core + 1) * 16, 0, :] = o[NPIX:NPIX + 16]
        full[core * 16:(core + 1) * 16, 1:, :] = o[:NPIX].reshape(16, 196, C)
    return full
